# revision 1
# baseline (speedup 1.0000x reference)
"""Trainium2 Bass kernel for nn_AttentionFusion (conv reducer + text cross
attention + text-modulated visual self attention).

Sharding: pure data parallelism over batch (32 samples -> 8 cores x 4
samples), weights replicated.  Per sample everything runs in
channels-on-partitions layout:

  conv1 (264->128, 3x3)  : 9-tap PSUM-accumulated matmuls from a zero-padded
                           [128,34,34] SBUF image; the 8 spatial channels are
                           host-im2col'ed into one K=72 matmul
  GN + ReLU              : bn_stats + group combine via tiny selector matmuls,
                           applied as one ACT op (relu(x*scale+bias), per-
                           partition scale/bias APs)
  conv2 (128->256, 3x3)  : 9-tap matmuls from a persistent zero-padded tile
  vsr^T  [300,1024]      : reduce_w^T matmul (bias folded via ones row of the
                           coord chunk)
  text attention         : scoreT [20,n] -> exp -> wt^T = txtT_aug @ E / D
                           (softmax denominator via ones column of txtT_aug)
  k^T,q^T [258,1024]     : linear, mod multiply folded into PSUM evacuation
  v      [1024,259]      : linear with ones column for the denominator
  S'=score^T [m,n]       : qT/kT matmuls; softmax over partitions via exp +
                           v-ones-column denominator (no max subtraction:
                           logits are O(1) at this problem's scale)
  wv^T   [258,1024]      : v^T @ E' accumulated over m-chunks, normalized by
                           1/D (exp(-ln D)) broadcast with a ones matmul

Big matmuls run as float32r (1 cycle/row at N>=256, ~1.5e-4 rel err measured
on hw); small-N matmuls (stats/mod/broadcast) stay fp32.
"""

import os
import sys
from contextlib import ExitStack

for _p in ("/opt/trn_rl_repo",):
    if _p not in sys.path and os.path.isdir(_p):
        sys.path.insert(0, _p)

import numpy as np

import concourse.bacc as bacc
import concourse.mybir as mybir
import concourse.tile as tile
from concourse.bass import ts
from concourse.bass_utils import run_bass_kernel_spmd

F32 = mybir.dt.float32
F32R = mybir.dt.float32r
AF = mybir.ActivationFunctionType
ALU = mybir.AluOpType

N_CORES = 8
SPC = 4  # samples per core
HW = 1024
DS = 300
VF = 258
EPS = 1e-5
RS_TXT = 1.0 / float(np.sqrt(DS))
RS_VID = 1.0 / float(np.sqrt(VF))

TAPS = [(ty, tx) for ty in range(3) for tx in range(3)]

# matmul operand dtype for the big matmuls (float32r: 4x faster, ~1.5e-4 rel)
DTM = F32R if os.environ.get("KERNEL_F32R", "1") != "0" else F32

_PROGRAM_CACHE = {}

KCH = (128, 128, 3)  # feature K chunks: conv2-out x2, coord+ones
DSCH = (128, 128, 44)  # 300-dim chunks


def _patch_act_tables():
    """The act-table-load pass picks the first set containing each function,
    which thrashes between exp_and_others and natural_log for this kernel's
    Exp/Ln/Relu/Identity/Copy mix (57 table loads, ~73us).  All five live in
    natural_log_exp_and_others, so hide them from every other set (keeping
    dict order so act_func_set_id indices stay valid)."""
    import concourse.bacc as _bacc
    import concourse.hw_specs as _hw

    if getattr(_bacc, "_act_tables_patched", False):
        return
    _orig = _hw.get_activation_tables
    mine = {AF.Exp, AF.Ln, AF.Relu, AF.Identity, AF.Copy}

    def patched(module_arch):
        tabs = _orig(module_arch)
        out = {}
        for name, funcs in tabs.items():
            if name == "natural_log_exp_and_others" or not (mine & funcs):
                out[name] = funcs
            else:
                out[name] = funcs - mine
        return out

    _bacc.get_activation_tables = patched
    _bacc._act_tables_patched = True


def _patch_drain_barrier():
    """Split the kernel-tail drain's per-proc sem waits across engines (walrus
    allows 1 sync wait per drain; serial SP drains cost ~250ns each)."""
    import concourse.tile as tile_mod
    from concourse.vector_clock import ScopedClock

    if getattr(tile_mod, "_drain_patched", False):
        return

    def _patched(self, tick_clock, wait_clock):
        nc = self.nc
        drain_inst = nc.sync.drain()
        wait_clock.add_sem_waits(
            drain_inst.ins, ScopedClock({None: tick_clock.global_clock})
        )
        si = drain_inst.ins.sync_info
        waits = list(si.on_wait or [])
        if len(waits) > 1:
            si.on_wait = waits[:1]
            engines = [nc.sync, nc.scalar, nc.vector, nc.tensor, nc.gpsimd]
            for i in range(1, len(waits)):
                extra = engines[i % len(engines)].drain()
                extra.ins.sync_info = mybir.SyncInfo(
                    on_wait=[waits[i]], on_update=[]
                )
        nc.all_engine_barrier()
        assert self.sems is not None
        popped = nc._tile_sem_poison_stack.pop()
        assert popped is self._sem_poison
        nc.clear_and_free_semaphores(list(self.sems.allocated().values()))
        nc.all_engine_barrier()

    tile_mod.TileContext._drain_and_barrier = _patched
    tile_mod._drain_patched = True


def build_program():
    _patch_act_tables()
    _patch_drain_barrier()
    nc = bacc.Bacc()
    dt = F32

    # ---------------- DRAM declarations ----------------
    d_xv = nc.dram_tensor("xv", [SPC, 256, 34 * 34], DTM, kind="ExternalInput")
    d_xsp = nc.dram_tensor("xsp", [SPC, 72, HW], DTM, kind="ExternalInput")
    d_txt = nc.dram_tensor("txt", [SPC, 128, 60], DTM, kind="ExternalInput")
    d_txtT = nc.dram_tensor("txtT", [SPC, 20, DS + 1], DTM, kind="ExternalInput")
    d_w1v = nc.dram_tensor("w1v", [128, 18 * 128], DTM, kind="ExternalInput")
    d_w1s = nc.dram_tensor("w1s", [72, 128], DTM, kind="ExternalInput")
    d_w2 = nc.dram_tensor("w2", [128, 9 * 256], DTM, kind="ExternalInput")
    d_rw = nc.dram_tensor("rw", [VF + 1, DS], DTM, kind="ExternalInput")
    d_kw = nc.dram_tensor("kw", [VF + 1, VF], DTM, kind="ExternalInput")
    d_qw = nc.dram_tensor("qw", [VF + 1, VF], DTM, kind="ExternalInput")
    d_vw = nc.dram_tensor("vw", [VF + 1, VF + 2], DTM, kind="ExternalInput")
    d_incw = nc.dram_tensor("incw", [128, 3 * VF], dt, kind="ExternalInput")
    d_coord = nc.dram_tensor("coord3", [3, HW], DTM, kind="ExternalInput")
    d_gs1 = nc.dram_tensor("gsel1", [128, 32], dt, kind="ExternalInput")
    d_gs1T = nc.dram_tensor("gsel1T", [32, 128], dt, kind="ExternalInput")
    d_gs2 = nc.dram_tensor("gsel2", [128, 64], dt, kind="ExternalInput")
    d_gs2T = nc.dram_tensor("gsel2T", [32, 256], dt, kind="ExternalInput")
    d_ones = nc.dram_tensor("ones1", [1, 128], DTM, kind="ExternalInput")
    d_ones20 = nc.dram_tensor("ones20", [20, 128], DTM, kind="ExternalInput")
    d_ones128 = nc.dram_tensor("ones128", [128, 128], DTM, kind="ExternalInput")
    d_sv1 = nc.dram_tensor("svec1", [128, 6], dt, kind="ExternalInput")
    d_sv2 = nc.dram_tensor("svec2", [128, 6], dt, kind="ExternalInput")
    d_zpad = nc.dram_tensor("zpad", [128, 34 * 34], DTM, kind="ExternalInput")
    d_out = nc.dram_tensor("out", [SPC, 558, HW], dt, kind="ExternalOutput")

    with tile.TileContext(nc) as tc, ExitStack() as ctx:
        wpool = ctx.enter_context(tc.tile_pool(name="weights", bufs=1))
        inpool = ctx.enter_context(tc.tile_pool(name="inputs", bufs=2))
        spool = ctx.enter_context(tc.tile_pool(name="work", bufs=1))
        opool = ctx.enter_context(tc.tile_pool(name="outs", bufs=1))
        epool = ctx.enter_context(tc.tile_pool(name="estream", bufs=3))
        pacc = ctx.enter_context(tc.tile_pool(name="pacc", bufs=3, space="PSUM"))
        pconv = ctx.enter_context(tc.tile_pool(name="pconv", bufs=2, space="PSUM"))
        pstream = ctx.enter_context(tc.tile_pool(name="pstream", bufs=2, space="PSUM"))
        psmall = ctx.enter_context(tc.tile_pool(name="psmall", bufs=1, space="PSUM"))

        # ------------- mod deps + conv1 weights first -------------
        sv1 = wpool.tile([128, 6], dt, name="sv1")
        nc.gpsimd.dma_start(sv1[:, :], d_sv1[:, :])
        b1_sb, g1_sb, bt1_sb = sv1[:, 0:1], sv1[:, 1:2], sv1[:, 2:3]
        incb_sb = sv1[:, 3:6]
        incw_sb3 = wpool.tile([128, 3, VF], dt, name="incw_sb3")
        nc.gpsimd.dma_start(
            incw_sb3[:, :, :].rearrange("p a b -> p (a b)"), d_incw[:, :]
        )
        incw_sb = [incw_sb3[:, 0, :], incw_sb3[:, 1, :], incw_sb3[:44, 2, :]]
        w1v_sb = wpool.tile([128, 18, 128], DTM, name="w1v_sb")
        nc.gpsimd.dma_start(
            w1v_sb[:, :, :].rearrange("p a b -> p (a b)"), d_w1v[:, :]
        )
        w1s_sb = wpool.tile([72, 128], DTM, name="w1s_sb")
        nc.gpsimd.dma_start(w1s_sb[:, :], d_w1s[:, :])

        def load_chunked(dram, rows_per_chunk, width):
            tiles = []
            r0 = 0
            for ci, rows in enumerate(rows_per_chunk):
                t_ = wpool.tile([rows, width], dram.dtype, name=f"{dram.name}_c{ci}")
                nc.gpsimd.dma_start(t_[:, :], dram[r0 : r0 + rows, :])
                tiles.append(t_)
                r0 += rows
            return tiles

        # ------------- sample-0 inputs before the bulk weights -------------
        def load_inputs(s):
            xv = inpool.tile([128, 2, 34, 34], DTM, name="xv")
            for kc in range(2):
                for r0, r1 in ((0, 18), (18, 34)):
                    nc.sync.dma_start(
                        xv[:, kc, r0:r1, :].rearrange("p h w -> p (h w)"),
                        d_xv[s, kc * 128 : (kc + 1) * 128, r0 * 34 : r1 * 34],
                    )
            xsp = inpool.tile([72, HW], DTM, name="xsp")
            nc.sync.dma_start(xsp[:, :], d_xsp[s])
            txtc = inpool.tile([128, 3, 20], DTM, name="txtc")
            nc.sync.dma_start(
                txtc[:, :, :].rearrange("p a b -> p (a b)"), d_txt[s]
            )
            txtT = inpool.tile([20, DS + 1], DTM, name="txtT")
            nc.sync.dma_start(txtT[:, :], d_txtT[s])
            return xv, xsp, txtc, txtT

        preloaded = load_inputs(0)

        w2_sb = wpool.tile([128, 9, 256], DTM, name="w2_sb")
        nc.gpsimd.dma_start(
            w2_sb[:, :, :].rearrange("p a b -> p (a b)"), d_w2[:, :]
        )
        gs1_sb = wpool.tile([128, 32], dt, name="gs1_sb")
        nc.gpsimd.dma_start(gs1_sb[:, :], d_gs1[:, :])
        gs1T_sb = wpool.tile([32, 128], dt, name="gs1T_sb")
        nc.gpsimd.dma_start(gs1T_sb[:, :], d_gs1T[:, :])

        # ---- modulation vectors for all samples, batched (N=4 matmuls) ----
        txtall = wpool.tile([128, SPC, 3, 20], DTM, name="txtall")
        for s4 in range(SPC):
            nc.gpsimd.dma_start(
                txtall[:, s4, :, :].rearrange("p c l -> p (c l)"), d_txt[s4]
            )
        tmax4 = wpool.tile([128, 3, SPC], dt, name="tmax4")
        for kc in range(3):
            kr = DSCH[kc]
            for s4 in range(SPC):
                nc.vector.reduce_max(
                    tmax4[:kr, kc, s4 : s4 + 1],
                    txtall[:kr, s4, kc, :],
                    axis=mybir.AxisListType.X,
                )
        modall = wpool.tile([128, 3, SPC], dt, name="modall")
        MOD_ROWS0 = (128, 128, 2)
        for mb in range(3):
            mr = MOD_ROWS0[mb]
            mps4 = psmall.tile([128, SPC], dt, tag="small", name=f"modps4_{mb}")
            for kc in range(3):
                kr = DSCH[kc]
                nc.tensor.matmul(
                    mps4[:mr, :],
                    incw_sb[kc][:, mb * 128 : mb * 128 + mr],
                    tmax4[:kr, kc, :],
                    start=(kc == 0),
                    stop=(kc == 2),
                )
            nc.scalar.activation(
                modall[:mr, mb, :], mps4[:mr, :], AF.Identity,
                bias=incb_sb[:mr, mb : mb + 1],
            )

        # ------------- remaining weights -------------
        sv2 = wpool.tile([128, 6], dt, name="sv2")
        nc.gpsimd.dma_start(sv2[:, :], d_sv2[:, :])
        b2_sb, g2_sb, bt2_sb = sv2[:, 0:2], sv2[:, 2:4], sv2[:, 4:6]
        gs2_sb = wpool.tile([128, 2, 32], dt, name="gs2_sb")
        nc.gpsimd.dma_start(
            gs2_sb[:, :, :].rearrange("p a b -> p (a b)"), d_gs2[:, :]
        )
        gs2T_sb = wpool.tile([32, 2, 128], dt, name="gs2T_sb")
        nc.gpsimd.dma_start(
            gs2T_sb[:, :, :].rearrange("p a b -> p (a b)"), d_gs2T[:, :]
        )
        rw_sb = load_chunked(d_rw, KCH, DS)
        kw_sb = load_chunked(d_kw, KCH, VF)
        qw_sb = load_chunked(d_qw, KCH, VF)
        vw_sb = load_chunked(d_vw, KCH, VF + 2)
        coord_sb = wpool.tile([3, HW], DTM, name="coord_sb")
        nc.gpsimd.dma_start(coord_sb[:, :], d_coord[:, :])
        ones1_sb = wpool.tile([1, 128], DTM, name="ones1_sb")
        nc.gpsimd.dma_start(ones1_sb[:, :], d_ones[:, :])
        ones20_sb = wpool.tile([20, 128], DTM, name="ones20_sb")
        nc.gpsimd.dma_start(ones20_sb[:, :], d_ones20[:, :])
        ones128_sb = wpool.tile([128, 128], DTM, name="ones128_sb")
        nc.gpsimd.dma_start(ones128_sb[:, :], d_ones128[:, :])

        # persistent zero-padded conv2 input (border stays zero forever)
        y1pad = wpool.tile([128, 34, 34], DTM, name="y1pad")
        nc.gpsimd.dma_start(
            y1pad[:, :, :].rearrange("p h w -> p (h w)"), d_zpad[:, :]
        )

        def group_norm_finish(gstat_ps, cb_ps, gamma_ap, beta_ap, gsT_ap, mb_tag):
            """gstat_ps: [32,2] PSUM (mean, E[x2]) per group; cb_ps: [128,2]
            PSUM slice for the broadcast-back (same bank as gstat_ps).
            Returns sc [128,2] SBUF: col0 = scale', col1 = bias'."""
            gb = spool.tile([32, 4], dt, name=f"gb_{mb_tag}")
            nc.vector.tensor_copy(gb[:, 0:1], gstat_ps[:, 0:1])
            nc.vector.tensor_tensor(gb[:, 3:4], gb[:, 0:1], gb[:, 0:1], ALU.mult)
            nc.vector.tensor_tensor(
                gb[:, 1:2], gstat_ps[:, 1:2], gb[:, 3:4], ALU.subtract
            )
            nc.vector.tensor_scalar_add(gb[:, 1:2], gb[:, 1:2], float(EPS))
            nc.scalar.activation(gb[:, 2:3], gb[:, 1:2], AF.Ln)
            nc.scalar.activation(gb[:, 1:2], gb[:, 2:3], AF.Exp, scale=-0.5)
            nc.tensor.matmul(cb_ps, gsT_ap, gb[:, 0:2], start=True, stop=True)
            sc = spool.tile([128, 2], dt, name=f"sc_{mb_tag}")
            nc.vector.tensor_tensor(sc[:, 0:1], gamma_ap, cb_ps[:, 1:2], ALU.mult)
            nc.vector.tensor_tensor(sc[:, 1:2], cb_ps[:, 0:1], sc[:, 0:1], ALU.mult)
            nc.vector.tensor_tensor(sc[:, 1:2], beta_ap, sc[:, 1:2], ALU.subtract)
            return sc

        def channel_stats(y_flat_ap, tag):
            """y: [128,1024] SBUF -> st2 [128,2] SBUF = (E[x], E[x^2])."""
            bnst = spool.tile([128, 2, 6], dt, name=f"bnst_{tag}")
            nc.vector.bn_stats(bnst[:, 0, :], y_flat_ap[:, 0:512])
            nc.vector.bn_stats(bnst[:, 1, :], y_flat_ap[:, 512:1024])
            mv = spool.tile([128, 2], dt, name=f"mv_{tag}")
            nc.vector.bn_aggr(mv[:, :], bnst[:, :, :])
            st2 = spool.tile([128, 2], dt, name=f"st2_{tag}")
            nc.vector.tensor_copy(st2[:, 0:1], mv[:, 0:1])
            nc.vector.tensor_tensor(st2[:, 1:2], mv[:, 0:1], mv[:, 0:1], ALU.mult)
            nc.vector.tensor_tensor(st2[:, 1:2], st2[:, 1:2], mv[:, 1:2], ALU.add)
            return st2

        WT_ROWS = (128, 128, 45)
        WV_ROWS = (128, 128, 3)
        MOD_ROWS = (128, 128, 2)

        def emit_attn(sd, s, ni, ones_rbc=False):
            """Visual self-attention for one n-half of sample s (uses sd =
            per-sample tiles).  S' for chunk j+1 is emitted before wv of
            chunk j so the in-order PE queue never stalls on the exp.
            ones_rbc: compute the softmax denominator with an extra
            ones-matmul per chunk instead of the serial D-row chain (used on
            the tail where nothing hides that latency)."""
            kT, qT, ktail, qtail, vsb = (
                sd["kT"], sd["qT"], sd["ktail"], sd["qtail"], sd["vsb"]
            )
            wvout, dstage, D0, rbcv = sd["wvout"], sd["dstage"], sd["D0"], sd["rbcv"]

            def sprime(j):
                sps = pstream.tile(
                    [128, 512], F32, tag="stream", name=f"sps_{s}_{ni}_{j}"
                )
                nc.tensor.matmul(
                    sps[:, :], qT[:, 0, ts(j, 128)], kT[:, 0, ts(ni, 512)],
                    start=True, stop=False,
                )
                nc.tensor.matmul(
                    sps[:, :], qT[:, 1, ts(j, 128)], kT[:, 1, ts(ni, 512)],
                    start=False, stop=False,
                )
                nc.tensor.matmul(
                    sps[:, :], qtail[:, ts(j, 128)], ktail[:, ts(ni, 512)],
                    start=False, stop=True,
                )
                Ej = epool.tile([128, 512], DTM, tag="E", name=f"Ej_{s}_{ni}_{j}")
                nc.scalar.activation(Ej[:, :], sps[:, :], AF.Exp, scale=RS_VID)
                return Ej

            wvp = [
                pacc.tile([128, 512], F32, tag="acc", name=f"wvps_{s}_{mb}_{ni}")
                for mb in range(3)
            ]
            rbD = (
                psmall.tile([128, 512], F32, tag="small", name=f"rbD_{s}_{ni}")
                if ones_rbc
                else None
            )
            Enext = sprime(0)
            for j in range(8):
                Ej = Enext
                if j + 1 < 8:
                    Enext = sprime(j + 1)
                for mb in range(3):
                    cols = ((0, 128), (128, 256), (256, 259))[mb]
                    nc.tensor.matmul(
                        wvp[mb][: WV_ROWS[mb], :],
                        vsb[:, j, cols[0] : cols[1]],
                        Ej[:, :],
                        start=(j == 0),
                        stop=(j == 7),
                    )
                if ones_rbc:
                    nc.tensor.matmul(
                        rbD[:, :], ones128_sb[:, :], Ej[:, :],
                        start=(j == 0), stop=(j == 7),
                    )
            if ones_rbc:
                lnv = sd["lnv"]
                nc.scalar.activation(lnv[:, ts(ni, 512)], rbD[:, :], AF.Ln)
                nc.scalar.activation(
                    rbcv[:, ts(ni, 512)], lnv[:, ts(ni, 512)], AF.Exp, scale=-1.0
                )
            # evacuate (releases PSUM), then normalize off the critical path
            for mb in range(3):
                rows = (128, 128, 2)[mb]
                if mb == 2:
                    nc.vector.tensor_copy(
                        wvout[:rows, mb, ts(ni, 512)], wvp[mb][:rows, :]
                    )
                else:
                    nc.scalar.activation(
                        wvout[:rows, mb, ts(ni, 512)], wvp[mb][:rows, :], AF.Copy
                    )
            if not ones_rbc:
                nc.scalar.activation(
                    dstage[0:3, ts(ni, 512)], wvp[2][0:3, :], AF.Copy
                )
                nc.sync.dma_start(D0[:, ts(ni, 512)], dstage[2:3, ts(ni, 512)])
                nc.scalar.activation(D0[:, ts(ni, 512)], D0[:, ts(ni, 512)], AF.Ln)
                nc.scalar.activation(
                    D0[:, ts(ni, 512)], D0[:, ts(ni, 512)], AF.Exp, scale=-1.0
                )
                rb_ps = pstream.tile(
                    [128, 512], F32, tag="stream", name=f"rbv_{s}_{ni}"
                )
                nc.tensor.matmul(
                    rb_ps[:, :], ones1_sb[:, :], D0[:, ts(ni, 512)],
                    start=True, stop=True,
                )
                nc.vector.tensor_copy(rbcv[:, ts(ni, 512)], rb_ps[:, :])
            for mb in range(3):
                rows = (128, 128, 2)[mb]
                nc.vector.tensor_tensor(
                    wvout[:rows, mb, ts(ni, 512)],
                    wvout[:rows, mb, ts(ni, 512)],
                    rbcv[:rows, ts(ni, 512)],
                    ALU.mult,
                )
            if ones_rbc:
                # tail: stream each half out on the idle sync queue
                nc.sync.dma_start(
                    d_out[s, 0:128, ts(ni, 512)], wvout[:, 0, ts(ni, 512)]
                )
                nc.sync.dma_start(
                    d_out[s, 128:256, ts(ni, 512)], wvout[:, 1, ts(ni, 512)]
                )
                nc.sync.dma_start(
                    d_out[s, 256:258, ts(ni, 512)], wvout[:2, 2, ts(ni, 512)]
                )

        def attn_out_dma(sd, s):
            wvout = sd["wvout"]
            nc.gpsimd.dma_start(d_out[s, 0:128, :], wvout[:, 0, :])
            nc.gpsimd.dma_start(d_out[s, 128:256, :], wvout[:, 1, :])
            nc.gpsimd.dma_start(d_out[s, 256:258, :], wvout[:2, 2, :])

        # ---------------- per-sample pipeline (attention of sample s-1 is
        # interleaved with the conv front of sample s) ----------------
        def emit_conv1(s, xv, xsp):
            y1raw = spool.tile([128, HW], dt, name="y1raw")
            for ni in range(2):
                ps = pconv.tile([128, 512], F32, tag="conv", name=f"c1ps_{s}_{ni}")
                h0 = ni * 16
                idx = 0
                for kc in range(2):
                    for t, (ty, tx) in enumerate(TAPS):
                        nc.tensor.matmul(
                            ps[:, :],
                            w1v_sb[:, t * 2 + kc, :],
                            xv[:, kc, ty + h0 : ty + h0 + 16, tx : tx + 32],
                            start=(idx == 0),
                            stop=False,
                        )
                        idx += 1
                nc.tensor.matmul(
                    ps[:, :], w1s_sb[:, :], xsp[:, ts(ni, 512)],
                    start=False, stop=True,
                )
                nc.scalar.activation(
                    y1raw[:, ts(ni, 512)], ps[:, :], AF.Identity, bias=b1_sb[:, 0:1]
                )
            return y1raw

        prev = None
        ios = {0: preloaded}
        y1s = {}
        for s in range(SPC):
            xv, xsp, txtc, txtT = ios.pop(s) if s in ios else load_inputs(s)
            y1raw = y1s.pop(s) if s in y1s else emit_conv1(s, xv, xsp)

            # GN1 + ReLU -> padded conv2 input
            st2 = channel_stats(y1raw, "gn1")
            gtile1 = psmall.tile([128, 6], dt, tag="small", name=f"gst1_{s}")
            nc.tensor.matmul(
                gtile1[:32, 0:2], gs1_sb[:, :], st2[:, :], start=True, stop=True
            )
            sc1 = group_norm_finish(
                gtile1[:32, 0:2], gtile1[:, 2:4], g1_sb[:, 0:1], bt1_sb[:, 0:1],
                gs1T_sb[:, :], "gn1",
            )
            nc.scalar.activation(
                y1pad[:, 1:33, 1:33],
                y1raw[:, :].rearrange("p (h w) -> p h w", h=32),
                AF.Relu,
                bias=sc1[:, 1:2],
                scale=sc1[:, 0:1],
            )

            if prev is not None:
                emit_attn(prev, s - 1, 0)
            elif s + 1 < SPC:
                ios[s + 1] = load_inputs(s + 1)
                y1s[s + 1] = emit_conv1(s + 1, ios[s + 1][0], ios[s + 1][1])

            # -------- conv2 --------
            y2raw = spool.tile([128, 2, HW], dt, name="y2raw")
            for mb in range(2):
                for ni in range(2):
                    ps = pconv.tile(
                        [128, 512], F32, tag="conv", name=f"c2ps_{s}_{mb}_{ni}"
                    )
                    h0 = ni * 16
                    for t, (ty, tx) in enumerate(TAPS):
                        nc.tensor.matmul(
                            ps[:, :],
                            w2_sb[:, t, ts(mb, 128)],
                            y1pad[:, ty + h0 : ty + h0 + 16, tx : tx + 32],
                            start=(t == 0),
                            stop=(t == 8),
                        )
                    nc.scalar.activation(
                        y2raw[:, mb, ts(ni, 512)], ps[:, :], AF.Identity,
                        bias=b2_sb[:, mb : mb + 1],
                    )

            # GN2 + ReLU -> xfeat
            st2a = channel_stats(y2raw[:, 0, :], "gn2a")
            st2b = channel_stats(y2raw[:, 1, :], "gn2b")
            gtile2 = psmall.tile([128, 6], dt, tag="small", name=f"gst2_{s}")
            nc.tensor.matmul(
                gtile2[:32, 0:2], gs2_sb[:, 0, :], st2a[:, :], start=True, stop=False
            )
            nc.tensor.matmul(
                gtile2[:32, 0:2], gs2_sb[:, 1, :], st2b[:, :], start=False, stop=True
            )
            xfeat = spool.tile([128, 2, HW], DTM, name="xfeat")
            for mb in range(2):
                sc2 = group_norm_finish(
                    gtile2[:32, 0:2],
                    gtile2[:, 2 + 2 * mb : 4 + 2 * mb],
                    g2_sb[:, mb : mb + 1],
                    bt2_sb[:, mb : mb + 1],
                    gs2T_sb[:, mb, :],
                    f"gn2_{mb}",
                )
                nc.scalar.activation(
                    xfeat[:, mb, :],
                    y2raw[:, mb, :],
                    AF.Relu,
                    bias=sc2[:, 1:2],
                    scale=sc2[:, 0:1],
                )

            if prev is not None:
                emit_attn(prev, s - 1, 1)
                attn_out_dma(prev, s - 1)
            elif s + 2 < SPC:
                ios[s + 2] = load_inputs(s + 2)
                y1s[s + 2] = emit_conv1(s + 2, ios[s + 2][0], ios[s + 2][1])

            def xfc(kc):
                if kc < 2:
                    return xfeat[:, kc, :]
                return coord_sb[:, :]

            # -------- vsr^T [300, 1024] --------
            vsrT = spool.tile([128, 3, HW], DTM, name="vsrT")
            for mb in range(3):
                mr = DSCH[mb]
                for ni in range(2):
                    ps = pconv.tile(
                        [128, 512], F32, tag="conv", name=f"vsrps_{s}_{mb}_{ni}"
                    )
                    for kc in range(3):
                        nc.tensor.matmul(
                            ps[:mr, :],
                            rw_sb[kc][:, mb * 128 : mb * 128 + mr],
                            xfc(kc)[:, ts(ni, 512)],
                            start=(kc == 0),
                            stop=(kc == 2),
                        )
                    if (mb + ni) % 2 == 0:
                        nc.vector.tensor_copy(vsrT[:mr, mb, ts(ni, 512)], ps[:mr, :])
                    else:
                        nc.scalar.activation(
                            vsrT[:mr, mb, ts(ni, 512)], ps[:mr, :], AF.Copy
                        )

            def emit_score_wt():
                # -------- text cross attention --------
                E_t = spool.tile([20, HW], DTM, name="E_t")
                for ni in range(2):
                    ps = pstream.tile([128, 512], F32, tag="stream", name=f"stps_{s}_{ni}")
                    for kc in range(3):
                        kr = DSCH[kc]
                        nc.tensor.matmul(
                            ps[:20, :],
                            txtc[:kr, kc, :],
                            vsrT[:kr, kc, ts(ni, 512)],
                            start=(kc == 0),
                            stop=(kc == 2),
                        )
                    nc.scalar.activation(
                        E_t[:, ts(ni, 512)], ps[:20, :], AF.Exp, scale=RS_TXT
                    )

                lnbc = spool.tile([128, HW], dt, name="lnbc")
                rbc = spool.tile([128, HW], dt, name="rbc")
                wtout = opool.tile([128, 3, HW], dt, name="wtout")
                for ni in range(2):
                    rb_ps = pstream.tile(
                        [128, 512], F32, tag="stream", name=f"rbt_{s}_{ni}"
                    )
                    nc.tensor.matmul(
                        rb_ps[:, :], ones20_sb[:, :], E_t[:, ts(ni, 512)],
                        start=True, stop=True,
                    )
                    nc.scalar.activation(lnbc[:, ts(ni, 512)], rb_ps[:, :], AF.Ln)
                    nc.scalar.activation(
                        rbc[:, ts(ni, 512)], lnbc[:, ts(ni, 512)], AF.Exp, scale=-1.0
                    )
                    wtp = []
                    for mb in range(3):
                        ps = pacc.tile(
                            [128, 512], F32, tag="acc", name=f"wtps_{s}_{mb}_{ni}"
                        )
                        nc.tensor.matmul(
                            ps[: WT_ROWS[mb], :],
                            txtT[:, mb * 128 : mb * 128 + WT_ROWS[mb]],
                            E_t[:, ts(ni, 512)],
                            start=True,
                            stop=True,
                        )
                        wtp.append(ps)
                    for mb in range(3):
                        rows = (128, 128, 44)[mb]
                        nc.vector.tensor_tensor(
                            wtout[:rows, mb, ts(ni, 512)],
                            wtp[mb][:rows, :],
                            rbc[:rows, ts(ni, 512)],
                            ALU.mult,
                        )
                nc.gpsimd.dma_start(d_out[s, 258:386, :], wtout[:, 0, :])
                nc.gpsimd.dma_start(d_out[s, 386:514, :], wtout[:, 1, :])
                nc.gpsimd.dma_start(d_out[s, 514:558, :], wtout[:44, 2, :])


            def emit_kq_v():
                # -------- k^T, q^T (mod folded into evacuation) --------
                kT = spool.tile([128, 2, HW], DTM, name="kT")
                qT = spool.tile([128, 2, HW], DTM, name="qT")
                ktail = spool.tile([2, HW], DTM, name="ktail")
                qtail = spool.tile([2, HW], DTM, name="qtail")
                for w_sb, dstT, dtail, nm in (
                    (kw_sb, kT, ktail, "k"),
                    (qw_sb, qT, qtail, "q"),
                ):
                    for mb in range(2):
                        for ni in range(2):
                            ps = pconv.tile(
                                [128, 512], F32, tag="conv", name=f"{nm}ps_{s}_{mb}_{ni}"
                            )
                            for kc in range(3):
                                nc.tensor.matmul(
                                    ps[:, :],
                                    w_sb[kc][:, ts(mb, 128)],
                                    xfc(kc)[:, ts(ni, 512)],
                                    start=(kc == 0),
                                    stop=(kc == 2),
                                )
                            if (mb + ni) % 2 == 0:
                                nc.vector.tensor_tensor(
                                    dstT[:, mb, ts(ni, 512)],
                                    ps[:, :],
                                    modall[:, mb, s : s + 1].to_broadcast((128, 512)),
                                    ALU.mult,
                                )
                            else:
                                nc.scalar.activation(
                                    dstT[:, mb, ts(ni, 512)], ps[:, :], AF.Identity,
                                    scale=modall[:, mb, s : s + 1],
                                )
                    for ni in range(2):
                        ps = pstream.tile(
                            [128, 512], F32, tag="stream", name=f"{nm}tps_{s}_{ni}"
                        )
                        for kc in range(3):
                            nc.tensor.matmul(
                                ps[:2, :],
                                w_sb[kc][:, 256:258],
                                xfc(kc)[:, ts(ni, 512)],
                                start=(kc == 0),
                                stop=(kc == 2),
                            )
                        nc.vector.tensor_tensor(
                            dtail[:, ts(ni, 512)],
                            ps[:2, :],
                            modall[0:2, 2, s : s + 1].to_broadcast((2, 512)),
                            ALU.mult,
                        )

                # -------- v [1024, 260] (m on partitions, ones column) --------
                vsb = spool.tile([128, 8, VF + 2], DTM, name="vsb")
                for j in range(8):
                    ps = pconv.tile([128, 512], F32, tag="conv", name=f"vps_{s}_{j}")
                    for kc in range(3):
                        nc.tensor.matmul(
                            ps[:, : VF + 2],
                            xfc(kc)[:, ts(j, 128)],
                            vw_sb[kc][:, :],
                            start=(kc == 0),
                            stop=(kc == 2),
                        )
                    if j % 2 == 0:
                        nc.vector.tensor_copy(vsb[:, j, :], ps[:, : VF + 2])
                    else:
                        nc.scalar.activation(vsb[:, j, :], ps[:, : VF + 2], AF.Copy)


                return kT, qT, ktail, qtail, vsb

            emit_score_wt()
            kT, qT, ktail, qtail, vsb = emit_kq_v()
            prev = {
                "kT": kT, "qT": qT, "ktail": ktail, "qtail": qtail,
                "vsb": vsb,
                "wvout": opool.tile([128, 3, HW], dt, name="wvout"),
                "dstage": spool.tile([3, HW], DTM, name="dstage"),
                "D0": spool.tile([1, HW], DTM, name="D0"),
                "rbcv": spool.tile([128, HW], dt, name="rbcv"),
                "lnv": spool.tile([128, HW], dt, name="lnv"),
            }
            if s == SPC - 1:
                emit_attn(prev, s, 0, ones_rbc=True)
                emit_attn(prev, s, 1, ones_rbc=True)

    nc.finalize()
    return nc


def _prep_inputs(inputs):
    """Host-side marshalling: shard over batch, transpose weights, im2col the
    spatial channels, build constant helper tensors."""
    f = np.float32
    video = np.asarray(inputs["video_feat"], f)  # [32,256,32,32]
    spat = np.asarray(inputs["spatial_feat"], f)  # [32,8,32,32]
    txt = np.asarray(inputs["txt"], f)  # [32,300,20]
    B = video.shape[0]

    xv = np.zeros((B, 256, 34, 34), f)
    xv[:, :, 1:33, 1:33] = video
    xv = xv.reshape(B, 256, 34 * 34)

    sp_pad = np.zeros((B, 8, 34, 34), f)
    sp_pad[:, :, 1:33, 1:33] = spat
    xsp = np.stack(
        [sp_pad[:, :, ty : ty + 32, tx : tx + 32] for (ty, tx) in TAPS], axis=1
    ).reshape(B, 72, HW)

    txtT = np.concatenate([txt.transpose(0, 2, 1), np.ones((B, 20, 1), f)], axis=2)
    txtp = np.zeros((B, 128, 3, 20), f)
    txtp[:, :, 0, :] = txt[:, 0:128]
    txtp[:, :, 1, :] = txt[:, 128:256]
    txtp[:, :44, 2, :] = txt[:, 256:300]
    txtp = txtp.reshape(B, 128, 60)

    w1 = np.asarray(inputs["conv1_w"], f)  # [128,264,3,3]
    w1v9 = w1[:, :256].transpose(2, 3, 1, 0).reshape(9, 256, 128)
    # SBUF layout [p, (t,kc), m]: partition p = in-channel within chunk
    w1v = np.zeros((128, 18, 128), f)
    for t in range(9):
        for kc in range(2):
            w1v[:, t * 2 + kc, :] = w1v9[t, kc * 128 : (kc + 1) * 128, :]
    w1v = w1v.reshape(128, 18 * 128)
    w1s = np.ascontiguousarray(w1[:, 256:].transpose(2, 3, 1, 0).reshape(72, 128))
    w29 = np.asarray(inputs["conv2_w"], f).transpose(2, 3, 1, 0).reshape(9, 128, 256)
    w2 = np.ascontiguousarray(w29.transpose(1, 0, 2).reshape(128, 9 * 256))

    def aug(w, b):  # [out,in] torch Linear -> [in+1, out] with bias row
        return np.ascontiguousarray(np.concatenate([w.T, b[None, :]], axis=0).astype(f))

    rw = aug(np.asarray(inputs["reduce_w"], f), np.asarray(inputs["reduce_b"], f))
    kw = aug(np.asarray(inputs["k_w"], f), np.asarray(inputs["k_b"], f))
    qw = aug(np.asarray(inputs["q_w"], f), np.asarray(inputs["q_b"], f))
    vw = np.zeros((VF + 1, VF + 2), f)
    vw[:VF, :VF] = np.asarray(inputs["v_w"], f).T
    vw[VF, :VF] = np.asarray(inputs["v_b"], f)
    vw[VF, VF] = 1.0
    incw300 = np.asarray(inputs["inc_w"], f).T  # [300,258]
    incw = np.zeros((128, 3, VF), f)
    incw[:, 0, :] = incw300[0:128]
    incw[:, 1, :] = incw300[128:256]
    incw[:44, 2, :] = incw300[256:300]
    incw = incw.reshape(128, 3 * VF)

    xr = np.linspace(-1.0, 1.0, 32, dtype=f)
    yy, xx = np.meshgrid(xr, xr, indexing="ij")
    coord3 = np.stack([xx.ravel(), yy.ravel(), np.ones(HW, f)]).astype(f)

    cidx = np.arange(128)
    gsel1 = np.zeros((128, 32), f)
    gsel1[cidx, cidx // 4] = 0.25
    gsel1T = np.zeros((32, 128), f)
    gsel1T[cidx // 4, cidx] = 1.0
    gsel2 = np.zeros((2, 128, 32), f)
    gsel2T = np.zeros((2, 32, 128), f)
    for kc in range(2):
        g = (kc * 128 + cidx) // 8
        gsel2[kc, cidx, g] = 0.125
        gsel2T[kc, g, cidx] = 1.0
    gsel2 = np.ascontiguousarray(gsel2.transpose(1, 0, 2).reshape(128, 64))
    gsel2T = np.ascontiguousarray(gsel2T.transpose(1, 0, 2).reshape(32, 256))

    sv1 = np.zeros((128, 6), f)
    sv1[:, 0] = np.asarray(inputs["conv1_b"], f)
    sv1[:, 1] = np.asarray(inputs["gn1_g"], f)
    sv1[:, 2] = np.asarray(inputs["gn1_b"], f)
    sv1[:, 3] = np.asarray(inputs["inc_b"], f)[0:128]
    sv1[:, 4] = np.asarray(inputs["inc_b"], f)[128:256]
    sv1[0:2, 5] = np.asarray(inputs["inc_b"], f)[256:258]
    sv2 = np.zeros((128, 6), f)
    sv2[:, 0:2] = np.asarray(inputs["conv2_b"], f).reshape(2, 128).T
    sv2[:, 2:4] = np.asarray(inputs["gn2_g"], f).reshape(2, 128).T
    sv2[:, 4:6] = np.asarray(inputs["gn2_b"], f).reshape(2, 128).T

    shared = {
        "w1v": w1v, "w1s": w1s, "w2": w2, "rw": rw, "kw": kw, "qw": qw,
        "vw": vw, "incw": incw, "coord3": coord3,
        "gsel1": gsel1, "gsel1T": gsel1T, "gsel2": gsel2, "gsel2T": gsel2T,
        "ones1": np.ones((1, 128), f),
        "ones20": np.ones((20, 128), f),
        "ones128": np.ones((128, 128), f),
        "svec1": sv1, "svec2": sv2,
        "zpad": np.zeros((128, 34 * 34), f),
    }

    in_maps = []
    for c in range(N_CORES):
        sl = slice(c * SPC, (c + 1) * SPC)
        m = dict(shared)
        m["xv"] = np.ascontiguousarray(xv[sl])
        m["xsp"] = np.ascontiguousarray(xsp[sl])
        m["txt"] = np.ascontiguousarray(txtp[sl])
        m["txtT"] = np.ascontiguousarray(txtT[sl])
        in_maps.append(m)
    return in_maps


def get_program():
    if "nc" not in _PROGRAM_CACHE:
        _PROGRAM_CACHE["nc"] = build_program()
    return _PROGRAM_CACHE["nc"]


def kernel(**inputs) -> np.ndarray:
    nc = get_program()
    in_maps = _prep_inputs(inputs)
    res = run_bass_kernel_spmd(nc, in_maps, list(range(N_CORES)))
    outs = [res.results[c]["out"] for c in range(N_CORES)]  # [4,558,1024] each
    full = np.concatenate(outs, axis=0).reshape(32, 558, 32, 32)
    return full.astype(np.float32)



# revision 12
# speedup vs baseline: 1.2211x; 1.2211x over previous
"""Trainium2 Bass kernel for nn_AttentionFusion — fp8-DoubleRow rewrite.

Sharding: pure data parallelism over batch (32 samples -> 8 cores x 4
samples), weights replicated.

All heavy matmuls run as fp8e4m3 DoubleRow (2 k-tiles per instruction,
0.5 cycles/row — 4x the f32r row rate for K-chunked contractions), with
power-of-2 scale management so every tensor sits in e4m3's healthy range.
Precision placement (validated vs the jax reference, rel_l2 ~1.05e-2 vs
the 2e-2 gate):
  conv1        acts single fp8 x weights single fp8 (tap-chunk pairs)
  conv2        acts single fp8 (y1pad) x weights hi+lo exact pairs
  vsr          xfeat single fp8 x reduce_w hi+lo exact pairs
  E_t/rb/wt    bf16 (text branch is error-dominant)
  k,q          single fp8 both sides (mod folded into q_w host-side)
  v            xfeat fp8 x v_w hi+lo exact (v_w drives a systematic
               mean-activation error if quantized single)
  S'/E/wv      fp8 throughout; softmax denominators via an fp8 ones-matmul
               (rbD) + DVE reciprocal, numerator/denominator share E
  output       bf16 (cast to f32 on host)

Softmax denominators use nc.vector.reciprocal + TT-mult (no Ln/Exp chains,
no partition-move DMAs).
"""

import os
import sys
from contextlib import ExitStack

for _p in ("/opt/trn_rl_repo",):
    if _p not in sys.path and os.path.isdir(_p):
        sys.path.insert(0, _p)

import numpy as np
import ml_dtypes

import concourse.bacc as bacc
import concourse.mybir as mybir
import concourse.tile as tile
from concourse.bass import ts
from concourse.bass_utils import run_bass_kernel_spmd

F32 = mybir.dt.float32
BF16 = mybir.dt.bfloat16
FP8 = mybir.dt.float8e4
AF = mybir.ActivationFunctionType
ALU = mybir.AluOpType
DR = mybir.MatmulPerfMode.DoubleRow

NP_F8 = ml_dtypes.float8_e4m3
NP_BF = ml_dtypes.bfloat16

N_CORES = 8
SPC = 4  # samples per core
HW = 1024
DS = 300
VF = 258
EPS = 1e-5

# power-of-2 scale plan
SXV = 16.0      # conv1 input activations
FW1 = 512.0     # conv1 weights               -> y1 psum x 8192
S2A = 16.0      # y1pad storage (folded into gn1 affine)
FW2 = 512.0     # conv2 weights               -> y2 psum x 8192
SX = 16.0       # xfeat storage (folded into gn2 affine)
SC = 64.0       # coords + ones row storage
FRW = 512.0     # reduce_w                    -> vsr psum x 8192
FKW = 2048.0    # k_w                         -> k  psum x 32768
FQW = 512.0     # q_w*mod^2                   -> q' psum x 8192
FVW = 2048.0    # v_w                         -> v  psum x 32768
SKQV_EVAC = 1.0 / 512.0   # k->x64, q->x16, v->x64 storage scales
SV = 64.0       # v storage scale == ones128 value
S1 = SXV * FW1            # 8192
S2 = S2A * FW2            # 8192
SVSR = SX * FRW           # 8192
EPS1 = EPS * S1 * S1
EPS2 = EPS * S2 * S2
ESC_V = (1.0 / float(np.sqrt(VF))) / 1024.0
ESC_T = (1.0 / float(np.sqrt(DS))) / SVSR

TAPS = [(ty, tx) for ty in range(3) for tx in range(3)]

_PROGRAM_CACHE = {}


def _patch_act_tables():
    """Keep Exp/Ln/Relu/Identity/Copy pinned to one act table set so the
    act-table-load pass doesn't thrash between sets."""
    import concourse.bacc as _bacc
    import concourse.hw_specs as _hw

    if getattr(_bacc, "_act_tables_patched", False):
        return
    _orig = _hw.get_activation_tables
    mine = {AF.Exp, AF.Ln, AF.Relu, AF.Identity, AF.Copy}

    def patched(module_arch):
        tabs = _orig(module_arch)
        out = {}
        for name, funcs in tabs.items():
            if name == "natural_log_exp_and_others" or not (mine & funcs):
                out[name] = funcs
            else:
                out[name] = funcs - mine
        return out

    _bacc.get_activation_tables = patched
    _bacc._act_tables_patched = True


def _patch_drain_barrier():
    """Split the kernel-tail drain's per-proc sem waits across engines."""
    import concourse.tile as tile_mod
    from concourse.vector_clock import ScopedClock

    if getattr(tile_mod, "_drain_patched", False):
        return

    def _patched(self, tick_clock, wait_clock):
        nc = self.nc
        drain_inst = nc.sync.drain()
        wait_clock.add_sem_waits(
            drain_inst.ins, ScopedClock({None: tick_clock.global_clock})
        )
        si = drain_inst.ins.sync_info
        waits = list(si.on_wait or [])
        if len(waits) > 1:
            si.on_wait = waits[:1]
            engines = [nc.sync, nc.scalar, nc.vector, nc.tensor, nc.gpsimd]
            for i in range(1, len(waits)):
                extra = engines[i % len(engines)].drain()
                extra.ins.sync_info = mybir.SyncInfo(
                    on_wait=[waits[i]], on_update=[]
                )
        nc.all_engine_barrier()
        assert self.sems is not None
        popped = nc._tile_sem_poison_stack.pop()
        assert popped is self._sem_poison
        nc.clear_and_free_semaphores(list(self.sems.allocated().values()))
        nc.all_engine_barrier()

    tile_mod.TileContext._drain_and_barrier = _patched
    tile_mod._drain_patched = True


def build_program():
    _patch_act_tables()
    _patch_drain_barrier()
    nc = bacc.Bacc()
    dt = F32

    # ---------------- DRAM declarations ----------------
    d_xv = nc.dram_tensor("xv", [SPC, 128, 2 * 34 * 34], FP8, kind="ExternalInput")
    d_xsp = nc.dram_tensor("xsp", [SPC, 74, 2 * HW], FP8, kind="ExternalInput")
    d_txtc = nc.dram_tensor("txtc", [SPC, 128, 3 * 20], BF16, kind="ExternalInput")
    d_txtT = nc.dram_tensor("txtT", [SPC, 20, DS], BF16, kind="ExternalInput")
    d_qw = nc.dram_tensor("qw", [SPC, 128, 2 * VF], FP8, kind="ExternalInput")
    d_qwc2 = nc.dram_tensor("qwc2", [SPC, 3, 2 * VF], FP8, kind="ExternalInput")
    d_w1v = nc.dram_tensor("w1v", [128, 9 * 2 * 128], FP8, kind="ExternalInput")
    d_w1s = nc.dram_tensor("w1s", [74, 2 * 128], FP8, kind="ExternalInput")
    d_w2 = nc.dram_tensor("w2", [128, 9 * 2 * 2 * 128], FP8, kind="ExternalInput")
    d_rwh = nc.dram_tensor("rwh", [128, 2 * DS], FP8, kind="ExternalInput")
    d_rwl = nc.dram_tensor("rwl", [128, 2 * DS], FP8, kind="ExternalInput")
    d_rwc2 = nc.dram_tensor("rwc2", [3, 2 * DS], FP8, kind="ExternalInput")
    d_kw = nc.dram_tensor("kw", [128, 2 * VF], FP8, kind="ExternalInput")
    d_kwc2 = nc.dram_tensor("kwc2", [3, 2 * VF], FP8, kind="ExternalInput")
    d_vwh = nc.dram_tensor("vwh", [128, 2 * VF], FP8, kind="ExternalInput")
    d_vwl = nc.dram_tensor("vwl", [128, 2 * VF], FP8, kind="ExternalInput")
    d_vwc2 = nc.dram_tensor("vwc2", [3, 2 * VF], FP8, kind="ExternalInput")
    d_coordp = nc.dram_tensor("coordp", [3, 2 * HW], FP8, kind="ExternalInput")
    d_onesp = nc.dram_tensor("onesp", [128, 2 * 128], FP8, kind="ExternalInput")
    d_ones20 = nc.dram_tensor("ones20", [20, 128], BF16, kind="ExternalInput")
    d_gs1 = nc.dram_tensor("gsel1", [128, 32], dt, kind="ExternalInput")
    d_gs1T = nc.dram_tensor("gsel1T", [32, 128], dt, kind="ExternalInput")
    d_gs2 = nc.dram_tensor("gsel2", [128, 2 * 16], dt, kind="ExternalInput")
    d_gs2T = nc.dram_tensor("gsel2T", [16, 2 * 128], dt, kind="ExternalInput")
    d_sv1 = nc.dram_tensor("svec1", [128, 2], dt, kind="ExternalInput")
    d_sv2 = nc.dram_tensor("svec2", [128, 6], dt, kind="ExternalInput")
    d_zpad = nc.dram_tensor("zpad", [128, 34 * 34], FP8, kind="ExternalInput")
    d_out = nc.dram_tensor("out", [SPC, 558, HW], BF16, kind="ExternalOutput")

    with tile.TileContext(nc) as tc, ExitStack() as ctx:
        wpool = ctx.enter_context(tc.tile_pool(name="weights", bufs=1))
        inpool = ctx.enter_context(tc.tile_pool(name="inputs", bufs=2))
        spool = ctx.enter_context(tc.tile_pool(name="work", bufs=1))
        opool = ctx.enter_context(tc.tile_pool(name="outs", bufs=1))
        epool = ctx.enter_context(tc.tile_pool(name="estream", bufs=3))
        pA = ctx.enter_context(tc.tile_pool(name="pA", bufs=1, space="PSUM"))
        pB = ctx.enter_context(tc.tile_pool(name="pB", bufs=1, space="PSUM"))
        pS = ctx.enter_context(tc.tile_pool(name="pS", bufs=2, space="PSUM"))
        pC = ctx.enter_context(tc.tile_pool(name="pC", bufs=2, space="PSUM"))

        # ---------- conv1 weights + sample-0 inputs first ----------
        sv1 = wpool.tile([128, 2], dt, name="sv1")
        nc.gpsimd.dma_start(sv1[:, :], d_sv1[:, :])
        g1_ap, b1_ap = sv1[:, 0:1], sv1[:, 1:2]
        w1v_sb = wpool.tile([128, 9, 2, 128], FP8, name="w1v_sb")
        nc.gpsimd.dma_start(
            w1v_sb[:, :, :, :].rearrange("p a b c -> p (a b c)"), d_w1v[:, :]
        )
        w1s_sb = wpool.tile([74, 2, 128], FP8, name="w1s_sb")
        nc.gpsimd.dma_start(
            w1s_sb[:, :, :].rearrange("p a b -> p (a b)"), d_w1s[:, :]
        )
        gs1_sb = wpool.tile([128, 32], dt, name="gs1_sb")
        nc.gpsimd.dma_start(gs1_sb[:, :], d_gs1[:, :])
        gs1T_sb = wpool.tile([32, 128], dt, name="gs1T_sb")
        nc.gpsimd.dma_start(gs1T_sb[:, :], d_gs1T[:, :])

        def load_inputs(s):
            xv = inpool.tile([128, 2, 34, 34], FP8, name="xv")
            nc.sync.dma_start(
                xv[:, :, :, :].rearrange("p a h w -> p (a h w)"), d_xv[s]
            )
            xsp = inpool.tile([74, 2, HW], FP8, name="xsp")
            nc.sync.dma_start(
                xsp[:, :, :].rearrange("p a b -> p (a b)"), d_xsp[s]
            )
            txtc = inpool.tile([128, 3, 20], BF16, name="txtc")
            nc.sync.dma_start(
                txtc[:, :, :].rearrange("p a b -> p (a b)"), d_txtc[s]
            )
            txtT = inpool.tile([20, DS], BF16, name="txtT")
            nc.sync.dma_start(txtT[:, :], d_txtT[s])
            qw = inpool.tile([128, 2, VF], FP8, name="qw")
            nc.sync.dma_start(
                qw[:, :, :].rearrange("p a b -> p (a b)"), d_qw[s]
            )
            qwc2 = inpool.tile([3, 2, VF], FP8, name="qwc2")
            nc.sync.dma_start(
                qwc2[:, :, :].rearrange("p a b -> p (a b)"), d_qwc2[s]
            )
            return dict(xv=xv, xsp=xsp, txtc=txtc, txtT=txtT, qw=qw, qwc2=qwc2)

        preloaded = load_inputs(0)

        # ---------- remaining weights ----------
        w2_sb = wpool.tile([128, 9, 2, 2, 128], FP8, name="w2_sb")
        nc.gpsimd.dma_start(
            w2_sb[:, :, :, :, :].rearrange("p a b c d -> p (a b c d)"), d_w2[:, :]
        )
        sv2 = wpool.tile([128, 6], dt, name="sv2")
        nc.gpsimd.dma_start(sv2[:, :], d_sv2[:, :])
        g2_ap, b2_ap, b2s_ap = sv2[:, 0:2], sv2[:, 2:4], sv2[:, 4:6]
        gs2_sb = wpool.tile([128, 2, 16], dt, name="gs2_sb")
        nc.gpsimd.dma_start(
            gs2_sb[:, :, :].rearrange("p a b -> p (a b)"), d_gs2[:, :]
        )
        gs2T_sb = wpool.tile([16, 2, 128], dt, name="gs2T_sb")
        nc.gpsimd.dma_start(
            gs2T_sb[:, :, :].rearrange("p a b -> p (a b)"), d_gs2T[:, :]
        )
        rwh_sb = wpool.tile([128, 2, DS], FP8, name="rwh_sb")
        nc.gpsimd.dma_start(rwh_sb[:, :, :].rearrange("p a b -> p (a b)"), d_rwh[:, :])
        rwl_sb = wpool.tile([128, 2, DS], FP8, name="rwl_sb")
        nc.gpsimd.dma_start(rwl_sb[:, :, :].rearrange("p a b -> p (a b)"), d_rwl[:, :])
        rwc2_sb = wpool.tile([3, 2, DS], FP8, name="rwc2_sb")
        nc.gpsimd.dma_start(rwc2_sb[:, :, :].rearrange("p a b -> p (a b)"), d_rwc2[:, :])
        kw_sb = wpool.tile([128, 2, VF], FP8, name="kw_sb")
        nc.gpsimd.dma_start(kw_sb[:, :, :].rearrange("p a b -> p (a b)"), d_kw[:, :])
        kwc2_sb = wpool.tile([3, 2, VF], FP8, name="kwc2_sb")
        nc.gpsimd.dma_start(kwc2_sb[:, :, :].rearrange("p a b -> p (a b)"), d_kwc2[:, :])
        vwh_sb = wpool.tile([128, 2, VF], FP8, name="vwh_sb")
        nc.gpsimd.dma_start(vwh_sb[:, :, :].rearrange("p a b -> p (a b)"), d_vwh[:, :])
        vwl_sb = wpool.tile([128, 2, VF], FP8, name="vwl_sb")
        nc.gpsimd.dma_start(vwl_sb[:, :, :].rearrange("p a b -> p (a b)"), d_vwl[:, :])
        vwc2_sb = wpool.tile([3, 2, VF], FP8, name="vwc2_sb")
        nc.gpsimd.dma_start(vwc2_sb[:, :, :].rearrange("p a b -> p (a b)"), d_vwc2[:, :])
        coordp_sb = wpool.tile([3, 2, HW], FP8, name="coordp_sb")
        nc.gpsimd.dma_start(
            coordp_sb[:, :, :].rearrange("p a b -> p (a b)"), d_coordp[:, :]
        )
        onesp_sb = wpool.tile([128, 2, 128], FP8, name="onesp_sb")
        nc.gpsimd.dma_start(
            onesp_sb[:, :, :].rearrange("p a b -> p (a b)"), d_onesp[:, :]
        )
        ones20_sb = wpool.tile([20, 128], BF16, name="ones20_sb")
        nc.gpsimd.dma_start(ones20_sb[:, :], d_ones20[:, :])

        # persistent zero-padded conv2 input (border stays zero forever)
        y1pad = wpool.tile([128, 1, 34, 34], FP8, name="y1pad")
        nc.gpsimd.dma_start(
            y1pad[:, :, :, :].rearrange("p a h w -> p (a h w)"), d_zpad[:, :]
        )
        # persistent k/q tail tiles; plane 1 must stay zero (S' tail pairs)
        ktail = wpool.tile([2, 2, HW], FP8, name="ktail")
        nc.vector.memset(ktail[:, :, :].rearrange("p a b -> p (a b)"), 0)
        qtail = wpool.tile([2, 2, HW], FP8, name="qtail")
        nc.vector.memset(qtail[:, :, :].rearrange("p a b -> p (a b)"), 0)

        # ---------------- helpers ----------------
        def group_norm_finish(gstat_ps, cb_ps, gamma_ap, beta_ap, gsT_ap,
                              eps_s, groups, tag, bias_col=None):
            """gstat_ps: [G,2] PSUM (mean, E[x2]) per group (scaled domain);
            cb_ps: [128,2] PSUM for the broadcast-back.  Returns sc [128,2]
            SBUF: col0 = scale, col1 = bias for act(relu, psum-input).
            bias_col: [128,1] host column of b*S to subtract from the
            broadcast channel mean (act input psum is un-biased)."""
            gb = spool.tile([groups, 4], dt, name=f"gb_{tag}")
            nc.vector.tensor_copy(gb[:, 0:1], gstat_ps[:, 0:1])
            nc.vector.tensor_tensor(gb[:, 3:4], gb[:, 0:1], gb[:, 0:1], ALU.mult)
            nc.vector.tensor_tensor(
                gb[:, 1:2], gstat_ps[:, 1:2], gb[:, 3:4], ALU.subtract
            )
            nc.vector.tensor_scalar_add(gb[:, 1:2], gb[:, 1:2], float(eps_s))
            nc.scalar.activation(gb[:, 2:3], gb[:, 1:2], AF.Ln)
            nc.scalar.activation(gb[:, 1:2], gb[:, 2:3], AF.Exp, scale=-0.5)
            nc.tensor.matmul(cb_ps, gsT_ap, gb[:, 0:2], start=True, stop=True)
            sc = spool.tile([128, 3], dt, name=f"sc_{tag}")
            nc.vector.tensor_tensor(sc[:, 0:1], gamma_ap, cb_ps[:, 1:2], ALU.mult)
            if bias_col is not None:
                nc.vector.tensor_tensor(
                    sc[:, 2:3], cb_ps[:, 0:1], bias_col, ALU.subtract
                )
                mu_ap = sc[:, 2:3]
            else:
                mu_ap = cb_ps[:, 0:1]
            nc.vector.tensor_tensor(sc[:, 1:2], mu_ap, sc[:, 0:1], ALU.mult)
            nc.vector.tensor_tensor(sc[:, 1:2], beta_ap, sc[:, 1:2], ALU.subtract)
            return sc

        def channel_stats(ps_a, ps_b, tag, bias_col=None):
            """Two [128,512] PSUM halves -> st2 [128,2] = (mean_b, E_b[x^2])."""
            bnst = spool.tile([128, 2, 6], dt, name=f"bnst_{tag}")
            nc.vector.bn_stats(bnst[:, 0, :], ps_a)
            nc.vector.bn_stats(bnst[:, 1, :], ps_b)
            mv = spool.tile([128, 2], dt, name=f"mv_{tag}")
            nc.vector.bn_aggr(mv[:, :], bnst[:, :, :])
            st2 = spool.tile([128, 2], dt, name=f"st2_{tag}")
            if bias_col is not None:
                nc.vector.tensor_tensor(st2[:, 0:1], mv[:, 0:1], bias_col, ALU.add)
            else:
                nc.vector.tensor_copy(st2[:, 0:1], mv[:, 0:1])
            nc.vector.tensor_tensor(st2[:, 1:2], st2[:, 0:1], st2[:, 0:1], ALU.mult)
            nc.vector.tensor_tensor(st2[:, 1:2], st2[:, 1:2], mv[:, 1:2], ALU.add)
            return st2

        # ---------------- attention for one n-half ----------------
        def emit_attn(sd, s, ni):
            kT, qT, vsb, wvout = sd["kT"], sd["qT"], sd["vsb"], sd["wvout"]
            wv01 = pA.tile([128, 1024], F32, tag="attn", name=f"wv01_{s}_{ni}")
            wvD = pB.tile([128, 1024], F32, tag="attn", name=f"wvD_{s}_{ni}")

            for jp in range(4):
                Ep = epool.tile([128, 2, 512], FP8, tag="E", name=f"E_{s}_{ni}_{jp}")
                for jj in range(2):
                    j = 2 * jp + jj
                    sps = pS.tile([128, 512], F32, tag="ps", name=f"sps_{s}_{ni}_{j}")
                    nc.tensor.matmul(
                        sps[:, :], qT[:, :, ts(j, 128)], kT[:, :, ts(ni, 512)],
                        start=True, stop=False, perf_mode=DR,
                    )
                    nc.tensor.matmul(
                        sps[:, :], qtail[:, :, ts(j, 128)], ktail[:, :, ts(ni, 512)],
                        start=False, stop=True, perf_mode=DR,
                    )
                    nc.scalar.activation(Ep[:, jj, :], sps[:, :], AF.Exp, scale=ESC_V)
                st, sp = (jp == 0), (jp == 3)
                nc.tensor.matmul(
                    wv01[:, 0:512], vsb[:, jp, :, 0:128], Ep[:, :, :],
                    start=st, stop=sp, perf_mode=DR,
                )
                nc.tensor.matmul(
                    wv01[:, 512:1024], vsb[:, jp, :, 128:256], Ep[:, :, :],
                    start=st, stop=sp, perf_mode=DR,
                )
                nc.tensor.matmul(
                    wvD[0:2, 512:1024], vsb[:, jp, :, 256:258], Ep[:, :, :],
                    start=st, stop=sp, perf_mode=DR,
                )
                nc.tensor.matmul(
                    wvD[:, 0:512], onesp_sb[:, :, :], Ep[:, :, :],
                    start=st, stop=sp, perf_mode=DR,
                )
            rbc = spool.tile([128, 512], dt, name=f"rbcv_{ni}")
            nc.vector.reciprocal(rbc[:, :], wvD[:, 0:512])
            nc.vector.tensor_tensor(
                wvout[:, 0, ts(ni, 512)], wv01[:, 0:512], rbc[:, :], ALU.mult
            )
            nc.vector.tensor_tensor(
                wvout[:, 1, ts(ni, 512)], wv01[:, 512:1024], rbc[:, :], ALU.mult
            )
            nc.vector.tensor_tensor(
                wvout[0:2, 2, ts(ni, 512)], wvD[0:2, 512:1024], rbc[0:2, :], ALU.mult
            )

        def attn_out_dma(sd, s):
            wvout = sd["wvout"]
            nc.gpsimd.dma_start(d_out[s, 0:128, :], wvout[:, 0, :])
            nc.gpsimd.dma_start(d_out[s, 128:256, :], wvout[:, 1, :])
            nc.sync.dma_start(d_out[s, 256:258, :], wvout[:2, 2, :])

        # ---------------- conv1 ----------------
        def emit_conv1(s, io):
            xv, xsp = io["xv"], io["xsp"]
            pss = []
            for ni in range(2):
                ps = pC.tile([128, 512], F32, tag="conv", name=f"c1ps_{s}_{ni}")
                h0 = ni * 16
                for t, (ty, tx) in enumerate(TAPS):
                    nc.tensor.matmul(
                        ps[:, :],
                        w1v_sb[:, t, :, :],
                        xv[:, :, ty + h0 : ty + h0 + 16, tx : tx + 32],
                        start=(t == 0), stop=False, perf_mode=DR,
                    )
                nc.tensor.matmul(
                    ps[:, :], w1s_sb[:, :, :], xsp[:, :, ts(ni, 512)],
                    start=False, stop=True, perf_mode=DR,
                )
                pss.append(ps)
            return pss

        # ---------------- per-sample pipeline ----------------
        prev = None
        ios = {0: preloaded}
        for s in range(SPC):
            io = ios.pop(s) if s in ios else load_inputs(s)
            if s + 1 < SPC:
                ios[s + 1] = load_inputs(s + 1)
            txtc, txtT, qw_s, qwc2_s = io["txtc"], io["txtT"], io["qw"], io["qwc2"]

            ps1 = emit_conv1(s, io)

            # GN1 + ReLU -> y1pad fp8 (conv1 bias folded into xsp ones rows)
            st2 = channel_stats(ps1[0][:, :], ps1[1][:, :], f"gn1_{s}")
            gt1 = pS.tile([128, 4], dt, tag="ps", name=f"gst1_{s}")
            nc.tensor.matmul(
                gt1[:32, 0:2], gs1_sb[:, :], st2[:, :], start=True, stop=True
            )
            sc1 = group_norm_finish(
                gt1[:32, 0:2], gt1[:, 2:4],
                g1_ap, b1_ap, gs1T_sb[:, :], EPS1, 32, f"gn1_{s}",
            )
            for ni in range(2):
                nc.scalar.activation(
                    y1pad[:, 0, 1 + ni * 16 : 17 + ni * 16, 1:33],
                    ps1[ni][:, :].rearrange("p (h w) -> p h w", h=16),
                    AF.Relu, bias=sc1[:, 1:2], scale=sc1[:, 0:1],
                )

            if prev is not None:
                emit_attn(prev, s - 1, 0)

            # -------- conv2 (W hi/lo pairs, stride-0 moving) --------
            xfeat = spool.tile([128, 2, HW], FP8, name="xfeat")
            for mb in range(2):
                ps2 = []
                for ni in range(2):
                    ps = pC.tile([128, 512], F32, tag="conv", name=f"c2ps_{s}_{mb}_{ni}")
                    h0 = ni * 16
                    for t, (ty, tx) in enumerate(TAPS):
                        nc.tensor.matmul(
                            ps[:, :],
                            w2_sb[:, t, mb, :, :],
                            y1pad[:, 0:1, ty + h0 : ty + h0 + 16, tx : tx + 32]
                            .to_broadcast((128, 2, 16, 32)),
                            start=(t == 0), stop=(t == 8), perf_mode=DR,
                        )
                    ps2.append(ps)
                st2b = channel_stats(
                    ps2[0][:, :], ps2[1][:, :], f"gn2_{s}_{mb}",
                    bias_col=b2s_ap[:, mb : mb + 1],
                )
                gt2 = pS.tile([128, 4], dt, tag="ps", name=f"gst2_{s}_{mb}")
                nc.tensor.matmul(
                    gt2[:16, 0:2], gs2_sb[:, mb, :], st2b[:, :], start=True, stop=True
                )
                sc2 = group_norm_finish(
                    gt2[:16, 0:2], gt2[:, 2:4],
                    g2_ap[:, mb : mb + 1], b2_ap[:, mb : mb + 1],
                    gs2T_sb[:, mb, :], EPS2, 16, f"gn2_{s}_{mb}",
                    bias_col=b2s_ap[:, mb : mb + 1],
                )
                for ni in range(2):
                    nc.scalar.activation(
                        xfeat[:, mb, ts(ni, 512)], ps2[ni][:, :],
                        AF.Relu, bias=sc2[:, 1:2], scale=sc2[:, 0:1],
                    )

            if prev is not None:
                emit_attn(prev, s - 1, 1)
                attn_out_dma(prev, s - 1)

            # -------- vsr^T [300, 1024] bf16 (rw hi/lo pairs) --------
            vsrT = spool.tile([128, 3, HW], BF16, name="vsrT")
            DSCH = (128, 128, 44)
            for mb in range(3):
                mr = DSCH[mb]
                m0 = mb * 128
                for ni in range(2):
                    ps = pC.tile([128, 512], F32, tag="conv", name=f"vsr_{s}_{mb}_{ni}")
                    nc.tensor.matmul(
                        ps[:mr, :], rwh_sb[:, :, m0 : m0 + mr],
                        xfeat[:, :, ts(ni, 512)],
                        start=True, stop=False, perf_mode=DR,
                    )
                    nc.tensor.matmul(
                        ps[:mr, :], rwl_sb[:, :, m0 : m0 + mr],
                        xfeat[:, :, ts(ni, 512)],
                        start=False, stop=False, perf_mode=DR,
                    )
                    nc.tensor.matmul(
                        ps[:mr, :], rwc2_sb[:, :, m0 : m0 + mr],
                        coordp_sb[:, :, ts(ni, 512)],
                        start=False, stop=True, perf_mode=DR,
                    )
                    k = (mb * 2 + ni) % 3
                    if k == 0:
                        nc.vector.tensor_copy(vsrT[:mr, mb, ts(ni, 512)], ps[:mr, :])
                    elif k == 1:
                        nc.scalar.activation(
                            vsrT[:mr, mb, ts(ni, 512)], ps[:mr, :], AF.Copy
                        )
                    else:
                        nc.gpsimd.tensor_copy(vsrT[:mr, mb, ts(ni, 512)], ps[:mr, :])

            # -------- text cross attention (bf16) --------
            E_t = spool.tile([20, HW], BF16, name="E_t")
            for ni in range(2):
                ps = pS.tile([128, 512], F32, tag="ps", name=f"et_{s}_{ni}")
                for kc in range(3):
                    kr = DSCH[kc]
                    nc.tensor.matmul(
                        ps[:20, :], txtc[:kr, kc, :], vsrT[:kr, kc, ts(ni, 512)],
                        start=(kc == 0), stop=(kc == 2),
                    )
                nc.scalar.activation(
                    E_t[:, ts(ni, 512)], ps[:20, :], AF.Exp, scale=ESC_T
                )
            rbc_t = spool.tile([128, HW], dt, name="rbc_t")
            for ni in range(2):
                ps = pS.tile([128, 512], F32, tag="ps", name=f"rbt_{s}_{ni}")
                nc.tensor.matmul(
                    ps[:, :], ones20_sb[:, :], E_t[:, ts(ni, 512)],
                    start=True, stop=True,
                )
                nc.vector.reciprocal(rbc_t[:, ts(ni, 512)], ps[:, :])
            wtout = opool.tile([128, 3, HW], BF16, name="wtout")
            WT_ROWS = (128, 128, 44)
            for mb in range(3):
                for ni in range(2):
                    ps = pC.tile(
                        [128, 512], F32, tag="conv", name=f"wt_{s}_{mb}_{ni}"
                    )
                    nc.tensor.matmul(
                        ps[: WT_ROWS[mb], :],
                        txtT[:, mb * 128 : mb * 128 + WT_ROWS[mb]],
                        E_t[:, ts(ni, 512)],
                        start=True, stop=True,
                    )
                    nc.vector.tensor_tensor(
                        wtout[: WT_ROWS[mb], mb, ts(ni, 512)],
                        ps[: WT_ROWS[mb], :],
                        rbc_t[: WT_ROWS[mb], ts(ni, 512)],
                        ALU.mult,
                    )
            nc.gpsimd.dma_start(d_out[s, 258:386, :], wtout[:, 0, :])
            nc.gpsimd.dma_start(d_out[s, 386:514, :], wtout[:, 1, :])
            nc.gpsimd.dma_start(d_out[s, 514:558, :], wtout[:44, 2, :])

            # -------- k^T, q^T fp8 (single-fp8 both sides) --------
            kT = spool.tile([128, 2, HW], FP8, name="kT")
            qT = spool.tile([128, 2, HW], FP8, name="qT")
            for wi, (w_sb, wc2_sb, dstT) in enumerate(
                ((kw_sb, kwc2_sb, kT), (qw_s, qwc2_s, qT))
            ):
                for mb in range(2):
                    for ni in range(2):
                        ps = pC.tile(
                            [128, 512], F32, tag="conv", name=f"kq_{s}_{wi}_{mb}_{ni}"
                        )
                        nc.tensor.matmul(
                            ps[:, :], w_sb[:, :, ts(mb, 128)],
                            xfeat[:, :, ts(ni, 512)],
                            start=True, stop=False, perf_mode=DR,
                        )
                        nc.tensor.matmul(
                            ps[:, :], wc2_sb[:, :, ts(mb, 128)],
                            coordp_sb[:, :, ts(ni, 512)],
                            start=False, stop=True, perf_mode=DR,
                        )
                        k = (wi * 4 + mb * 2 + ni) % 4
                        if k in (0, 2):
                            nc.scalar.activation(
                                dstT[:, mb, ts(ni, 512)], ps[:, :], AF.Copy,
                                scale=SKQV_EVAC,
                            )
                        elif k == 1:
                            nc.vector.tensor_scalar_mul(
                                dstT[:, mb, ts(ni, 512)], ps[:, :], SKQV_EVAC
                            )
                        else:
                            nc.gpsimd.tensor_scalar_mul(
                                dstT[:, mb, ts(ni, 512)], ps[:, :], SKQV_EVAC
                            )
            # tails (output cols 256,257 of k and q)
            pkt = pA.tile([2, 1024], F32, tag="attn", name=f"ktp_{s}")
            pqt = pB.tile([2, 1024], F32, tag="attn", name=f"qtp_{s}")
            for ni in range(2):
                nc.tensor.matmul(
                    pkt[:, ts(ni, 512)], kw_sb[:, :, 256:258],
                    xfeat[:, :, ts(ni, 512)],
                    start=True, stop=False, perf_mode=DR,
                )
                nc.tensor.matmul(
                    pkt[:, ts(ni, 512)], kwc2_sb[:, :, 256:258],
                    coordp_sb[:, :, ts(ni, 512)],
                    start=False, stop=True, perf_mode=DR,
                )
                nc.tensor.matmul(
                    pqt[:, ts(ni, 512)], qw_s[:, :, 256:258],
                    xfeat[:, :, ts(ni, 512)],
                    start=True, stop=False, perf_mode=DR,
                )
                nc.tensor.matmul(
                    pqt[:, ts(ni, 512)], qwc2_s[:, :, 256:258],
                    coordp_sb[:, :, ts(ni, 512)],
                    start=False, stop=True, perf_mode=DR,
                )
            nc.vector.tensor_scalar_mul(
                ktail[:, 0, :], pkt[:, :], SKQV_EVAC
            )
            nc.scalar.activation(
                qtail[:, 0, :], pqt[:, :], AF.Copy, scale=SKQV_EVAC
            )

            # -------- v [1024, 258] fp8 (vw hi/lo pairs) --------
            vsb = spool.tile([128, 4, 2, VF], FP8, name="vsb")
            for j in range(8):
                ps = pC.tile([128, VF], F32, tag="conv", name=f"v_{s}_{j}")
                nc.tensor.matmul(
                    ps[:, :], xfeat[:, :, ts(j, 128)], vwh_sb[:, :, :],
                    start=True, stop=False, perf_mode=DR,
                )
                nc.tensor.matmul(
                    ps[:, :], xfeat[:, :, ts(j, 128)], vwl_sb[:, :, :],
                    start=False, stop=False, perf_mode=DR,
                )
                nc.tensor.matmul(
                    ps[:, :], coordp_sb[:, :, ts(j, 128)], vwc2_sb[:, :, :],
                    start=False, stop=True, perf_mode=DR,
                )
                k = j % 4
                if k in (0, 2):
                    nc.gpsimd.tensor_scalar_mul(
                        vsb[:, j // 2, j % 2, :], ps[:, :], SKQV_EVAC
                    )
                elif k == 1:
                    nc.vector.tensor_scalar_mul(
                        vsb[:, j // 2, j % 2, :], ps[:, :], SKQV_EVAC
                    )
                else:
                    nc.scalar.activation(
                        vsb[:, j // 2, j % 2, :], ps[:, :], AF.Copy,
                        scale=SKQV_EVAC,
                    )

            prev = {
                "kT": kT, "qT": qT, "vsb": vsb,
                "wvout": opool.tile([128, 3, HW], BF16, name="wvout"),
            }
            if s == SPC - 1:
                emit_attn(prev, s, 0)
                emit_attn(prev, s, 1)
                attn_out_dma(prev, s)

    nc.finalize()
    return nc


def _q8(x):
    return np.asarray(x, np.float32).astype(NP_F8)


def _hilo(x):
    h = _q8(x)
    l = _q8(np.asarray(x, np.float32) - h.astype(np.float32))
    return h, l


def _prep_inputs(inputs):
    """Host-side marshalling: shard over batch, scale + quantize weights,
    im2col the spatial channels, fold mod^2 into q_w, hi/lo-split the
    error-critical weights."""
    f = np.float32
    video = np.asarray(inputs["video_feat"], f)
    spat = np.asarray(inputs["spatial_feat"], f)
    txt = np.asarray(inputs["txt"], f)
    B = video.shape[0]

    # conv1 inputs: video padded, x SXV, fp8, partition-major [128, 2, 1156]
    xv = np.zeros((B, 256, 34, 34), f)
    xv[:, :, 1:33, 1:33] = video * SXV
    xv = _q8(np.ascontiguousarray(
        xv.reshape(B, 2, 128, 34 * 34).transpose(0, 2, 1, 3)
    ).reshape(B, 128, 2 * 34 * 34))

    # spatial: host im2col (9 taps x 8 ch = 72 rows) + 2 bias-ones rows
    sp_pad = np.zeros((B, 8, 34, 34), f)
    sp_pad[:, :, 1:33, 1:33] = spat * SXV
    xsp_v = np.stack(
        [sp_pad[:, :, ty : ty + 32, tx : tx + 32] for (ty, tx) in TAPS], axis=1
    ).reshape(B, 72, HW)
    xsp = np.zeros((B, 74, 2, HW), f)
    xsp[:, :72, 0, :] = xsp_v
    xsp[:, 72, 0, :] = SXV
    xsp[:, 73, 0, :] = SXV
    xsp = _q8(xsp.reshape(B, 74, 2 * HW))

    # conv1 weights: [c_in(128), tap, chunk, c_out] x FW1 single fp8
    w1 = np.asarray(inputs["conv1_w"], f)
    w1v9 = w1[:, :256].transpose(2, 3, 1, 0).reshape(9, 2, 128, 128)  # t,c,ci,co
    w1v = _q8(np.ascontiguousarray(
        w1v9.transpose(2, 0, 1, 3)).reshape(128, 9 * 2 * 128) * FW1)
    # spatial weights + bias rows (hi/lo of b1*FW1, moving value SXV both)
    b1 = np.asarray(inputs["conv1_b"], f)
    w1s_rows = np.zeros((74, 2, 128), f)
    w1s_rows[:72, 0, :] = w1[:, 256:].transpose(2, 3, 1, 0).reshape(72, 128) * FW1
    bh = _q8(b1 * FW1).astype(f)
    w1s_rows[72, 0, :] = bh
    w1s_rows[73, 0, :] = b1 * FW1 - bh
    w1s = _q8(w1s_rows.reshape(74, 2 * 128))

    # conv2 weights: [c_in, tap, mb, hl, c_out], hi/lo exact, x FW2
    w29 = np.asarray(inputs["conv2_w"], f).transpose(2, 3, 1, 0).reshape(9, 128, 256)
    w2s = w29 * FW2
    w2h = _q8(w2s)
    w2l = _q8(w2s - w2h.astype(f))
    w2 = np.zeros((128, 9, 2, 2, 128), NP_F8)
    for mb in range(2):
        w2[:, :, mb, 0, :] = w2h.transpose(1, 0, 2)[:, :, mb * 128 : (mb + 1) * 128]
        w2[:, :, mb, 1, :] = w2l.transpose(1, 0, 2)[:, :, mb * 128 : (mb + 1) * 128]
    w2 = w2.reshape(128, 9 * 2 * 2 * 128)

    # reduce_w: [in, 2(chunk), 300]: feature rows x FRW hi/lo; coord+bias
    # chunk separately x (SVSR/SC)
    rw = np.asarray(inputs["reduce_w"], f)     # [300, 258]
    rb = np.asarray(inputs["reduce_b"], f)
    rwT = rw.T                                  # [258, 300]
    rw_feat = np.stack([rwT[0:128], rwT[128:256]], axis=1) * FRW  # [128,2,300]
    rwh, rwl = _hilo(rw_feat)
    rw_c2 = np.zeros((3, 2, DS), f)
    c2 = np.stack([rwT[256], rwT[257], rb], axis=0) * (SVSR / SC)  # [3,300]
    c2h = _q8(c2).astype(f)
    rw_c2[:, 0, :] = c2h
    rw_c2[:, 1, :] = c2 - c2h
    rwc2 = _q8(rw_c2)

    def kq_pack(wmat, bias, f_w, f_c2):
        """wmat [258,258] torch (out,in); returns main [128,2,258] and
        c2 [3,2,258] (plane1 zeros) fp8."""
        wT = np.asarray(wmat, f).T  # [in 258, out 258]
        main = np.stack([wT[0:128], wT[128:256]], axis=1) * f_w
        c2m = np.zeros((3, 2, VF), f)
        c2m[0:2, 0, :] = wT[256:258] * f_c2
        c2m[2, 0, :] = np.asarray(bias, f) * f_c2
        return _q8(main), _q8(c2m)

    # k coord rows: (coord*SC)*(w*g) = w_contrib*(SX*FKW) -> g = SX*FKW/SC
    kw, kwc2 = kq_pack(inputs["k_w"], inputs["k_b"], FKW, SX * FKW / SC)

    # v_w hi/lo: main [128,2,258] x FVW ; c2 [3,2,258] = (h,l) planes
    vwT = np.asarray(inputs["v_w"], f).T
    vb = np.asarray(inputs["v_b"], f)
    v_feat = np.stack([vwT[0:128], vwT[128:256]], axis=1) * FVW
    vwh, vwl = _hilo(v_feat)
    vc2 = np.zeros((3, 2, VF), f)
    c2v = np.concatenate([vwT[256:258], vb[None]], axis=0) * (SX * FVW / SC)
    c2vh = _q8(c2v).astype(f)
    vc2[:, 0, :] = c2vh
    vc2[:, 1, :] = c2v - c2vh
    vwc2 = _q8(vc2)

    # q_w with mod^2 folded, per sample
    incw = np.asarray(inputs["inc_w"], f)
    incb = np.asarray(inputs["inc_b"], f)
    mod = np.maximum.reduce(txt, axis=2) @ incw.T + incb   # [B, 258]
    qwT = np.asarray(inputs["q_w"], f).T                    # [in, out]
    qb_ = np.asarray(inputs["q_b"], f)
    qw_all = np.zeros((B, 128, 2, VF), NP_F8)
    qwc2_all = np.zeros((B, 3, 2, VF), NP_F8)
    for b in range(B):
        m2 = (mod[b] ** 2)[None, :]                         # [1, out]
        qmain = np.stack([qwT[0:128], qwT[128:256]], axis=1) * (FQW * m2[:, None, :])
        qw_all[b] = _q8(qmain)
        qc2 = np.zeros((3, 2, VF), f)
        qc2[0:2, 0, :] = qwT[256:258] * (SX * FQW / SC) * m2
        qc2[2, 0, :] = qb_ * (SX * FQW / SC) * m2[0]
        qwc2_all[b] = _q8(qc2)

    # coords pair tile: rows (x, y, ones) x SC, both planes identical
    xr = np.linspace(-1.0, 1.0, 32, dtype=f)
    yy, xx = np.meshgrid(xr, xr, indexing="ij")
    coord3 = np.stack([xx.ravel(), yy.ravel(), np.ones(HW, f)]).astype(f) * SC
    coordp = _q8(np.stack([coord3, coord3], axis=1).reshape(3, 2 * HW))

    onesp = _q8(np.full((128, 2 * 128), SV, f))

    # text tensors bf16
    txtc = np.zeros((B, 128, 3, 20), f)
    txtc[:, :, 0, :] = txt[:, 0:128]
    txtc[:, :, 1, :] = txt[:, 128:256]
    txtc[:, :44, 2, :] = txt[:, 256:300]
    txtc = txtc.reshape(B, 128, 60).astype(NP_BF)
    txtT = np.ascontiguousarray(txt.transpose(0, 2, 1)).astype(NP_BF)

    # GN selectors + affine columns
    cidx = np.arange(128)
    gsel1 = np.zeros((128, 32), f)
    gsel1[cidx, cidx // 4] = 0.25
    gsel1T = np.zeros((32, 128), f)
    gsel1T[cidx // 4, cidx] = 1.0
    gsel2 = np.zeros((128, 2, 16), f)
    gsel2T = np.zeros((16, 2, 128), f)
    for mb in range(2):
        g = cidx // 8
        gsel2[cidx, mb, g] = 0.125
        gsel2T[g, mb, cidx] = 1.0
    gsel2 = gsel2.reshape(128, 32)
    gsel2T = gsel2T.reshape(16, 256)

    sv1 = np.zeros((128, 2), f)
    sv1[:, 0] = np.asarray(inputs["gn1_g"], f) * S2A
    sv1[:, 1] = np.asarray(inputs["gn1_b"], f) * S2A
    b2 = np.asarray(inputs["conv2_b"], f)
    sv2 = np.zeros((128, 6), f)
    sv2[:, 0:2] = (np.asarray(inputs["gn2_g"], f) * SX).reshape(2, 128).T
    sv2[:, 2:4] = (np.asarray(inputs["gn2_b"], f) * SX).reshape(2, 128).T
    sv2[:, 4:6] = (b2 * S2).reshape(2, 128).T

    shared = {
        "w1v": w1v, "w1s": w1s, "w2": w2,
        "rwh": rwh.reshape(128, 2 * DS), "rwl": rwl.reshape(128, 2 * DS),
        "rwc2": rwc2.reshape(3, 2 * DS),
        "kw": kw.reshape(128, 2 * VF), "kwc2": kwc2.reshape(3, 2 * VF),
        "vwh": vwh.reshape(128, 2 * VF), "vwl": vwl.reshape(128, 2 * VF),
        "vwc2": vwc2.reshape(3, 2 * VF),
        "coordp": coordp, "onesp": onesp,
        "ones20": np.ones((20, 128), NP_BF),
        "gsel1": gsel1, "gsel1T": gsel1T, "gsel2": gsel2, "gsel2T": gsel2T,
        "svec1": sv1, "svec2": sv2,
        "zpad": np.zeros((128, 34 * 34), NP_F8),
    }

    in_maps = []
    for c in range(N_CORES):
        sl = slice(c * SPC, (c + 1) * SPC)
        m = dict(shared)
        m["xv"] = np.ascontiguousarray(xv[sl])
        m["xsp"] = np.ascontiguousarray(xsp[sl])
        m["txtc"] = np.ascontiguousarray(txtc[sl])
        m["txtT"] = np.ascontiguousarray(txtT[sl])
        m["qw"] = np.ascontiguousarray(qw_all[sl].reshape(SPC, 128, 2 * VF))
        m["qwc2"] = np.ascontiguousarray(qwc2_all[sl].reshape(SPC, 3, 2 * VF))
        in_maps.append(m)
    return in_maps


def get_program():
    if "nc" not in _PROGRAM_CACHE:
        _PROGRAM_CACHE["nc"] = build_program()
    return _PROGRAM_CACHE["nc"]


def kernel(**inputs) -> np.ndarray:
    nc = get_program()
    in_maps = _prep_inputs(inputs)
    res = run_bass_kernel_spmd(nc, in_maps, list(range(N_CORES)))
    outs = [res.results[c]["out"].astype(np.float32) for c in range(N_CORES)]
    full = np.concatenate(outs, axis=0).reshape(32, 558, 32, 32)
    return full.astype(np.float32)


# revision 13
# speedup vs baseline: 1.3368x; 1.0947x over previous
"""Trainium2 Bass kernel for nn_AttentionFusion — fp8-DoubleRow rewrite.

Sharding: pure data parallelism over batch (32 samples -> 8 cores x 4
samples), weights replicated.

All heavy matmuls run as fp8e4m3 DoubleRow (2 k-tiles per instruction,
0.5 cycles/row — 4x the f32r row rate for K-chunked contractions), with
power-of-2 scale management so every tensor sits in e4m3's healthy range.
Precision placement (validated vs the jax reference, rel_l2 ~1.05e-2 vs
the 2e-2 gate):
  conv1        acts single fp8 x weights single fp8 (tap-chunk pairs)
  conv2        acts single fp8 (y1pad) x weights hi+lo exact pairs
  vsr          xfeat single fp8 x reduce_w hi+lo exact pairs
  E_t/rb/wt    bf16 (text branch is error-dominant)
  k,q          single fp8 both sides (mod folded into q_w host-side)
  v            xfeat fp8 x v_w hi+lo exact (v_w drives a systematic
               mean-activation error if quantized single)
  S'/E/wv      fp8 throughout; softmax denominators via an fp8 ones-matmul
               (rbD) + DVE reciprocal, numerator/denominator share E
  output       bf16 (cast to f32 on host)

Softmax denominators use nc.vector.reciprocal + TT-mult (no Ln/Exp chains,
no partition-move DMAs).
"""

import os
import sys
from contextlib import ExitStack

for _p in ("/opt/trn_rl_repo",):
    if _p not in sys.path and os.path.isdir(_p):
        sys.path.insert(0, _p)

import numpy as np
import ml_dtypes

import concourse.bacc as bacc
import concourse.mybir as mybir
import concourse.tile as tile
from concourse.bass import ts
from concourse.bass_utils import run_bass_kernel_spmd

F32 = mybir.dt.float32
BF16 = mybir.dt.bfloat16
FP8 = mybir.dt.float8e4
AF = mybir.ActivationFunctionType
ALU = mybir.AluOpType
DR = mybir.MatmulPerfMode.DoubleRow

NP_F8 = ml_dtypes.float8_e4m3
NP_BF = ml_dtypes.bfloat16

N_CORES = 8
SPC = 4  # samples per core
HW = 1024
DS = 300
VF = 258
EPS = 1e-5

# power-of-2 scale plan
SXV = 16.0      # conv1 input activations
FW1 = 512.0     # conv1 weights               -> y1 psum x 8192
S2A = 16.0      # y1pad storage (folded into gn1 affine)
FW2 = 512.0     # conv2 weights               -> y2 psum x 8192
SX = 16.0       # xfeat storage (folded into gn2 affine)
SC = 64.0       # coords + ones row storage
FRW = 512.0     # reduce_w                    -> vsr psum x 8192
FKW = 2048.0    # k_w                         -> k  psum x 32768
FQW = 512.0     # q_w*mod^2                   -> q' psum x 8192
FVW = 2048.0    # v_w                         -> v  psum x 32768
SKQV_EVAC = 1.0 / 512.0   # k->x64, q->x16, v->x64 storage scales
SV = 64.0       # v storage scale == ones128 value
S1 = SXV * FW1            # 8192
S2 = S2A * FW2            # 8192
SVSR = SX * FRW           # 8192
EPS1 = EPS * S1 * S1
EPS2 = EPS * S2 * S2
ESC_V = (1.0 / float(np.sqrt(VF))) / 1024.0
ESC_T = (1.0 / float(np.sqrt(DS))) / SVSR

TAPS = [(ty, tx) for ty in range(3) for tx in range(3)]

_PROGRAM_CACHE = {}


def _patch_act_tables():
    """Keep Exp/Ln/Relu/Identity/Copy pinned to one act table set so the
    act-table-load pass doesn't thrash between sets."""
    import concourse.bacc as _bacc
    import concourse.hw_specs as _hw

    if getattr(_bacc, "_act_tables_patched", False):
        return
    _orig = _hw.get_activation_tables
    mine = {AF.Exp, AF.Ln, AF.Relu, AF.Identity, AF.Copy}

    def patched(module_arch):
        tabs = _orig(module_arch)
        out = {}
        for name, funcs in tabs.items():
            if name == "natural_log_exp_and_others" or not (mine & funcs):
                out[name] = funcs
            else:
                out[name] = funcs - mine
        return out

    _bacc.get_activation_tables = patched
    _bacc._act_tables_patched = True


def _patch_drain_barrier():
    """Split the kernel-tail drain's per-proc sem waits across engines."""
    import concourse.tile as tile_mod
    from concourse.vector_clock import ScopedClock

    if getattr(tile_mod, "_drain_patched", False):
        return

    def _patched(self, tick_clock, wait_clock):
        nc = self.nc
        drain_inst = nc.sync.drain()
        wait_clock.add_sem_waits(
            drain_inst.ins, ScopedClock({None: tick_clock.global_clock})
        )
        si = drain_inst.ins.sync_info
        waits = list(si.on_wait or [])
        if len(waits) > 1:
            si.on_wait = waits[:1]
            engines = [nc.sync, nc.scalar, nc.vector, nc.tensor, nc.gpsimd]
            for i in range(1, len(waits)):
                extra = engines[i % len(engines)].drain()
                extra.ins.sync_info = mybir.SyncInfo(
                    on_wait=[waits[i]], on_update=[]
                )
        nc.all_engine_barrier()
        assert self.sems is not None
        popped = nc._tile_sem_poison_stack.pop()
        assert popped is self._sem_poison
        nc.clear_and_free_semaphores(list(self.sems.allocated().values()))
        nc.all_engine_barrier()

    tile_mod.TileContext._drain_and_barrier = _patched
    tile_mod._drain_patched = True


def build_program():
    _patch_act_tables()
    _patch_drain_barrier()
    nc = bacc.Bacc()
    dt = F32

    # ---------------- DRAM declarations ----------------
    d_xv = nc.dram_tensor("xv", [SPC, 128, 2 * 34 * 34], FP8, kind="ExternalInput")
    d_xsp = nc.dram_tensor("xsp", [SPC, 74, 2 * HW], FP8, kind="ExternalInput")
    d_txtc = nc.dram_tensor("txtc", [SPC, 128, 3 * 20], BF16, kind="ExternalInput")
    d_txtT = nc.dram_tensor("txtT", [SPC, 20, DS], BF16, kind="ExternalInput")
    d_qw = nc.dram_tensor("qw", [SPC, 128, 2 * VF], FP8, kind="ExternalInput")
    d_qwc2 = nc.dram_tensor("qwc2", [SPC, 3, 2 * VF], FP8, kind="ExternalInput")
    d_w1v = nc.dram_tensor("w1v", [128, 9 * 2 * 128], FP8, kind="ExternalInput")
    d_w1s = nc.dram_tensor("w1s", [74, 2 * 128], FP8, kind="ExternalInput")
    d_w2 = nc.dram_tensor("w2", [128, 9 * 2 * 2 * 128], FP8, kind="ExternalInput")
    d_rwh = nc.dram_tensor("rwh", [128, 2 * DS], FP8, kind="ExternalInput")
    d_rwl = nc.dram_tensor("rwl", [128, 2 * DS], FP8, kind="ExternalInput")
    d_rwc2 = nc.dram_tensor("rwc2", [3, 2 * DS], FP8, kind="ExternalInput")
    d_kw = nc.dram_tensor("kw", [128, 2 * VF], FP8, kind="ExternalInput")
    d_kwc2 = nc.dram_tensor("kwc2", [3, 2 * VF], FP8, kind="ExternalInput")
    d_vwh = nc.dram_tensor("vwh", [128, 2 * VF], FP8, kind="ExternalInput")
    d_vwl = nc.dram_tensor("vwl", [128, 2 * VF], FP8, kind="ExternalInput")
    d_vwc2 = nc.dram_tensor("vwc2", [3, 2 * VF], FP8, kind="ExternalInput")
    d_coordp = nc.dram_tensor("coordp", [3, 2 * HW], FP8, kind="ExternalInput")
    d_onesp = nc.dram_tensor("onesp", [128, 2 * 128], FP8, kind="ExternalInput")
    d_ones20 = nc.dram_tensor("ones20", [20, 128], BF16, kind="ExternalInput")
    d_gs1 = nc.dram_tensor("gsel1", [128, 32], dt, kind="ExternalInput")
    d_gs1T = nc.dram_tensor("gsel1T", [32, 128], dt, kind="ExternalInput")
    d_gs2 = nc.dram_tensor("gsel2", [128, 2 * 16], dt, kind="ExternalInput")
    d_gs2T = nc.dram_tensor("gsel2T", [16, 2 * 128], dt, kind="ExternalInput")
    d_sv1 = nc.dram_tensor("svec1", [128, 2], dt, kind="ExternalInput")
    d_sv2 = nc.dram_tensor("svec2", [128, 6], dt, kind="ExternalInput")
    d_zpad = nc.dram_tensor("zpad", [128, 34 * 34], FP8, kind="ExternalInput")
    d_out = nc.dram_tensor("out", [SPC, 558, HW], BF16, kind="ExternalOutput")

    with tile.TileContext(nc) as tc, ExitStack() as ctx:
        wpool = ctx.enter_context(tc.tile_pool(name="weights", bufs=1))
        inpool = ctx.enter_context(tc.tile_pool(name="inputs", bufs=2))
        spool = ctx.enter_context(tc.tile_pool(name="work", bufs=1))
        opool = ctx.enter_context(tc.tile_pool(name="outs", bufs=1))
        epool = ctx.enter_context(tc.tile_pool(name="estream", bufs=3))
        pA = ctx.enter_context(tc.tile_pool(name="pA", bufs=1, space="PSUM"))
        pB = ctx.enter_context(tc.tile_pool(name="pB", bufs=1, space="PSUM"))
        pS = ctx.enter_context(tc.tile_pool(name="pS", bufs=2, space="PSUM"))
        pC = ctx.enter_context(tc.tile_pool(name="pC", bufs=2, space="PSUM"))

        # ---------- conv1 weights + sample-0 inputs first ----------
        sv1 = wpool.tile([128, 2], dt, name="sv1")
        nc.sync.dma_start(sv1[:, :], d_sv1[:, :])
        g1_ap, b1_ap = sv1[:, 0:1], sv1[:, 1:2]
        w1v_sb = wpool.tile([128, 9, 2, 128], FP8, name="w1v_sb")
        nc.sync.dma_start(
            w1v_sb[:, :, :, :].rearrange("p a b c -> p (a b c)"), d_w1v[:, :]
        )
        w1s_sb = wpool.tile([74, 2, 128], FP8, name="w1s_sb")
        nc.sync.dma_start(
            w1s_sb[:, :, :].rearrange("p a b -> p (a b)"), d_w1s[:, :]
        )
        gs1_sb = wpool.tile([128, 32], dt, name="gs1_sb")
        nc.sync.dma_start(gs1_sb[:, :], d_gs1[:, :])
        gs1T_sb = wpool.tile([32, 128], dt, name="gs1T_sb")
        nc.sync.dma_start(gs1T_sb[:, :], d_gs1T[:, :])

        def load_inputs(s):
            xv = inpool.tile([128, 2, 34, 34], FP8, name="xv")
            nc.sync.dma_start(
                xv[:, :, :, :].rearrange("p a h w -> p (a h w)"), d_xv[s]
            )
            xsp = inpool.tile([74, 2, HW], FP8, name="xsp")
            nc.sync.dma_start(
                xsp[:, :, :].rearrange("p a b -> p (a b)"), d_xsp[s]
            )
            txtc = inpool.tile([128, 3, 20], BF16, name="txtc")
            nc.sync.dma_start(
                txtc[:, :, :].rearrange("p a b -> p (a b)"), d_txtc[s]
            )
            txtT = inpool.tile([20, DS], BF16, name="txtT")
            nc.sync.dma_start(txtT[:, :], d_txtT[s])
            qw = inpool.tile([128, 2, VF], FP8, name="qw")
            nc.sync.dma_start(
                qw[:, :, :].rearrange("p a b -> p (a b)"), d_qw[s]
            )
            qwc2 = inpool.tile([3, 2, VF], FP8, name="qwc2")
            nc.sync.dma_start(
                qwc2[:, :, :].rearrange("p a b -> p (a b)"), d_qwc2[s]
            )
            return dict(xv=xv, xsp=xsp, txtc=txtc, txtT=txtT, qw=qw, qwc2=qwc2)

        preloaded = load_inputs(0)

        # ---------- remaining weights ----------
        w2_sb = wpool.tile([128, 9, 2, 2, 128], FP8, name="w2_sb")
        nc.sync.dma_start(
            w2_sb[:, :, :, :, :].rearrange("p a b c d -> p (a b c d)"), d_w2[:, :]
        )
        sv2 = wpool.tile([128, 6], dt, name="sv2")
        nc.sync.dma_start(sv2[:, :], d_sv2[:, :])
        g2_ap, b2_ap, b2s_ap = sv2[:, 0:2], sv2[:, 2:4], sv2[:, 4:6]
        gs2_sb = wpool.tile([128, 2, 16], dt, name="gs2_sb")
        nc.sync.dma_start(
            gs2_sb[:, :, :].rearrange("p a b -> p (a b)"), d_gs2[:, :]
        )
        gs2T_sb = wpool.tile([16, 2, 128], dt, name="gs2T_sb")
        nc.sync.dma_start(
            gs2T_sb[:, :, :].rearrange("p a b -> p (a b)"), d_gs2T[:, :]
        )
        rwh_sb = wpool.tile([128, 2, DS], FP8, name="rwh_sb")
        nc.sync.dma_start(rwh_sb[:, :, :].rearrange("p a b -> p (a b)"), d_rwh[:, :])
        rwl_sb = wpool.tile([128, 2, DS], FP8, name="rwl_sb")
        nc.sync.dma_start(rwl_sb[:, :, :].rearrange("p a b -> p (a b)"), d_rwl[:, :])
        rwc2_sb = wpool.tile([3, 2, DS], FP8, name="rwc2_sb")
        nc.sync.dma_start(rwc2_sb[:, :, :].rearrange("p a b -> p (a b)"), d_rwc2[:, :])
        kw_sb = wpool.tile([128, 2, VF], FP8, name="kw_sb")
        nc.sync.dma_start(kw_sb[:, :, :].rearrange("p a b -> p (a b)"), d_kw[:, :])
        kwc2_sb = wpool.tile([3, 2, VF], FP8, name="kwc2_sb")
        nc.sync.dma_start(kwc2_sb[:, :, :].rearrange("p a b -> p (a b)"), d_kwc2[:, :])
        vwh_sb = wpool.tile([128, 2, VF], FP8, name="vwh_sb")
        nc.sync.dma_start(vwh_sb[:, :, :].rearrange("p a b -> p (a b)"), d_vwh[:, :])
        vwl_sb = wpool.tile([128, 2, VF], FP8, name="vwl_sb")
        nc.sync.dma_start(vwl_sb[:, :, :].rearrange("p a b -> p (a b)"), d_vwl[:, :])
        vwc2_sb = wpool.tile([3, 2, VF], FP8, name="vwc2_sb")
        nc.sync.dma_start(vwc2_sb[:, :, :].rearrange("p a b -> p (a b)"), d_vwc2[:, :])
        coordp_sb = wpool.tile([3, 2, HW], FP8, name="coordp_sb")
        nc.sync.dma_start(
            coordp_sb[:, :, :].rearrange("p a b -> p (a b)"), d_coordp[:, :]
        )
        onesp_sb = wpool.tile([128, 2, 128], FP8, name="onesp_sb")
        nc.sync.dma_start(
            onesp_sb[:, :, :].rearrange("p a b -> p (a b)"), d_onesp[:, :]
        )
        ones20_sb = wpool.tile([20, 128], BF16, name="ones20_sb")
        nc.sync.dma_start(ones20_sb[:, :], d_ones20[:, :])

        # persistent zero-padded conv2 input (border stays zero forever)
        y1pad = wpool.tile([128, 1, 34, 34], FP8, name="y1pad")
        nc.sync.dma_start(
            y1pad[:, :, :, :].rearrange("p a h w -> p (a h w)"), d_zpad[:, :]
        )
        # persistent k/q tail tiles; plane 1 must stay zero (S' tail pairs)
        ktail = wpool.tile([2, 2, HW], FP8, name="ktail")
        nc.vector.memset(ktail[:, :, :].rearrange("p a b -> p (a b)"), 0)
        qtail = wpool.tile([2, 2, HW], FP8, name="qtail")
        nc.vector.memset(qtail[:, :, :].rearrange("p a b -> p (a b)"), 0)

        # ---------------- helpers ----------------
        def group_norm_finish(gstat_ps, cb_ps, gamma_ap, beta_ap, gsT_ap,
                              eps_s, groups, tag, bias_col=None):
            """gstat_ps: [G,2] PSUM (mean, E[x2]) per group (scaled domain);
            cb_ps: [128,2] PSUM for the broadcast-back.  Returns sc [128,2]
            SBUF: col0 = scale, col1 = bias for act(relu, psum-input).
            bias_col: [128,1] host column of b*S to subtract from the
            broadcast channel mean (act input psum is un-biased)."""
            gb = spool.tile([groups, 4], dt, name=f"gb_{tag}")
            nc.vector.tensor_copy(gb[:, 0:1], gstat_ps[:, 0:1])
            nc.vector.tensor_tensor(gb[:, 3:4], gb[:, 0:1], gb[:, 0:1], ALU.mult)
            nc.vector.tensor_tensor(
                gb[:, 1:2], gstat_ps[:, 1:2], gb[:, 3:4], ALU.subtract
            )
            nc.vector.tensor_scalar_add(gb[:, 1:2], gb[:, 1:2], float(eps_s))
            nc.scalar.activation(gb[:, 2:3], gb[:, 1:2], AF.Ln)
            nc.scalar.activation(gb[:, 1:2], gb[:, 2:3], AF.Exp, scale=-0.5)
            nc.tensor.matmul(cb_ps, gsT_ap, gb[:, 0:2], start=True, stop=True)
            sc = spool.tile([128, 3], dt, name=f"sc_{tag}")
            nc.vector.tensor_tensor(sc[:, 0:1], gamma_ap, cb_ps[:, 1:2], ALU.mult)
            if bias_col is not None:
                nc.vector.tensor_tensor(
                    sc[:, 2:3], cb_ps[:, 0:1], bias_col, ALU.subtract
                )
                mu_ap = sc[:, 2:3]
            else:
                mu_ap = cb_ps[:, 0:1]
            nc.vector.tensor_tensor(sc[:, 1:2], mu_ap, sc[:, 0:1], ALU.mult)
            nc.vector.tensor_tensor(sc[:, 1:2], beta_ap, sc[:, 1:2], ALU.subtract)
            return sc

        def channel_stats(ps_a, ps_b, tag, bias_col=None):
            """Two [128,512] PSUM halves -> st2 [128,2] = (mean_b, E_b[x^2])."""
            bnst = spool.tile([128, 2, 6], dt, name=f"bnst_{tag}")
            nc.vector.bn_stats(bnst[:, 0, :], ps_a)
            nc.vector.bn_stats(bnst[:, 1, :], ps_b)
            mv = spool.tile([128, 2], dt, name=f"mv_{tag}")
            nc.vector.bn_aggr(mv[:, :], bnst[:, :, :])
            st2 = spool.tile([128, 2], dt, name=f"st2_{tag}")
            if bias_col is not None:
                nc.vector.tensor_tensor(st2[:, 0:1], mv[:, 0:1], bias_col, ALU.add)
            else:
                nc.vector.tensor_copy(st2[:, 0:1], mv[:, 0:1])
            nc.vector.tensor_tensor(st2[:, 1:2], st2[:, 0:1], st2[:, 0:1], ALU.mult)
            nc.vector.tensor_tensor(st2[:, 1:2], st2[:, 1:2], mv[:, 1:2], ALU.add)
            return st2

        # ---------------- attention, jp-granular for interleaving ----------
        def attn_mm(sd, s, ni, jps):
            """S' + exp + wv accumulation for jp groups of one n-half."""
            kT, qT, vsb = sd["kT"], sd["qT"], sd["vsb"]
            if 0 in jps:
                sd["wv01"] = pA.tile([128, 1024], F32, tag="attn",
                                     name=f"wv01_{s}_{ni}")
                sd["wvD"] = pB.tile([128, 1024], F32, tag="attn",
                                    name=f"wvD_{s}_{ni}")
            wv01, wvD = sd["wv01"], sd["wvD"]
            for jp in jps:
                Ep = epool.tile([128, 2, 512], FP8, tag="E", name=f"E_{s}_{ni}_{jp}")
                for jj in range(2):
                    j = 2 * jp + jj
                    sps = pS.tile([128, 512], F32, tag="ps", name=f"sps_{s}_{ni}_{j}")
                    nc.tensor.matmul(
                        sps[:, :], qT[:, :, ts(j, 128)], kT[:, :, ts(ni, 512)],
                        start=True, stop=False, perf_mode=DR,
                    )
                    nc.tensor.matmul(
                        sps[:, :], qtail[:, :, ts(j, 128)], ktail[:, :, ts(ni, 512)],
                        start=False, stop=True, perf_mode=DR,
                    )
                    nc.scalar.activation(Ep[:, jj, :], sps[:, :], AF.Exp, scale=ESC_V)
                st, sp = (jp == 0), (jp == 3)
                nc.tensor.matmul(
                    wv01[:, 0:512], vsb[:, jp, :, 0:128], Ep[:, :, :],
                    start=st, stop=sp, perf_mode=DR,
                )
                nc.tensor.matmul(
                    wv01[:, 512:1024], vsb[:, jp, :, 128:256], Ep[:, :, :],
                    start=st, stop=sp, perf_mode=DR,
                )
                nc.tensor.matmul(
                    wvD[0:2, 512:1024], vsb[:, jp, :, 256:258], Ep[:, :, :],
                    start=st, stop=sp, perf_mode=DR,
                )
                nc.tensor.matmul(
                    wvD[:, 0:512], onesp_sb[:, :, :], Ep[:, :, :],
                    start=st, stop=sp, perf_mode=DR,
                )

        def attn_fin(sd, s, ni):
            wv01, wvD, wvout = sd["wv01"], sd["wvD"], sd["wvout"]
            rbc = spool.tile([128, 512], dt, name=f"rbcv_{ni}")
            nc.vector.reciprocal(rbc[:, :], wvD[:, 0:512])
            nc.vector.tensor_tensor(
                wvout[:, 0, ts(ni, 512)], wv01[:, 0:512], rbc[:, :], ALU.mult
            )
            nc.vector.tensor_tensor(
                wvout[:, 1, ts(ni, 512)], wv01[:, 512:1024], rbc[:, :], ALU.mult
            )
            nc.vector.tensor_tensor(
                wvout[0:2, 2, ts(ni, 512)], wvD[0:2, 512:1024], rbc[0:2, :], ALU.mult
            )

        def attn_out_dma(sd, s):
            wvout = sd["wvout"]
            nc.gpsimd.dma_start(d_out[s, 0:128, :], wvout[:, 0, :])
            nc.gpsimd.dma_start(d_out[s, 128:256, :], wvout[:, 1, :])
            nc.gpsimd.dma_start(d_out[s, 256:258, :], wvout[:2, 2, :])

        # ---------------- conv helpers ----------------
        def emit_conv1(s, io):
            xv, xsp = io["xv"], io["xsp"]
            pss = []
            for ni in range(2):
                ps = pC.tile([128, 512], F32, tag="conv", name=f"c1ps_{s}_{ni}")
                h0 = ni * 16
                for t, (ty, tx) in enumerate(TAPS):
                    nc.tensor.matmul(
                        ps[:, :],
                        w1v_sb[:, t, :, :],
                        xv[:, :, ty + h0 : ty + h0 + 16, tx : tx + 32],
                        start=(t == 0), stop=False, perf_mode=DR,
                    )
                nc.tensor.matmul(
                    ps[:, :], w1s_sb[:, :, :], xsp[:, :, ts(ni, 512)],
                    start=False, stop=True, perf_mode=DR,
                )
                pss.append(ps)
            return pss

        def emit_conv2_mb(s, mb):
            ps2 = []
            for ni in range(2):
                ps = pC.tile([128, 512], F32, tag="conv", name=f"c2ps_{s}_{mb}_{ni}")
                h0 = ni * 16
                for t, (ty, tx) in enumerate(TAPS):
                    nc.tensor.matmul(
                        ps[:, :],
                        w2_sb[:, t, mb, :, :],
                        y1pad[:, 0:1, ty + h0 : ty + h0 + 16, tx : tx + 32]
                        .to_broadcast((128, 2, 16, 32)),
                        start=(t == 0), stop=(t == 8), perf_mode=DR,
                    )
                ps2.append(ps)
            return ps2

        def gn2_finish_relu(s, mb, ps2, xfeat):
            st2b = channel_stats(
                ps2[0][:, :], ps2[1][:, :], f"gn2_{s}_{mb}",
                bias_col=b2s_ap[:, mb : mb + 1],
            )
            gt2 = pS.tile([128, 4], dt, tag="ps", name=f"gst2_{s}_{mb}")
            nc.tensor.matmul(
                gt2[:16, 0:2], gs2_sb[:, mb, :], st2b[:, :], start=True, stop=True
            )
            sc2 = group_norm_finish(
                gt2[:16, 0:2], gt2[:, 2:4],
                g2_ap[:, mb : mb + 1], b2_ap[:, mb : mb + 1],
                gs2T_sb[:, mb, :], EPS2, 16, f"gn2_{s}_{mb}",
                bias_col=b2s_ap[:, mb : mb + 1],
            )
            for ni in range(2):
                nc.scalar.activation(
                    xfeat[:, mb, ts(ni, 512)], ps2[ni][:, :],
                    AF.Relu, bias=sc2[:, 1:2], scale=sc2[:, 0:1],
                )

        # ---------------- per-sample pipeline ----------------
        prev = None
        ios = {0: preloaded}
        for s in range(SPC):
            io = ios.pop(s)
            if s + 1 < SPC:
                ios[s + 1] = load_inputs(s + 1)
            txtc, txtT, qw_s, qwc2_s = io["txtc"], io["txtT"], io["qw"], io["qwc2"]

            ps1 = emit_conv1(s, io)
            st2 = channel_stats(ps1[0][:, :], ps1[1][:, :], f"gn1_{s}")
            if prev is not None:
                attn_mm(prev, s - 1, 0, [0])
            gt1 = pS.tile([128, 4], dt, tag="ps", name=f"gst1_{s}")
            nc.tensor.matmul(
                gt1[:32, 0:2], gs1_sb[:, :], st2[:, :], start=True, stop=True
            )
            sc1 = group_norm_finish(
                gt1[:32, 0:2], gt1[:, 2:4],
                g1_ap, b1_ap, gs1T_sb[:, :], EPS1, 32, f"gn1_{s}",
            )
            for ni in range(2):
                nc.scalar.activation(
                    y1pad[:, 0, 1 + ni * 16 : 17 + ni * 16, 1:33],
                    ps1[ni][:, :].rearrange("p (h w) -> p h w", h=16),
                    AF.Relu, bias=sc1[:, 1:2], scale=sc1[:, 0:1],
                )
            if prev is not None:
                attn_mm(prev, s - 1, 0, [1, 2, 3])
                attn_fin(prev, s - 1, 0)

            # -------- conv2 (W hi/lo pairs, stride-0 moving) --------
            xfeat = spool.tile([128, 2, HW], FP8, name="xfeat")
            ps2a = emit_conv2_mb(s, 0)
            if prev is not None:
                attn_mm(prev, s - 1, 1, [0, 1])
            gn2_finish_relu(s, 0, ps2a, xfeat)
            ps2b = emit_conv2_mb(s, 1)
            if prev is not None:
                attn_mm(prev, s - 1, 1, [2, 3])
                attn_fin(prev, s - 1, 1)
                attn_out_dma(prev, s - 1)
            gn2_finish_relu(s, 1, ps2b, xfeat)

            # -------- vsr^T [300, 1024] bf16 (rw hi/lo pairs) --------
            vsrT = spool.tile([128, 3, HW], BF16, name="vsrT")
            DSCH = (128, 128, 44)
            for mb in range(3):
                mr = DSCH[mb]
                m0 = mb * 128
                for ni in range(2):
                    ps = pC.tile([128, 512], F32, tag="conv", name=f"vsr_{s}_{mb}_{ni}")
                    nc.tensor.matmul(
                        ps[:mr, :], rwh_sb[:, :, m0 : m0 + mr],
                        xfeat[:, :, ts(ni, 512)],
                        start=True, stop=False, perf_mode=DR,
                    )
                    nc.tensor.matmul(
                        ps[:mr, :], rwl_sb[:, :, m0 : m0 + mr],
                        xfeat[:, :, ts(ni, 512)],
                        start=False, stop=False, perf_mode=DR,
                    )
                    nc.tensor.matmul(
                        ps[:mr, :], rwc2_sb[:, :, m0 : m0 + mr],
                        coordp_sb[:, :, ts(ni, 512)],
                        start=False, stop=True, perf_mode=DR,
                    )
                    if (mb * 2 + ni) % 2 == 0:
                        nc.vector.tensor_copy(vsrT[:mr, mb, ts(ni, 512)], ps[:mr, :])
                    else:
                        nc.scalar.activation(
                            vsrT[:mr, mb, ts(ni, 512)], ps[:mr, :], AF.Copy
                        )

            # -------- text cross attention (bf16) --------
            E_t = spool.tile([20, HW], BF16, name="E_t")
            for ni in range(2):
                ps = pS.tile([128, 512], F32, tag="ps", name=f"et_{s}_{ni}")
                for kc in range(3):
                    kr = DSCH[kc]
                    nc.tensor.matmul(
                        ps[:20, :], txtc[:kr, kc, :], vsrT[:kr, kc, ts(ni, 512)],
                        start=(kc == 0), stop=(kc == 2),
                    )
                nc.scalar.activation(
                    E_t[:, ts(ni, 512)], ps[:20, :], AF.Exp, scale=ESC_T
                )
            rbc_t = spool.tile([128, HW], dt, name="rbc_t")
            for ni in range(2):
                ps = pS.tile([128, 512], F32, tag="ps", name=f"rbt_{s}_{ni}")
                nc.tensor.matmul(
                    ps[:, :], ones20_sb[:, :], E_t[:, ts(ni, 512)],
                    start=True, stop=True,
                )
                nc.vector.reciprocal(rbc_t[:, ts(ni, 512)], ps[:, :])
            wtout = opool.tile([128, 3, HW], BF16, name="wtout")
            WT_ROWS = (128, 128, 44)
            for mb in range(3):
                for ni in range(2):
                    ps = pC.tile(
                        [128, 512], F32, tag="conv", name=f"wt_{s}_{mb}_{ni}"
                    )
                    nc.tensor.matmul(
                        ps[: WT_ROWS[mb], :],
                        txtT[:, mb * 128 : mb * 128 + WT_ROWS[mb]],
                        E_t[:, ts(ni, 512)],
                        start=True, stop=True,
                    )
                    nc.vector.tensor_tensor(
                        wtout[: WT_ROWS[mb], mb, ts(ni, 512)],
                        ps[: WT_ROWS[mb], :],
                        rbc_t[: WT_ROWS[mb], ts(ni, 512)],
                        ALU.mult,
                    )
            nc.gpsimd.dma_start(d_out[s, 258:386, :], wtout[:, 0, :])
            nc.gpsimd.dma_start(d_out[s, 386:514, :], wtout[:, 1, :])
            nc.gpsimd.dma_start(d_out[s, 514:558, :], wtout[:44, 2, :])

            # -------- k^T, q^T fp8 (single-fp8 both sides) --------
            kT = spool.tile([128, 2, HW], FP8, name="kT")
            qT = spool.tile([128, 2, HW], FP8, name="qT")
            for wi, (w_sb, wc2_sb, dstT) in enumerate(
                ((kw_sb, kwc2_sb, kT), (qw_s, qwc2_s, qT))
            ):
                for mb in range(2):
                    for ni in range(2):
                        ps = pC.tile(
                            [128, 512], F32, tag="conv", name=f"kq_{s}_{wi}_{mb}_{ni}"
                        )
                        nc.tensor.matmul(
                            ps[:, :], w_sb[:, :, ts(mb, 128)],
                            xfeat[:, :, ts(ni, 512)],
                            start=True, stop=False, perf_mode=DR,
                        )
                        nc.tensor.matmul(
                            ps[:, :], wc2_sb[:, :, ts(mb, 128)],
                            coordp_sb[:, :, ts(ni, 512)],
                            start=False, stop=True, perf_mode=DR,
                        )
                        if (wi * 4 + mb * 2 + ni) % 2 == 0:
                            nc.scalar.activation(
                                dstT[:, mb, ts(ni, 512)], ps[:, :], AF.Copy,
                                scale=SKQV_EVAC,
                            )
                        else:
                            nc.vector.tensor_scalar_mul(
                                dstT[:, mb, ts(ni, 512)], ps[:, :], SKQV_EVAC
                            )
            # tails (output cols 256,257 of k and q)
            pkt = pA.tile([2, 1024], F32, tag="attn", name=f"ktp_{s}")
            pqt = pB.tile([2, 1024], F32, tag="attn", name=f"qtp_{s}")
            for ni in range(2):
                nc.tensor.matmul(
                    pkt[:, ts(ni, 512)], kw_sb[:, :, 256:258],
                    xfeat[:, :, ts(ni, 512)],
                    start=True, stop=False, perf_mode=DR,
                )
                nc.tensor.matmul(
                    pkt[:, ts(ni, 512)], kwc2_sb[:, :, 256:258],
                    coordp_sb[:, :, ts(ni, 512)],
                    start=False, stop=True, perf_mode=DR,
                )
                nc.tensor.matmul(
                    pqt[:, ts(ni, 512)], qw_s[:, :, 256:258],
                    xfeat[:, :, ts(ni, 512)],
                    start=True, stop=False, perf_mode=DR,
                )
                nc.tensor.matmul(
                    pqt[:, ts(ni, 512)], qwc2_s[:, :, 256:258],
                    coordp_sb[:, :, ts(ni, 512)],
                    start=False, stop=True, perf_mode=DR,
                )
            nc.vector.tensor_scalar_mul(ktail[:, 0, :], pkt[:, :], SKQV_EVAC)
            nc.scalar.activation(
                qtail[:, 0, :], pqt[:, :], AF.Copy, scale=SKQV_EVAC
            )

            # -------- v [1024, 258] fp8 (vw hi/lo pairs) --------
            vsb = spool.tile([128, 4, 2, VF], FP8, name="vsb")
            for j in range(8):
                ps = pC.tile([128, VF], F32, tag="conv", name=f"v_{s}_{j}")
                nc.tensor.matmul(
                    ps[:, :], xfeat[:, :, ts(j, 128)], vwh_sb[:, :, :],
                    start=True, stop=False, perf_mode=DR,
                )
                nc.tensor.matmul(
                    ps[:, :], xfeat[:, :, ts(j, 128)], vwl_sb[:, :, :],
                    start=False, stop=False, perf_mode=DR,
                )
                nc.tensor.matmul(
                    ps[:, :], coordp_sb[:, :, ts(j, 128)], vwc2_sb[:, :, :],
                    start=False, stop=True, perf_mode=DR,
                )
                if j % 2 == 0:
                    nc.vector.tensor_scalar_mul(
                        vsb[:, j // 2, j % 2, :], ps[:, :], SKQV_EVAC
                    )
                else:
                    nc.scalar.activation(
                        vsb[:, j // 2, j % 2, :], ps[:, :], AF.Copy,
                        scale=SKQV_EVAC,
                    )

            prev = {
                "kT": kT, "qT": qT, "vsb": vsb,
                "wvout": opool.tile([128, 3, HW], BF16, name="wvout"),
            }
            if s == SPC - 1:
                attn_mm(prev, s, 0, [0, 1, 2, 3])
                attn_fin(prev, s, 0)
                attn_mm(prev, s, 1, [0, 1, 2, 3])
                attn_fin(prev, s, 1)
                attn_out_dma(prev, s)

    nc.finalize()
    return nc


def _q8(x):
    return np.asarray(x, np.float32).astype(NP_F8)


def _hilo(x):
    h = _q8(x)
    l = _q8(np.asarray(x, np.float32) - h.astype(np.float32))
    return h, l


def _prep_inputs(inputs):
    """Host-side marshalling: shard over batch, scale + quantize weights,
    im2col the spatial channels, fold mod^2 into q_w, hi/lo-split the
    error-critical weights."""
    f = np.float32
    video = np.asarray(inputs["video_feat"], f)
    spat = np.asarray(inputs["spatial_feat"], f)
    txt = np.asarray(inputs["txt"], f)
    B = video.shape[0]

    # conv1 inputs: video padded, x SXV, fp8, partition-major [128, 2, 1156]
    xv = np.zeros((B, 256, 34, 34), f)
    xv[:, :, 1:33, 1:33] = video * SXV
    xv = _q8(np.ascontiguousarray(
        xv.reshape(B, 2, 128, 34 * 34).transpose(0, 2, 1, 3)
    ).reshape(B, 128, 2 * 34 * 34))

    # spatial: host im2col (9 taps x 8 ch = 72 rows) + 2 bias-ones rows
    sp_pad = np.zeros((B, 8, 34, 34), f)
    sp_pad[:, :, 1:33, 1:33] = spat * SXV
    xsp_v = np.stack(
        [sp_pad[:, :, ty : ty + 32, tx : tx + 32] for (ty, tx) in TAPS], axis=1
    ).reshape(B, 72, HW)
    xsp = np.zeros((B, 74, 2, HW), f)
    xsp[:, :72, 0, :] = xsp_v
    xsp[:, 72, 0, :] = SXV
    xsp[:, 73, 0, :] = SXV
    xsp = _q8(xsp.reshape(B, 74, 2 * HW))

    # conv1 weights: [c_in(128), tap, chunk, c_out] x FW1 single fp8
    w1 = np.asarray(inputs["conv1_w"], f)
    w1v9 = w1[:, :256].transpose(2, 3, 1, 0).reshape(9, 2, 128, 128)  # t,c,ci,co
    w1v = _q8(np.ascontiguousarray(
        w1v9.transpose(2, 0, 1, 3)).reshape(128, 9 * 2 * 128) * FW1)
    # spatial weights + bias rows (hi/lo of b1*FW1, moving value SXV both)
    b1 = np.asarray(inputs["conv1_b"], f)
    w1s_rows = np.zeros((74, 2, 128), f)
    w1s_rows[:72, 0, :] = w1[:, 256:].transpose(2, 3, 1, 0).reshape(72, 128) * FW1
    bh = _q8(b1 * FW1).astype(f)
    w1s_rows[72, 0, :] = bh
    w1s_rows[73, 0, :] = b1 * FW1 - bh
    w1s = _q8(w1s_rows.reshape(74, 2 * 128))

    # conv2 weights: [c_in, tap, mb, hl, c_out], hi/lo exact, x FW2
    w29 = np.asarray(inputs["conv2_w"], f).transpose(2, 3, 1, 0).reshape(9, 128, 256)
    w2s = w29 * FW2
    w2h = _q8(w2s)
    w2l = _q8(w2s - w2h.astype(f))
    w2 = np.zeros((128, 9, 2, 2, 128), NP_F8)
    for mb in range(2):
        w2[:, :, mb, 0, :] = w2h.transpose(1, 0, 2)[:, :, mb * 128 : (mb + 1) * 128]
        w2[:, :, mb, 1, :] = w2l.transpose(1, 0, 2)[:, :, mb * 128 : (mb + 1) * 128]
    w2 = w2.reshape(128, 9 * 2 * 2 * 128)

    # reduce_w: [in, 2(chunk), 300]: feature rows x FRW hi/lo; coord+bias
    # chunk separately x (SVSR/SC)
    rw = np.asarray(inputs["reduce_w"], f)     # [300, 258]
    rb = np.asarray(inputs["reduce_b"], f)
    rwT = rw.T                                  # [258, 300]
    rw_feat = np.stack([rwT[0:128], rwT[128:256]], axis=1) * FRW  # [128,2,300]
    rwh, rwl = _hilo(rw_feat)
    rw_c2 = np.zeros((3, 2, DS), f)
    c2 = np.stack([rwT[256], rwT[257], rb], axis=0) * (SVSR / SC)  # [3,300]
    c2h = _q8(c2).astype(f)
    rw_c2[:, 0, :] = c2h
    rw_c2[:, 1, :] = c2 - c2h
    rwc2 = _q8(rw_c2)

    def kq_pack(wmat, bias, f_w, f_c2):
        """wmat [258,258] torch (out,in); returns main [128,2,258] and
        c2 [3,2,258] (plane1 zeros) fp8."""
        wT = np.asarray(wmat, f).T  # [in 258, out 258]
        main = np.stack([wT[0:128], wT[128:256]], axis=1) * f_w
        c2m = np.zeros((3, 2, VF), f)
        c2m[0:2, 0, :] = wT[256:258] * f_c2
        c2m[2, 0, :] = np.asarray(bias, f) * f_c2
        return _q8(main), _q8(c2m)

    # k coord rows: (coord*SC)*(w*g) = w_contrib*(SX*FKW) -> g = SX*FKW/SC
    kw, kwc2 = kq_pack(inputs["k_w"], inputs["k_b"], FKW, SX * FKW / SC)

    # v_w hi/lo: main [128,2,258] x FVW ; c2 [3,2,258] = (h,l) planes
    vwT = np.asarray(inputs["v_w"], f).T
    vb = np.asarray(inputs["v_b"], f)
    v_feat = np.stack([vwT[0:128], vwT[128:256]], axis=1) * FVW
    vwh, vwl = _hilo(v_feat)
    vc2 = np.zeros((3, 2, VF), f)
    c2v = np.concatenate([vwT[256:258], vb[None]], axis=0) * (SX * FVW / SC)
    c2vh = _q8(c2v).astype(f)
    vc2[:, 0, :] = c2vh
    vc2[:, 1, :] = c2v - c2vh
    vwc2 = _q8(vc2)

    # q_w with mod^2 folded, per sample
    incw = np.asarray(inputs["inc_w"], f)
    incb = np.asarray(inputs["inc_b"], f)
    mod = np.maximum.reduce(txt, axis=2) @ incw.T + incb   # [B, 258]
    qwT = np.asarray(inputs["q_w"], f).T                    # [in, out]
    qb_ = np.asarray(inputs["q_b"], f)
    qw_all = np.zeros((B, 128, 2, VF), NP_F8)
    qwc2_all = np.zeros((B, 3, 2, VF), NP_F8)
    for b in range(B):
        m2 = (mod[b] ** 2)[None, :]                         # [1, out]
        qmain = np.stack([qwT[0:128], qwT[128:256]], axis=1) * (FQW * m2[:, None, :])
        qw_all[b] = _q8(qmain)
        qc2 = np.zeros((3, 2, VF), f)
        qc2[0:2, 0, :] = qwT[256:258] * (SX * FQW / SC) * m2
        qc2[2, 0, :] = qb_ * (SX * FQW / SC) * m2[0]
        qwc2_all[b] = _q8(qc2)

    # coords pair tile: rows (x, y, ones) x SC, both planes identical
    xr = np.linspace(-1.0, 1.0, 32, dtype=f)
    yy, xx = np.meshgrid(xr, xr, indexing="ij")
    coord3 = np.stack([xx.ravel(), yy.ravel(), np.ones(HW, f)]).astype(f) * SC
    coordp = _q8(np.stack([coord3, coord3], axis=1).reshape(3, 2 * HW))

    onesp = _q8(np.full((128, 2 * 128), SV, f))

    # text tensors bf16
    txtc = np.zeros((B, 128, 3, 20), f)
    txtc[:, :, 0, :] = txt[:, 0:128]
    txtc[:, :, 1, :] = txt[:, 128:256]
    txtc[:, :44, 2, :] = txt[:, 256:300]
    txtc = txtc.reshape(B, 128, 60).astype(NP_BF)
    txtT = np.ascontiguousarray(txt.transpose(0, 2, 1)).astype(NP_BF)

    # GN selectors + affine columns
    cidx = np.arange(128)
    gsel1 = np.zeros((128, 32), f)
    gsel1[cidx, cidx // 4] = 0.25
    gsel1T = np.zeros((32, 128), f)
    gsel1T[cidx // 4, cidx] = 1.0
    gsel2 = np.zeros((128, 2, 16), f)
    gsel2T = np.zeros((16, 2, 128), f)
    for mb in range(2):
        g = cidx // 8
        gsel2[cidx, mb, g] = 0.125
        gsel2T[g, mb, cidx] = 1.0
    gsel2 = gsel2.reshape(128, 32)
    gsel2T = gsel2T.reshape(16, 256)

    sv1 = np.zeros((128, 2), f)
    sv1[:, 0] = np.asarray(inputs["gn1_g"], f) * S2A
    sv1[:, 1] = np.asarray(inputs["gn1_b"], f) * S2A
    b2 = np.asarray(inputs["conv2_b"], f)
    sv2 = np.zeros((128, 6), f)
    sv2[:, 0:2] = (np.asarray(inputs["gn2_g"], f) * SX).reshape(2, 128).T
    sv2[:, 2:4] = (np.asarray(inputs["gn2_b"], f) * SX).reshape(2, 128).T
    sv2[:, 4:6] = (b2 * S2).reshape(2, 128).T

    shared = {
        "w1v": w1v, "w1s": w1s, "w2": w2,
        "rwh": rwh.reshape(128, 2 * DS), "rwl": rwl.reshape(128, 2 * DS),
        "rwc2": rwc2.reshape(3, 2 * DS),
        "kw": kw.reshape(128, 2 * VF), "kwc2": kwc2.reshape(3, 2 * VF),
        "vwh": vwh.reshape(128, 2 * VF), "vwl": vwl.reshape(128, 2 * VF),
        "vwc2": vwc2.reshape(3, 2 * VF),
        "coordp": coordp, "onesp": onesp,
        "ones20": np.ones((20, 128), NP_BF),
        "gsel1": gsel1, "gsel1T": gsel1T, "gsel2": gsel2, "gsel2T": gsel2T,
        "svec1": sv1, "svec2": sv2,
        "zpad": np.zeros((128, 34 * 34), NP_F8),
    }

    in_maps = []
    for c in range(N_CORES):
        sl = slice(c * SPC, (c + 1) * SPC)
        m = dict(shared)
        m["xv"] = np.ascontiguousarray(xv[sl])
        m["xsp"] = np.ascontiguousarray(xsp[sl])
        m["txtc"] = np.ascontiguousarray(txtc[sl])
        m["txtT"] = np.ascontiguousarray(txtT[sl])
        m["qw"] = np.ascontiguousarray(qw_all[sl].reshape(SPC, 128, 2 * VF))
        m["qwc2"] = np.ascontiguousarray(qwc2_all[sl].reshape(SPC, 3, 2 * VF))
        in_maps.append(m)
    return in_maps


def get_program():
    if "nc" not in _PROGRAM_CACHE:
        _PROGRAM_CACHE["nc"] = build_program()
    return _PROGRAM_CACHE["nc"]


def kernel(**inputs) -> np.ndarray:
    nc = get_program()
    in_maps = _prep_inputs(inputs)
    res = run_bass_kernel_spmd(nc, in_maps, list(range(N_CORES)))
    outs = [res.results[c]["out"].astype(np.float32) for c in range(N_CORES)]
    full = np.concatenate(outs, axis=0).reshape(32, 558, 32, 32)
    return full.astype(np.float32)


# revision 15
# speedup vs baseline: 1.3383x; 1.0011x over previous
"""Trainium2 Bass kernel for nn_AttentionFusion — fp8-DoubleRow rewrite.

Sharding: pure data parallelism over batch (32 samples -> 8 cores x 4
samples), weights replicated.

All heavy matmuls run as fp8e4m3 DoubleRow (2 k-tiles per instruction,
0.5 cycles/row — 4x the f32r row rate for K-chunked contractions), with
power-of-2 scale management so every tensor sits in e4m3's healthy range.
Precision placement (validated vs the jax reference, rel_l2 ~1.05e-2 vs
the 2e-2 gate):
  conv1        acts single fp8 x weights single fp8 (tap-chunk pairs)
  conv2        acts single fp8 (y1pad) x weights hi+lo exact pairs
  vsr          xfeat single fp8 x reduce_w hi+lo exact pairs
  E_t/rb/wt    bf16 (text branch is error-dominant)
  k,q          single fp8 both sides (mod folded into q_w host-side)
  v            xfeat fp8 x v_w hi+lo exact (v_w drives a systematic
               mean-activation error if quantized single)
  S'/E/wv      fp8 throughout; softmax denominators via an fp8 ones-matmul
               (rbD) + DVE reciprocal, numerator/denominator share E
  output       bf16 (cast to f32 on host)

Softmax denominators use nc.vector.reciprocal + TT-mult (no Ln/Exp chains,
no partition-move DMAs).
"""

import os
import sys
from contextlib import ExitStack

for _p in ("/opt/trn_rl_repo",):
    if _p not in sys.path and os.path.isdir(_p):
        sys.path.insert(0, _p)

import numpy as np
import ml_dtypes

import concourse.bacc as bacc
import concourse.mybir as mybir
import concourse.tile as tile
from concourse.bass import ts
from concourse.bass_utils import run_bass_kernel_spmd

F32 = mybir.dt.float32
BF16 = mybir.dt.bfloat16
FP8 = mybir.dt.float8e4
AF = mybir.ActivationFunctionType
ALU = mybir.AluOpType
DR = mybir.MatmulPerfMode.DoubleRow

NP_F8 = ml_dtypes.float8_e4m3
NP_BF = ml_dtypes.bfloat16

N_CORES = 8
SPC = 4  # samples per core
HW = 1024
DS = 300
VF = 258
EPS = 1e-5

# power-of-2 scale plan
SXV = 16.0      # conv1 input activations
FW1 = 512.0     # conv1 weights               -> y1 psum x 8192
S2A = 16.0      # y1pad storage (folded into gn1 affine)
FW2 = 512.0     # conv2 weights               -> y2 psum x 8192
SX = 16.0       # xfeat storage (folded into gn2 affine)
SC = 64.0       # coords + ones row storage
FRW = 512.0     # reduce_w                    -> vsr psum x 8192
FKW = 2048.0    # k_w                         -> k  psum x 32768
FQW = 512.0     # q_w*mod^2                   -> q' psum x 8192
FVW = 2048.0    # v_w                         -> v  psum x 32768
SKQV_EVAC = 1.0 / 512.0   # k->x64, q->x16, v->x64 storage scales
SV = 64.0       # v storage scale == ones128 value
S1 = SXV * FW1            # 8192
S2 = S2A * FW2            # 8192
SVSR = SX * FRW           # 8192
EPS1 = EPS * S1 * S1
EPS2 = EPS * S2 * S2
ESC_V = (1.0 / float(np.sqrt(VF))) / 1024.0
ESC_T = (1.0 / float(np.sqrt(DS))) / SVSR

TAPS = [(ty, tx) for ty in range(3) for tx in range(3)]

_PROGRAM_CACHE = {}


def _patch_act_tables():
    """Keep Exp/Ln/Relu/Identity/Copy pinned to one act table set so the
    act-table-load pass doesn't thrash between sets."""
    import concourse.bacc as _bacc
    import concourse.hw_specs as _hw

    if getattr(_bacc, "_act_tables_patched", False):
        return
    _orig = _hw.get_activation_tables
    mine = {AF.Exp, AF.Ln, AF.Relu, AF.Identity, AF.Copy}

    def patched(module_arch):
        tabs = _orig(module_arch)
        out = {}
        for name, funcs in tabs.items():
            if name == "natural_log_exp_and_others" or not (mine & funcs):
                out[name] = funcs
            else:
                out[name] = funcs - mine
        return out

    _bacc.get_activation_tables = patched
    _bacc._act_tables_patched = True


def _patch_drain_barrier():
    """Split the kernel-tail drain's per-proc sem waits across engines."""
    import concourse.tile as tile_mod
    from concourse.vector_clock import ScopedClock

    if getattr(tile_mod, "_drain_patched", False):
        return

    def _patched(self, tick_clock, wait_clock):
        nc = self.nc
        drain_inst = nc.sync.drain()
        wait_clock.add_sem_waits(
            drain_inst.ins, ScopedClock({None: tick_clock.global_clock})
        )
        si = drain_inst.ins.sync_info
        waits = list(si.on_wait or [])
        if len(waits) > 1:
            si.on_wait = waits[:1]
            engines = [nc.sync, nc.scalar, nc.vector, nc.tensor, nc.gpsimd]
            for i in range(1, len(waits)):
                extra = engines[i % len(engines)].drain()
                extra.ins.sync_info = mybir.SyncInfo(
                    on_wait=[waits[i]], on_update=[]
                )
        nc.all_engine_barrier()
        assert self.sems is not None
        popped = nc._tile_sem_poison_stack.pop()
        assert popped is self._sem_poison
        nc.clear_and_free_semaphores(list(self.sems.allocated().values()))
        nc.all_engine_barrier()

    tile_mod.TileContext._drain_and_barrier = _patched
    tile_mod._drain_patched = True


def build_program():
    _patch_act_tables()
    _patch_drain_barrier()
    nc = bacc.Bacc()
    dt = F32

    # ---------------- DRAM declarations ----------------
    d_xv = nc.dram_tensor("xv", [SPC, 128, 2 * 34 * 34], FP8, kind="ExternalInput")
    d_xsp = nc.dram_tensor("xsp", [SPC, 74, 2 * HW], FP8, kind="ExternalInput")
    d_txtc = nc.dram_tensor("txtc", [SPC, 128, 3 * 20], BF16, kind="ExternalInput")
    d_txtT = nc.dram_tensor("txtT", [SPC, 20, DS], BF16, kind="ExternalInput")
    d_qw = nc.dram_tensor("qw", [SPC, 128, 2 * VF], FP8, kind="ExternalInput")
    d_qwc2 = nc.dram_tensor("qwc2", [SPC, 3, 2 * VF], FP8, kind="ExternalInput")
    d_w1v = nc.dram_tensor("w1v", [128, 9 * 2 * 128], FP8, kind="ExternalInput")
    d_w1s = nc.dram_tensor("w1s", [74, 2 * 128], FP8, kind="ExternalInput")
    d_w2 = nc.dram_tensor("w2", [128, 9 * 2 * 2 * 128], FP8, kind="ExternalInput")
    d_rwh = nc.dram_tensor("rwh", [128, 2 * DS], FP8, kind="ExternalInput")
    d_rwl = nc.dram_tensor("rwl", [128, 2 * DS], FP8, kind="ExternalInput")
    d_rwc2 = nc.dram_tensor("rwc2", [3, 2 * DS], FP8, kind="ExternalInput")
    d_kw = nc.dram_tensor("kw", [128, 2 * VF], FP8, kind="ExternalInput")
    d_kwc2 = nc.dram_tensor("kwc2", [3, 2 * VF], FP8, kind="ExternalInput")
    d_vwh = nc.dram_tensor("vwh", [128, 2 * VF], FP8, kind="ExternalInput")
    d_vwl = nc.dram_tensor("vwl", [128, 2 * VF], FP8, kind="ExternalInput")
    d_vwc2 = nc.dram_tensor("vwc2", [3, 2 * VF], FP8, kind="ExternalInput")
    d_coordp = nc.dram_tensor("coordp", [3, 2 * HW], FP8, kind="ExternalInput")
    d_onesp = nc.dram_tensor("onesp", [128, 2 * 128], FP8, kind="ExternalInput")
    d_ones20 = nc.dram_tensor("ones20", [20, 128], BF16, kind="ExternalInput")
    d_gs1 = nc.dram_tensor("gsel1", [128, 32], dt, kind="ExternalInput")
    d_gs1T = nc.dram_tensor("gsel1T", [32, 128], dt, kind="ExternalInput")
    d_gs2 = nc.dram_tensor("gsel2", [128, 2 * 16], dt, kind="ExternalInput")
    d_gs2T = nc.dram_tensor("gsel2T", [16, 2 * 128], dt, kind="ExternalInput")
    d_sv1 = nc.dram_tensor("svec1", [128, 2], dt, kind="ExternalInput")
    d_sv2 = nc.dram_tensor("svec2", [128, 6], dt, kind="ExternalInput")
    d_zpad = nc.dram_tensor("zpad", [128, 34 * 34], FP8, kind="ExternalInput")
    d_out = nc.dram_tensor("out", [SPC, 558, HW], BF16, kind="ExternalOutput")

    with tile.TileContext(nc) as tc, ExitStack() as ctx:
        wpool = ctx.enter_context(tc.tile_pool(name="weights", bufs=1))
        inpool = ctx.enter_context(tc.tile_pool(name="inputs", bufs=2))
        spool = ctx.enter_context(tc.tile_pool(name="work", bufs=1))
        opool = ctx.enter_context(tc.tile_pool(name="outs", bufs=1))
        epool = ctx.enter_context(tc.tile_pool(name="estream", bufs=3))
        pbig = ctx.enter_context(tc.tile_pool(name="pbig", bufs=3, space="PSUM"))
        pS = ctx.enter_context(tc.tile_pool(name="pS", bufs=2, space="PSUM"))

        # ---------- conv1 weights + sample-0 inputs first ----------
        sv1 = wpool.tile([128, 2], dt, name="sv1")
        nc.sync.dma_start(sv1[:, :], d_sv1[:, :])
        g1_ap, b1_ap = sv1[:, 0:1], sv1[:, 1:2]
        w1v_sb = wpool.tile([128, 9, 2, 128], FP8, name="w1v_sb")
        nc.sync.dma_start(
            w1v_sb[:, :, :, :].rearrange("p a b c -> p (a b c)"), d_w1v[:, :]
        )
        w1s_sb = wpool.tile([74, 2, 128], FP8, name="w1s_sb")
        nc.sync.dma_start(
            w1s_sb[:, :, :].rearrange("p a b -> p (a b)"), d_w1s[:, :]
        )
        gs1_sb = wpool.tile([128, 32], dt, name="gs1_sb")
        nc.sync.dma_start(gs1_sb[:, :], d_gs1[:, :])
        gs1T_sb = wpool.tile([32, 128], dt, name="gs1T_sb")
        nc.sync.dma_start(gs1T_sb[:, :], d_gs1T[:, :])

        def load_inputs(s):
            xv = inpool.tile([128, 2, 34, 34], FP8, name="xv")
            nc.sync.dma_start(
                xv[:, :, :, :].rearrange("p a h w -> p (a h w)"), d_xv[s]
            )
            xsp = inpool.tile([74, 2, HW], FP8, name="xsp")
            nc.sync.dma_start(
                xsp[:, :, :].rearrange("p a b -> p (a b)"), d_xsp[s]
            )
            txtc = inpool.tile([128, 3, 20], BF16, name="txtc")
            nc.sync.dma_start(
                txtc[:, :, :].rearrange("p a b -> p (a b)"), d_txtc[s]
            )
            txtT = inpool.tile([20, DS], BF16, name="txtT")
            nc.sync.dma_start(txtT[:, :], d_txtT[s])
            qw = inpool.tile([128, 2, VF], FP8, name="qw")
            nc.sync.dma_start(
                qw[:, :, :].rearrange("p a b -> p (a b)"), d_qw[s]
            )
            qwc2 = inpool.tile([3, 2, VF], FP8, name="qwc2")
            nc.sync.dma_start(
                qwc2[:, :, :].rearrange("p a b -> p (a b)"), d_qwc2[s]
            )
            return dict(xv=xv, xsp=xsp, txtc=txtc, txtT=txtT, qw=qw, qwc2=qwc2)

        preloaded = load_inputs(0)

        # ---------- remaining weights ----------
        w2_sb = wpool.tile([128, 9, 2, 2, 128], FP8, name="w2_sb")
        nc.sync.dma_start(
            w2_sb[:, :, :, :, :].rearrange("p a b c d -> p (a b c d)"), d_w2[:, :]
        )
        sv2 = wpool.tile([128, 6], dt, name="sv2")
        nc.sync.dma_start(sv2[:, :], d_sv2[:, :])
        g2_ap, b2_ap, b2s_ap = sv2[:, 0:2], sv2[:, 2:4], sv2[:, 4:6]
        gs2_sb = wpool.tile([128, 2, 16], dt, name="gs2_sb")
        nc.sync.dma_start(
            gs2_sb[:, :, :].rearrange("p a b -> p (a b)"), d_gs2[:, :]
        )
        gs2T_sb = wpool.tile([16, 2, 128], dt, name="gs2T_sb")
        nc.sync.dma_start(
            gs2T_sb[:, :, :].rearrange("p a b -> p (a b)"), d_gs2T[:, :]
        )
        rwh_sb = wpool.tile([128, 2, DS], FP8, name="rwh_sb")
        nc.sync.dma_start(rwh_sb[:, :, :].rearrange("p a b -> p (a b)"), d_rwh[:, :])
        rwl_sb = wpool.tile([128, 2, DS], FP8, name="rwl_sb")
        nc.sync.dma_start(rwl_sb[:, :, :].rearrange("p a b -> p (a b)"), d_rwl[:, :])
        rwc2_sb = wpool.tile([3, 2, DS], FP8, name="rwc2_sb")
        nc.sync.dma_start(rwc2_sb[:, :, :].rearrange("p a b -> p (a b)"), d_rwc2[:, :])
        kw_sb = wpool.tile([128, 2, VF], FP8, name="kw_sb")
        nc.sync.dma_start(kw_sb[:, :, :].rearrange("p a b -> p (a b)"), d_kw[:, :])
        kwc2_sb = wpool.tile([3, 2, VF], FP8, name="kwc2_sb")
        nc.sync.dma_start(kwc2_sb[:, :, :].rearrange("p a b -> p (a b)"), d_kwc2[:, :])
        vwh_sb = wpool.tile([128, 2, VF], FP8, name="vwh_sb")
        nc.sync.dma_start(vwh_sb[:, :, :].rearrange("p a b -> p (a b)"), d_vwh[:, :])
        vwl_sb = wpool.tile([128, 2, VF], FP8, name="vwl_sb")
        nc.sync.dma_start(vwl_sb[:, :, :].rearrange("p a b -> p (a b)"), d_vwl[:, :])
        vwc2_sb = wpool.tile([3, 2, VF], FP8, name="vwc2_sb")
        nc.sync.dma_start(vwc2_sb[:, :, :].rearrange("p a b -> p (a b)"), d_vwc2[:, :])
        coordp_sb = wpool.tile([3, 2, HW], FP8, name="coordp_sb")
        nc.sync.dma_start(
            coordp_sb[:, :, :].rearrange("p a b -> p (a b)"), d_coordp[:, :]
        )
        onesp_sb = wpool.tile([128, 2, 128], FP8, name="onesp_sb")
        nc.sync.dma_start(
            onesp_sb[:, :, :].rearrange("p a b -> p (a b)"), d_onesp[:, :]
        )
        ones20_sb = wpool.tile([20, 128], BF16, name="ones20_sb")
        nc.sync.dma_start(ones20_sb[:, :], d_ones20[:, :])

        # persistent zero-padded conv2 input (border stays zero forever)
        y1pad = wpool.tile([128, 1, 34, 34], FP8, name="y1pad")
        nc.sync.dma_start(
            y1pad[:, :, :, :].rearrange("p a h w -> p (a h w)"), d_zpad[:, :]
        )
        # persistent k/q tail tiles; plane 1 must stay zero (S' tail pairs)
        ktail = wpool.tile([2, 2, HW], FP8, name="ktail")
        nc.vector.memset(ktail[:, :, :].rearrange("p a b -> p (a b)"), 0)
        qtail = wpool.tile([2, 2, HW], FP8, name="qtail")
        nc.vector.memset(qtail[:, :, :].rearrange("p a b -> p (a b)"), 0)

        # ---------------- helpers ----------------
        def group_norm_finish(gstat_ps, cb_ps, gamma_ap, beta_ap, gsT_ap,
                              eps_s, groups, tag, bias_col=None):
            """gstat_ps: [G,2] PSUM (mean, E[x2]) per group (scaled domain);
            cb_ps: [128,2] PSUM for the broadcast-back.  Returns sc [128,2]
            SBUF: col0 = scale, col1 = bias for act(relu, psum-input).
            bias_col: [128,1] host column of b*S to subtract from the
            broadcast channel mean (act input psum is un-biased)."""
            gb = spool.tile([groups, 4], dt, name=f"gb_{tag}")
            nc.vector.tensor_copy(gb[:, 0:1], gstat_ps[:, 0:1])
            nc.vector.tensor_tensor(gb[:, 3:4], gb[:, 0:1], gb[:, 0:1], ALU.mult)
            nc.vector.tensor_tensor(
                gb[:, 1:2], gstat_ps[:, 1:2], gb[:, 3:4], ALU.subtract
            )
            nc.vector.tensor_scalar_add(gb[:, 1:2], gb[:, 1:2], float(eps_s))
            nc.scalar.activation(gb[:, 2:3], gb[:, 1:2], AF.Ln)
            nc.scalar.activation(gb[:, 1:2], gb[:, 2:3], AF.Exp, scale=-0.5)
            nc.tensor.matmul(cb_ps, gsT_ap, gb[:, 0:2], start=True, stop=True)
            sc = spool.tile([128, 3], dt, name=f"sc_{tag}")
            nc.vector.tensor_tensor(sc[:, 0:1], gamma_ap, cb_ps[:, 1:2], ALU.mult)
            if bias_col is not None:
                nc.vector.tensor_tensor(
                    sc[:, 2:3], cb_ps[:, 0:1], bias_col, ALU.subtract
                )
                mu_ap = sc[:, 2:3]
            else:
                mu_ap = cb_ps[:, 0:1]
            nc.vector.tensor_tensor(sc[:, 1:2], mu_ap, sc[:, 0:1], ALU.mult)
            nc.vector.tensor_tensor(sc[:, 1:2], beta_ap, sc[:, 1:2], ALU.subtract)
            return sc

        def channel_stats(ps_a, ps_b, tag, bias_col=None):
            """Two [128,512] PSUM halves -> st2 [128,2] = (mean_b, E_b[x^2])."""
            bnst = spool.tile([128, 2, 6], dt, name=f"bnst_{tag}")
            nc.vector.bn_stats(bnst[:, 0, :], ps_a)
            nc.vector.bn_stats(bnst[:, 1, :], ps_b)
            mv = spool.tile([128, 2], dt, name=f"mv_{tag}")
            nc.vector.bn_aggr(mv[:, :], bnst[:, :, :])
            st2 = spool.tile([128, 2], dt, name=f"st2_{tag}")
            if bias_col is not None:
                nc.vector.tensor_tensor(st2[:, 0:1], mv[:, 0:1], bias_col, ALU.add)
            else:
                nc.vector.tensor_copy(st2[:, 0:1], mv[:, 0:1])
            nc.vector.tensor_tensor(st2[:, 1:2], st2[:, 0:1], st2[:, 0:1], ALU.mult)
            nc.vector.tensor_tensor(st2[:, 1:2], st2[:, 1:2], mv[:, 1:2], ALU.add)
            return st2

        # ---------------- attention, jp-granular for interleaving ----------
        def attn_mm(sd, s, ni, jps):
            """S' + exp + wv accumulation for jp groups of one n-half."""
            kT, qT, vsb = sd["kT"], sd["qT"], sd["vsb"]
            if 0 in jps:
                sd["wv01"] = pbig.tile([128, 1024], F32, tag="big",
                                       name=f"wv01_{s}_{ni}")
                sd["wvD"] = pbig.tile([128, 1024], F32, tag="big",
                                      name=f"wvD_{s}_{ni}")
            wv01, wvD = sd["wv01"], sd["wvD"]
            for jp in jps:
                Ep = epool.tile([128, 2, 512], FP8, tag="E", name=f"E_{s}_{ni}_{jp}")
                for jj in range(2):
                    j = 2 * jp + jj
                    sps = pS.tile([128, 512], F32, tag="ps", name=f"sps_{s}_{ni}_{j}")
                    nc.tensor.matmul(
                        sps[:, :], qT[:, :, ts(j, 128)], kT[:, :, ts(ni, 512)],
                        start=True, stop=False, perf_mode=DR,
                    )
                    nc.tensor.matmul(
                        sps[:, :], qtail[:, :, ts(j, 128)], ktail[:, :, ts(ni, 512)],
                        start=False, stop=True, perf_mode=DR,
                    )
                    nc.scalar.activation(Ep[:, jj, :], sps[:, :], AF.Exp, scale=ESC_V)
                st, sp = (jp == 0), (jp == 3)
                nc.tensor.matmul(
                    wv01[:, 0:512], vsb[:, jp, :, 0:128], Ep[:, :, :],
                    start=st, stop=sp, perf_mode=DR,
                )
                nc.tensor.matmul(
                    wv01[:, 512:1024], vsb[:, jp, :, 128:256], Ep[:, :, :],
                    start=st, stop=sp, perf_mode=DR,
                )
                nc.tensor.matmul(
                    wvD[0:2, 512:1024], vsb[:, jp, :, 256:258], Ep[:, :, :],
                    start=st, stop=sp, perf_mode=DR,
                )
                nc.tensor.matmul(
                    wvD[:, 0:512], onesp_sb[:, :, :], Ep[:, :, :],
                    start=st, stop=sp, perf_mode=DR,
                )

        def attn_fin(sd, s, ni):
            wv01, wvD, wvout = sd["wv01"], sd["wvD"], sd["wvout"]
            rbc = spool.tile([128, 512], dt, name=f"rbcv_{ni}")
            nc.vector.reciprocal(rbc[:, :], wvD[:, 0:512])
            nc.vector.tensor_tensor(
                wvout[:, 0, ts(ni, 512)], wv01[:, 0:512], rbc[:, :], ALU.mult
            )
            nc.vector.tensor_tensor(
                wvout[:, 1, ts(ni, 512)], wv01[:, 512:1024], rbc[:, :], ALU.mult
            )
            nc.vector.tensor_tensor(
                wvout[0:2, 2, ts(ni, 512)], wvD[0:2, 512:1024], rbc[0:2, :], ALU.mult
            )

        def attn_out_dma(sd, s):
            wvout = sd["wvout"]
            nc.gpsimd.dma_start(d_out[s, 0:128, :], wvout[:, 0, :])
            nc.gpsimd.dma_start(d_out[s, 128:256, :], wvout[:, 1, :])
            nc.gpsimd.dma_start(d_out[s, 256:258, :], wvout[:2, 2, :])

        # ---------------- conv helpers ----------------
        def emit_conv1(s, io):
            xv, xsp = io["xv"], io["xsp"]
            ps = pbig.tile([128, 1024], F32, tag="big", name=f"c1ps_{s}")
            for ni in range(2):
                h0 = ni * 16
                for t, (ty, tx) in enumerate(TAPS):
                    nc.tensor.matmul(
                        ps[:, ts(ni, 512)],
                        w1v_sb[:, t, :, :],
                        xv[:, :, ty + h0 : ty + h0 + 16, tx : tx + 32],
                        start=(t == 0), stop=False, perf_mode=DR,
                    )
                nc.tensor.matmul(
                    ps[:, ts(ni, 512)], w1s_sb[:, :, :], xsp[:, :, ts(ni, 512)],
                    start=False, stop=True, perf_mode=DR,
                )
            return ps

        def emit_conv2_mb(s, mb):
            ps = pbig.tile([128, 1024], F32, tag="big", name=f"c2ps_{s}_{mb}")
            for ni in range(2):
                h0 = ni * 16
                for t, (ty, tx) in enumerate(TAPS):
                    nc.tensor.matmul(
                        ps[:, ts(ni, 512)],
                        w2_sb[:, t, mb, :, :],
                        y1pad[:, 0:1, ty + h0 : ty + h0 + 16, tx : tx + 32]
                        .to_broadcast((128, 2, 16, 32)),
                        start=(t == 0), stop=(t == 8), perf_mode=DR,
                    )
            return ps

        def gn2_finish_relu(s, mb, ps2, xfeat):
            st2b = channel_stats(
                ps2[:, 0:512], ps2[:, 512:1024], f"gn2_{s}_{mb}",
                bias_col=b2s_ap[:, mb : mb + 1],
            )
            gt2 = pS.tile([128, 4], dt, tag="ps", name=f"gst2_{s}_{mb}")
            nc.tensor.matmul(
                gt2[:16, 0:2], gs2_sb[:, mb, :], st2b[:, :], start=True, stop=True
            )
            sc2 = group_norm_finish(
                gt2[:16, 0:2], gt2[:, 2:4],
                g2_ap[:, mb : mb + 1], b2_ap[:, mb : mb + 1],
                gs2T_sb[:, mb, :], EPS2, 16, f"gn2_{s}_{mb}",
                bias_col=b2s_ap[:, mb : mb + 1],
            )
            nc.scalar.activation(
                xfeat[:, mb, :], ps2[:, :],
                AF.Relu, bias=sc2[:, 1:2], scale=sc2[:, 0:1],
            )

        # ---------------- per-sample pipeline ----------------
        prev = None
        ios = {0: preloaded}
        for s in range(SPC):
            io = ios.pop(s)
            if s + 1 < SPC:
                ios[s + 1] = load_inputs(s + 1)
            txtc, txtT, qw_s, qwc2_s = io["txtc"], io["txtT"], io["qw"], io["qwc2"]

            ps1 = emit_conv1(s, io)
            st2 = channel_stats(ps1[:, 0:512], ps1[:, 512:1024], f"gn1_{s}")
            if prev is not None:
                attn_mm(prev, s - 1, 0, [0])
            gt1 = pS.tile([128, 4], dt, tag="ps", name=f"gst1_{s}")
            nc.tensor.matmul(
                gt1[:32, 0:2], gs1_sb[:, :], st2[:, :], start=True, stop=True
            )
            sc1 = group_norm_finish(
                gt1[:32, 0:2], gt1[:, 2:4],
                g1_ap, b1_ap, gs1T_sb[:, :], EPS1, 32, f"gn1_{s}",
            )
            nc.scalar.activation(
                y1pad[:, 0, 1:33, 1:33],
                ps1[:, :].rearrange("p (h w) -> p h w", h=32),
                AF.Relu, bias=sc1[:, 1:2], scale=sc1[:, 0:1],
            )
            if prev is not None:
                attn_mm(prev, s - 1, 0, [1, 2, 3])
                attn_fin(prev, s - 1, 0)

            # -------- conv2 (W hi/lo pairs, stride-0 moving) --------
            xfeat = spool.tile([128, 2, HW], FP8, name="xfeat")
            ps2a = emit_conv2_mb(s, 0)
            if prev is not None:
                attn_mm(prev, s - 1, 1, [0, 1])
            gn2_finish_relu(s, 0, ps2a, xfeat)
            ps2b = emit_conv2_mb(s, 1)
            if prev is not None:
                attn_mm(prev, s - 1, 1, [2, 3])
                attn_fin(prev, s - 1, 1)
                attn_out_dma(prev, s - 1)
            gn2_finish_relu(s, 1, ps2b, xfeat)

            # -------- vsr^T [300, 1024] bf16 (rw hi/lo pairs) --------
            vsrT = spool.tile([128, 3, HW], BF16, name="vsrT")
            DSCH = (128, 128, 44)
            for mb in range(3):
                mr = DSCH[mb]
                m0 = mb * 128
                ps = pbig.tile([128, 1024], F32, tag="big", name=f"vsr_{s}_{mb}")
                for ni in range(2):
                    nc.tensor.matmul(
                        ps[:mr, ts(ni, 512)], rwh_sb[:, :, m0 : m0 + mr],
                        xfeat[:, :, ts(ni, 512)],
                        start=True, stop=False, perf_mode=DR,
                    )
                    nc.tensor.matmul(
                        ps[:mr, ts(ni, 512)], rwl_sb[:, :, m0 : m0 + mr],
                        xfeat[:, :, ts(ni, 512)],
                        start=False, stop=False, perf_mode=DR,
                    )
                    nc.tensor.matmul(
                        ps[:mr, ts(ni, 512)], rwc2_sb[:, :, m0 : m0 + mr],
                        coordp_sb[:, :, ts(ni, 512)],
                        start=False, stop=True, perf_mode=DR,
                    )
                if mb % 2 == 0:
                    nc.vector.tensor_copy(vsrT[:mr, mb, :], ps[:mr, :])
                else:
                    nc.scalar.activation(vsrT[:mr, mb, :], ps[:mr, :], AF.Copy)

            # -------- text cross attention (bf16) --------
            E_t = spool.tile([20, HW], BF16, name="E_t")
            pet = pbig.tile([20, 1024], F32, tag="big", name=f"et_{s}")
            for ni in range(2):
                for kc in range(3):
                    kr = DSCH[kc]
                    nc.tensor.matmul(
                        pet[:, ts(ni, 512)], txtc[:kr, kc, :],
                        vsrT[:kr, kc, ts(ni, 512)],
                        start=(kc == 0), stop=(kc == 2),
                    )
            nc.scalar.activation(E_t[:, :], pet[:, :], AF.Exp, scale=ESC_T)
            rbc_t = spool.tile([128, HW], dt, name="rbc_t")
            prb = pbig.tile([128, 1024], F32, tag="big", name=f"rbt_{s}")
            for ni in range(2):
                nc.tensor.matmul(
                    prb[:, ts(ni, 512)], ones20_sb[:, :], E_t[:, ts(ni, 512)],
                    start=True, stop=True,
                )
            nc.vector.reciprocal(rbc_t[:, :], prb[:, :])
            wtout = opool.tile([128, 3, HW], BF16, name="wtout")
            WT_ROWS = (128, 128, 44)
            for mb in range(3):
                ps = pbig.tile([128, 1024], F32, tag="big", name=f"wt_{s}_{mb}")
                mr = WT_ROWS[mb]
                for ni in range(2):
                    nc.tensor.matmul(
                        ps[:mr, ts(ni, 512)],
                        txtT[:, mb * 128 : mb * 128 + mr],
                        E_t[:, ts(ni, 512)],
                        start=True, stop=True,
                    )
                nc.vector.tensor_tensor(
                    wtout[:mr, mb, :], ps[:mr, :], rbc_t[:mr, :], ALU.mult
                )
            nc.gpsimd.dma_start(d_out[s, 258:386, :], wtout[:, 0, :])
            nc.gpsimd.dma_start(d_out[s, 386:514, :], wtout[:, 1, :])
            nc.gpsimd.dma_start(d_out[s, 514:558, :], wtout[:44, 2, :])

            # -------- k^T, q^T fp8 (single-fp8 both sides) --------
            kT = spool.tile([128, 2, HW], FP8, name="kT")
            qT = spool.tile([128, 2, HW], FP8, name="qT")
            for wi, (w_sb, wc2_sb, dstT) in enumerate(
                ((kw_sb, kwc2_sb, kT), (qw_s, qwc2_s, qT))
            ):
                for mb in range(2):
                    ps = pbig.tile([128, 1024], F32, tag="big",
                                   name=f"kq_{s}_{wi}_{mb}")
                    for ni in range(2):
                        nc.tensor.matmul(
                            ps[:, ts(ni, 512)], w_sb[:, :, ts(mb, 128)],
                            xfeat[:, :, ts(ni, 512)],
                            start=True, stop=False, perf_mode=DR,
                        )
                        nc.tensor.matmul(
                            ps[:, ts(ni, 512)], wc2_sb[:, :, ts(mb, 128)],
                            coordp_sb[:, :, ts(ni, 512)],
                            start=False, stop=True, perf_mode=DR,
                        )
                    if (wi + mb) % 2 == 0:
                        nc.scalar.activation(
                            dstT[:, mb, :], ps[:, :], AF.Copy, scale=SKQV_EVAC
                        )
                    else:
                        nc.vector.tensor_scalar_mul(
                            dstT[:, mb, :], ps[:, :], SKQV_EVAC
                        )
            # tails (output cols 256,257 of k and q)
            pkt = pbig.tile([2, 1024], F32, tag="big", name=f"ktp_{s}")
            pqt = pbig.tile([2, 1024], F32, tag="big", name=f"qtp_{s}")
            for ni in range(2):
                nc.tensor.matmul(
                    pkt[:, ts(ni, 512)], kw_sb[:, :, 256:258],
                    xfeat[:, :, ts(ni, 512)],
                    start=True, stop=False, perf_mode=DR,
                )
                nc.tensor.matmul(
                    pkt[:, ts(ni, 512)], kwc2_sb[:, :, 256:258],
                    coordp_sb[:, :, ts(ni, 512)],
                    start=False, stop=True, perf_mode=DR,
                )
                nc.tensor.matmul(
                    pqt[:, ts(ni, 512)], qw_s[:, :, 256:258],
                    xfeat[:, :, ts(ni, 512)],
                    start=True, stop=False, perf_mode=DR,
                )
                nc.tensor.matmul(
                    pqt[:, ts(ni, 512)], qwc2_s[:, :, 256:258],
                    coordp_sb[:, :, ts(ni, 512)],
                    start=False, stop=True, perf_mode=DR,
                )
            nc.vector.tensor_scalar_mul(ktail[:, 0, :], pkt[:, :], SKQV_EVAC)
            nc.scalar.activation(
                qtail[:, 0, :], pqt[:, :], AF.Copy, scale=SKQV_EVAC
            )

            # -------- v [1024, 258] fp8 (vw hi/lo pairs), j pairs --------
            vsb = spool.tile([128, 4, 2, VF], FP8, name="vsb")
            for jp in range(4):
                ps = pbig.tile([128, 1024], F32, tag="big", name=f"v_{s}_{jp}")
                psv = ps[:, :].rearrange("p (a b) -> p a b", a=2)
                for jj in range(2):
                    j = 2 * jp + jj
                    nc.tensor.matmul(
                        psv[:, jj, 0:VF], xfeat[:, :, ts(j, 128)], vwh_sb[:, :, :],
                        start=True, stop=False, perf_mode=DR,
                    )
                    nc.tensor.matmul(
                        psv[:, jj, 0:VF], xfeat[:, :, ts(j, 128)], vwl_sb[:, :, :],
                        start=False, stop=False, perf_mode=DR,
                    )
                    nc.tensor.matmul(
                        psv[:, jj, 0:VF], coordp_sb[:, :, ts(j, 128)],
                        vwc2_sb[:, :, :],
                        start=False, stop=True, perf_mode=DR,
                    )
                if jp % 2 == 0:
                    nc.vector.tensor_scalar_mul(
                        vsb[:, jp, :, :], psv[:, :, 0:VF], SKQV_EVAC
                    )
                else:
                    nc.scalar.activation(
                        vsb[:, jp, :, :], psv[:, :, 0:VF], AF.Copy,
                        scale=SKQV_EVAC,
                    )

            prev = {
                "kT": kT, "qT": qT, "vsb": vsb,
                "wvout": opool.tile([128, 3, HW], BF16, name="wvout"),
            }
            if s == SPC - 1:
                attn_mm(prev, s, 0, [0, 1, 2, 3])
                attn_fin(prev, s, 0)
                attn_mm(prev, s, 1, [0, 1, 2, 3])
                attn_fin(prev, s, 1)
                attn_out_dma(prev, s)

    nc.finalize()
    return nc


def _q8(x):
    return np.asarray(x, np.float32).astype(NP_F8)


def _hilo(x):
    h = _q8(x)
    l = _q8(np.asarray(x, np.float32) - h.astype(np.float32))
    return h, l


def _prep_inputs(inputs):
    """Host-side marshalling: shard over batch, scale + quantize weights,
    im2col the spatial channels, fold mod^2 into q_w, hi/lo-split the
    error-critical weights."""
    f = np.float32
    video = np.asarray(inputs["video_feat"], f)
    spat = np.asarray(inputs["spatial_feat"], f)
    txt = np.asarray(inputs["txt"], f)
    B = video.shape[0]

    # conv1 inputs: video padded, x SXV, fp8, partition-major [128, 2, 1156]
    xv = np.zeros((B, 256, 34, 34), f)
    xv[:, :, 1:33, 1:33] = video * SXV
    xv = _q8(np.ascontiguousarray(
        xv.reshape(B, 2, 128, 34 * 34).transpose(0, 2, 1, 3)
    ).reshape(B, 128, 2 * 34 * 34))

    # spatial: host im2col (9 taps x 8 ch = 72 rows) + 2 bias-ones rows
    sp_pad = np.zeros((B, 8, 34, 34), f)
    sp_pad[:, :, 1:33, 1:33] = spat * SXV
    xsp_v = np.stack(
        [sp_pad[:, :, ty : ty + 32, tx : tx + 32] for (ty, tx) in TAPS], axis=1
    ).reshape(B, 72, HW)
    xsp = np.zeros((B, 74, 2, HW), f)
    xsp[:, :72, 0, :] = xsp_v
    xsp[:, 72, 0, :] = SXV
    xsp[:, 73, 0, :] = SXV
    xsp = _q8(xsp.reshape(B, 74, 2 * HW))

    # conv1 weights: [c_in(128), tap, chunk, c_out] x FW1 single fp8
    w1 = np.asarray(inputs["conv1_w"], f)
    w1v9 = w1[:, :256].transpose(2, 3, 1, 0).reshape(9, 2, 128, 128)  # t,c,ci,co
    w1v = _q8(np.ascontiguousarray(
        w1v9.transpose(2, 0, 1, 3)).reshape(128, 9 * 2 * 128) * FW1)
    # spatial weights + bias rows (hi/lo of b1*FW1, moving value SXV both)
    b1 = np.asarray(inputs["conv1_b"], f)
    w1s_rows = np.zeros((74, 2, 128), f)
    w1s_rows[:72, 0, :] = w1[:, 256:].transpose(2, 3, 1, 0).reshape(72, 128) * FW1
    bh = _q8(b1 * FW1).astype(f)
    w1s_rows[72, 0, :] = bh
    w1s_rows[73, 0, :] = b1 * FW1 - bh
    w1s = _q8(w1s_rows.reshape(74, 2 * 128))

    # conv2 weights: [c_in, tap, mb, hl, c_out], hi/lo exact, x FW2
    w29 = np.asarray(inputs["conv2_w"], f).transpose(2, 3, 1, 0).reshape(9, 128, 256)
    w2s = w29 * FW2
    w2h = _q8(w2s)
    w2l = _q8(w2s - w2h.astype(f))
    w2 = np.zeros((128, 9, 2, 2, 128), NP_F8)
    for mb in range(2):
        w2[:, :, mb, 0, :] = w2h.transpose(1, 0, 2)[:, :, mb * 128 : (mb + 1) * 128]
        w2[:, :, mb, 1, :] = w2l.transpose(1, 0, 2)[:, :, mb * 128 : (mb + 1) * 128]
    w2 = w2.reshape(128, 9 * 2 * 2 * 128)

    # reduce_w: [in, 2(chunk), 300]: feature rows x FRW hi/lo; coord+bias
    # chunk separately x (SVSR/SC)
    rw = np.asarray(inputs["reduce_w"], f)     # [300, 258]
    rb = np.asarray(inputs["reduce_b"], f)
    rwT = rw.T                                  # [258, 300]
    rw_feat = np.stack([rwT[0:128], rwT[128:256]], axis=1) * FRW  # [128,2,300]
    rwh, rwl = _hilo(rw_feat)
    rw_c2 = np.zeros((3, 2, DS), f)
    c2 = np.stack([rwT[256], rwT[257], rb], axis=0) * (SVSR / SC)  # [3,300]
    c2h = _q8(c2).astype(f)
    rw_c2[:, 0, :] = c2h
    rw_c2[:, 1, :] = c2 - c2h
    rwc2 = _q8(rw_c2)

    def kq_pack(wmat, bias, f_w, f_c2):
        """wmat [258,258] torch (out,in); returns main [128,2,258] and
        c2 [3,2,258] (plane1 zeros) fp8."""
        wT = np.asarray(wmat, f).T  # [in 258, out 258]
        main = np.stack([wT[0:128], wT[128:256]], axis=1) * f_w
        c2m = np.zeros((3, 2, VF), f)
        c2m[0:2, 0, :] = wT[256:258] * f_c2
        c2m[2, 0, :] = np.asarray(bias, f) * f_c2
        return _q8(main), _q8(c2m)

    # k coord rows: (coord*SC)*(w*g) = w_contrib*(SX*FKW) -> g = SX*FKW/SC
    kw, kwc2 = kq_pack(inputs["k_w"], inputs["k_b"], FKW, SX * FKW / SC)

    # v_w hi/lo: main [128,2,258] x FVW ; c2 [3,2,258] = (h,l) planes
    vwT = np.asarray(inputs["v_w"], f).T
    vb = np.asarray(inputs["v_b"], f)
    v_feat = np.stack([vwT[0:128], vwT[128:256]], axis=1) * FVW
    vwh, vwl = _hilo(v_feat)
    vc2 = np.zeros((3, 2, VF), f)
    c2v = np.concatenate([vwT[256:258], vb[None]], axis=0) * (SX * FVW / SC)
    c2vh = _q8(c2v).astype(f)
    vc2[:, 0, :] = c2vh
    vc2[:, 1, :] = c2v - c2vh
    vwc2 = _q8(vc2)

    # q_w with mod^2 folded, per sample
    incw = np.asarray(inputs["inc_w"], f)
    incb = np.asarray(inputs["inc_b"], f)
    mod = np.maximum.reduce(txt, axis=2) @ incw.T + incb   # [B, 258]
    qwT = np.asarray(inputs["q_w"], f).T                    # [in, out]
    qb_ = np.asarray(inputs["q_b"], f)
    qw_all = np.zeros((B, 128, 2, VF), NP_F8)
    qwc2_all = np.zeros((B, 3, 2, VF), NP_F8)
    for b in range(B):
        m2 = (mod[b] ** 2)[None, :]                         # [1, out]
        qmain = np.stack([qwT[0:128], qwT[128:256]], axis=1) * (FQW * m2[:, None, :])
        qw_all[b] = _q8(qmain)
        qc2 = np.zeros((3, 2, VF), f)
        qc2[0:2, 0, :] = qwT[256:258] * (SX * FQW / SC) * m2
        qc2[2, 0, :] = qb_ * (SX * FQW / SC) * m2[0]
        qwc2_all[b] = _q8(qc2)

    # coords pair tile: rows (x, y, ones) x SC, both planes identical
    xr = np.linspace(-1.0, 1.0, 32, dtype=f)
    yy, xx = np.meshgrid(xr, xr, indexing="ij")
    coord3 = np.stack([xx.ravel(), yy.ravel(), np.ones(HW, f)]).astype(f) * SC
    coordp = _q8(np.stack([coord3, coord3], axis=1).reshape(3, 2 * HW))

    onesp = _q8(np.full((128, 2 * 128), SV, f))

    # text tensors bf16
    txtc = np.zeros((B, 128, 3, 20), f)
    txtc[:, :, 0, :] = txt[:, 0:128]
    txtc[:, :, 1, :] = txt[:, 128:256]
    txtc[:, :44, 2, :] = txt[:, 256:300]
    txtc = txtc.reshape(B, 128, 60).astype(NP_BF)
    txtT = np.ascontiguousarray(txt.transpose(0, 2, 1)).astype(NP_BF)

    # GN selectors + affine columns
    cidx = np.arange(128)
    gsel1 = np.zeros((128, 32), f)
    gsel1[cidx, cidx // 4] = 0.25
    gsel1T = np.zeros((32, 128), f)
    gsel1T[cidx // 4, cidx] = 1.0
    gsel2 = np.zeros((128, 2, 16), f)
    gsel2T = np.zeros((16, 2, 128), f)
    for mb in range(2):
        g = cidx // 8
        gsel2[cidx, mb, g] = 0.125
        gsel2T[g, mb, cidx] = 1.0
    gsel2 = gsel2.reshape(128, 32)
    gsel2T = gsel2T.reshape(16, 256)

    sv1 = np.zeros((128, 2), f)
    sv1[:, 0] = np.asarray(inputs["gn1_g"], f) * S2A
    sv1[:, 1] = np.asarray(inputs["gn1_b"], f) * S2A
    b2 = np.asarray(inputs["conv2_b"], f)
    sv2 = np.zeros((128, 6), f)
    sv2[:, 0:2] = (np.asarray(inputs["gn2_g"], f) * SX).reshape(2, 128).T
    sv2[:, 2:4] = (np.asarray(inputs["gn2_b"], f) * SX).reshape(2, 128).T
    sv2[:, 4:6] = (b2 * S2).reshape(2, 128).T

    shared = {
        "w1v": w1v, "w1s": w1s, "w2": w2,
        "rwh": rwh.reshape(128, 2 * DS), "rwl": rwl.reshape(128, 2 * DS),
        "rwc2": rwc2.reshape(3, 2 * DS),
        "kw": kw.reshape(128, 2 * VF), "kwc2": kwc2.reshape(3, 2 * VF),
        "vwh": vwh.reshape(128, 2 * VF), "vwl": vwl.reshape(128, 2 * VF),
        "vwc2": vwc2.reshape(3, 2 * VF),
        "coordp": coordp, "onesp": onesp,
        "ones20": np.ones((20, 128), NP_BF),
        "gsel1": gsel1, "gsel1T": gsel1T, "gsel2": gsel2, "gsel2T": gsel2T,
        "svec1": sv1, "svec2": sv2,
        "zpad": np.zeros((128, 34 * 34), NP_F8),
    }

    in_maps = []
    for c in range(N_CORES):
        sl = slice(c * SPC, (c + 1) * SPC)
        m = dict(shared)
        m["xv"] = np.ascontiguousarray(xv[sl])
        m["xsp"] = np.ascontiguousarray(xsp[sl])
        m["txtc"] = np.ascontiguousarray(txtc[sl])
        m["txtT"] = np.ascontiguousarray(txtT[sl])
        m["qw"] = np.ascontiguousarray(qw_all[sl].reshape(SPC, 128, 2 * VF))
        m["qwc2"] = np.ascontiguousarray(qwc2_all[sl].reshape(SPC, 3, 2 * VF))
        in_maps.append(m)
    return in_maps


def get_program():
    if "nc" not in _PROGRAM_CACHE:
        _PROGRAM_CACHE["nc"] = build_program()
    return _PROGRAM_CACHE["nc"]


def kernel(**inputs) -> np.ndarray:
    nc = get_program()
    in_maps = _prep_inputs(inputs)
    res = run_bass_kernel_spmd(nc, in_maps, list(range(N_CORES)))
    outs = [res.results[c]["out"].astype(np.float32) for c in range(N_CORES)]
    full = np.concatenate(outs, axis=0).reshape(32, 558, 32, 32)
    return full.astype(np.float32)


# revision 17
# speedup vs baseline: 1.4415x; 1.0771x over previous
"""Trainium2 Bass kernel for nn_AttentionFusion — fp8-DoubleRow rewrite.

Sharding: pure data parallelism over batch (32 samples -> 8 cores x 4
samples), weights replicated.

All heavy matmuls run as fp8e4m3 DoubleRow (2 k-tiles per instruction,
0.5 cycles/row — 4x the f32r row rate for K-chunked contractions), with
power-of-2 scale management so every tensor sits in e4m3's healthy range.
Precision placement (validated vs the jax reference, rel_l2 ~1.05e-2 vs
the 2e-2 gate):
  conv1        acts single fp8 x weights single fp8 (tap-chunk pairs)
  conv2        acts single fp8 (y1pad) x weights hi+lo exact pairs
  vsr          xfeat single fp8 x reduce_w hi+lo exact pairs
  E_t/rb/wt    bf16 (text branch is error-dominant)
  k,q          single fp8 both sides (mod folded into q_w host-side)
  v            xfeat fp8 x v_w hi+lo exact (v_w drives a systematic
               mean-activation error if quantized single)
  S'/E/wv      fp8 throughout; softmax denominators via an fp8 ones-matmul
               (rbD) + DVE reciprocal, numerator/denominator share E
  output       bf16 (cast to f32 on host)

Softmax denominators use nc.vector.reciprocal + TT-mult (no Ln/Exp chains,
no partition-move DMAs).
"""

import os
import sys
from contextlib import ExitStack

for _p in ("/opt/trn_rl_repo",):
    if _p not in sys.path and os.path.isdir(_p):
        sys.path.insert(0, _p)

import numpy as np
import ml_dtypes

import concourse.bacc as bacc
import concourse.mybir as mybir
import concourse.tile as tile
from concourse.bass import ts
from concourse.bass_utils import run_bass_kernel_spmd

F32 = mybir.dt.float32
BF16 = mybir.dt.bfloat16
FP8 = mybir.dt.float8e4
AF = mybir.ActivationFunctionType
ALU = mybir.AluOpType
DR = mybir.MatmulPerfMode.DoubleRow

NP_F8 = ml_dtypes.float8_e4m3
NP_BF = ml_dtypes.bfloat16

N_CORES = 8
SPC = 4  # samples per core
HW = 1024
DS = 300
VF = 258
EPS = 1e-5

# power-of-2 scale plan
SXV = 16.0      # conv1 input activations
FW1 = 512.0     # conv1 weights               -> y1 psum x 8192
S2A = 16.0      # y1pad storage (folded into gn1 affine)
FW2 = 512.0     # conv2 weights               -> y2 psum x 8192
SX = 16.0       # xfeat storage (folded into gn2 affine)
SC = 64.0       # coords + ones row storage
FRW = 512.0     # reduce_w                    -> vsr psum x 8192
FKW = 2048.0    # k_w                         -> k  psum x 32768
FQW = 512.0     # q_w*mod^2                   -> q' psum x 8192
FVW = 2048.0    # v_w                         -> v  psum x 32768
SKQV_EVAC = 1.0 / 512.0   # k->x64, q->x16, v->x64 storage scales
SV = 64.0       # v storage scale == ones128 value
S1 = SXV * FW1            # 8192
S2 = S2A * FW2            # 8192
SVSR = SX * FRW           # 8192
EPS1 = EPS * S1 * S1
EPS2 = EPS * S2 * S2
ESC_V = (1.0 / float(np.sqrt(VF))) / 1024.0
ESC_T = (1.0 / float(np.sqrt(DS))) / SVSR

TAPS = [(ty, tx) for ty in range(3) for tx in range(3)]

_PROGRAM_CACHE = {}


def _patch_act_tables():
    """Keep Exp/Ln/Relu/Identity/Copy pinned to one act table set so the
    act-table-load pass doesn't thrash between sets."""
    import concourse.bacc as _bacc
    import concourse.hw_specs as _hw

    if getattr(_bacc, "_act_tables_patched", False):
        return
    _orig = _hw.get_activation_tables
    mine = {AF.Exp, AF.Ln, AF.Relu, AF.Identity, AF.Copy}

    def patched(module_arch):
        tabs = _orig(module_arch)
        out = {}
        for name, funcs in tabs.items():
            if name == "natural_log_exp_and_others" or not (mine & funcs):
                out[name] = funcs
            else:
                out[name] = funcs - mine
        return out

    _bacc.get_activation_tables = patched
    _bacc._act_tables_patched = True


def _patch_drain_barrier():
    """Split the kernel-tail drain's per-proc sem waits across engines."""
    import concourse.tile as tile_mod
    from concourse.vector_clock import ScopedClock

    if getattr(tile_mod, "_drain_patched", False):
        return

    def _patched(self, tick_clock, wait_clock):
        nc = self.nc
        drain_inst = nc.sync.drain()
        wait_clock.add_sem_waits(
            drain_inst.ins, ScopedClock({None: tick_clock.global_clock})
        )
        si = drain_inst.ins.sync_info
        waits = list(si.on_wait or [])
        if len(waits) > 1:
            si.on_wait = waits[:1]
            engines = [nc.sync, nc.scalar, nc.vector, nc.tensor, nc.gpsimd]
            for i in range(1, len(waits)):
                extra = engines[i % len(engines)].drain()
                extra.ins.sync_info = mybir.SyncInfo(
                    on_wait=[waits[i]], on_update=[]
                )
        nc.all_engine_barrier()
        assert self.sems is not None
        popped = nc._tile_sem_poison_stack.pop()
        assert popped is self._sem_poison
        nc.clear_and_free_semaphores(list(self.sems.allocated().values()))
        nc.all_engine_barrier()

    tile_mod.TileContext._drain_and_barrier = _patched
    tile_mod._drain_patched = True


def build_program():
    _patch_act_tables()
    _patch_drain_barrier()
    nc = bacc.Bacc()
    dt = F32

    # ---------------- DRAM declarations ----------------
    d_xv = nc.dram_tensor("xv", [SPC, 128, 2 * 34 * 34], FP8, kind="ExternalInput")
    d_xsp = nc.dram_tensor("xsp", [SPC, 74, 2 * HW], FP8, kind="ExternalInput")
    d_txtc = nc.dram_tensor("txtc", [SPC, 128, 3 * 20], BF16, kind="ExternalInput")
    d_txtT = nc.dram_tensor("txtT", [SPC, 20, DS], BF16, kind="ExternalInput")
    d_qw = nc.dram_tensor("qw", [SPC, 128, 2 * VF], FP8, kind="ExternalInput")
    d_qwc2 = nc.dram_tensor("qwc2", [SPC, 3, 2 * VF], FP8, kind="ExternalInput")
    d_w1v = nc.dram_tensor("w1v", [128, 9 * 2 * 128], FP8, kind="ExternalInput")
    d_w1s = nc.dram_tensor("w1s", [74, 2 * 128], FP8, kind="ExternalInput")
    d_w2 = nc.dram_tensor("w2", [128, 9 * 2 * 2 * 128], FP8, kind="ExternalInput")
    d_rwh = nc.dram_tensor("rwh", [128, 2 * DS], FP8, kind="ExternalInput")
    d_rwl = nc.dram_tensor("rwl", [128, 2 * DS], FP8, kind="ExternalInput")
    d_rwc2 = nc.dram_tensor("rwc2", [3, 2 * DS], FP8, kind="ExternalInput")
    d_kw = nc.dram_tensor("kw", [128, 2 * VF], FP8, kind="ExternalInput")
    d_kwc2 = nc.dram_tensor("kwc2", [3, 2 * VF], FP8, kind="ExternalInput")
    d_vwh = nc.dram_tensor("vwh", [128, 2 * VF], FP8, kind="ExternalInput")
    d_vwl = nc.dram_tensor("vwl", [128, 2 * VF], FP8, kind="ExternalInput")
    d_vwc2 = nc.dram_tensor("vwc2", [3, 2 * VF], FP8, kind="ExternalInput")
    d_coordp = nc.dram_tensor("coordp", [3, 2 * HW], FP8, kind="ExternalInput")
    d_onesp = nc.dram_tensor("onesp", [128, 2 * 128], FP8, kind="ExternalInput")
    d_ones20 = nc.dram_tensor("ones20", [20, 128], BF16, kind="ExternalInput")
    d_gs1 = nc.dram_tensor("gsel1", [128, 32], dt, kind="ExternalInput")
    d_gs1T = nc.dram_tensor("gsel1T", [32, 128], dt, kind="ExternalInput")
    d_gs2 = nc.dram_tensor("gsel2", [128, 2 * 16], dt, kind="ExternalInput")
    d_gs2T = nc.dram_tensor("gsel2T", [16, 2 * 128], dt, kind="ExternalInput")
    d_sv1 = nc.dram_tensor("svec1", [128, 2], dt, kind="ExternalInput")
    d_sv2 = nc.dram_tensor("svec2", [128, 6], dt, kind="ExternalInput")
    d_zpad = nc.dram_tensor("zpad", [128, 34 * 34], FP8, kind="ExternalInput")
    d_out = nc.dram_tensor("out", [SPC, 558, HW], BF16, kind="ExternalOutput")

    with tile.TileContext(nc) as tc, ExitStack() as ctx:
        wpool = ctx.enter_context(tc.tile_pool(name="weights", bufs=1))
        inpool = ctx.enter_context(tc.tile_pool(name="inputs", bufs=3))
        spool = ctx.enter_context(tc.tile_pool(name="work", bufs=1))
        opool = ctx.enter_context(tc.tile_pool(name="outs", bufs=1))
        epool = ctx.enter_context(tc.tile_pool(name="estream", bufs=3))
        pbig = ctx.enter_context(tc.tile_pool(name="pbig", bufs=3, space="PSUM"))
        pS = ctx.enter_context(tc.tile_pool(name="pS", bufs=2, space="PSUM"))

        # ---------- conv1 weights + sample-0 inputs first ----------
        sv1 = wpool.tile([128, 2], dt, name="sv1")
        nc.sync.dma_start(sv1[:, :], d_sv1[:, :])
        g1_ap, b1_ap = sv1[:, 0:1], sv1[:, 1:2]
        w1v_sb = wpool.tile([128, 9, 2, 128], FP8, name="w1v_sb")
        nc.sync.dma_start(
            w1v_sb[:, :, :, :].rearrange("p a b c -> p (a b c)"), d_w1v[:, :]
        )
        w1s_sb = wpool.tile([74, 2, 128], FP8, name="w1s_sb")
        nc.sync.dma_start(
            w1s_sb[:, :, :].rearrange("p a b -> p (a b)"), d_w1s[:, :]
        )
        gs1_sb = wpool.tile([128, 32], dt, name="gs1_sb")
        nc.sync.dma_start(gs1_sb[:, :], d_gs1[:, :])
        gs1T_sb = wpool.tile([32, 128], dt, name="gs1T_sb")
        nc.sync.dma_start(gs1T_sb[:, :], d_gs1T[:, :])

        def load_inputs(s):
            xv = inpool.tile([128, 2, 34, 34], FP8, name="xv")
            nc.sync.dma_start(
                xv[:, :, :, :].rearrange("p a h w -> p (a h w)"), d_xv[s]
            )
            xsp = inpool.tile([74, 2, HW], FP8, name="xsp")
            nc.sync.dma_start(
                xsp[:, :, :].rearrange("p a b -> p (a b)"), d_xsp[s]
            )
            txtc = inpool.tile([128, 3, 20], BF16, name="txtc")
            nc.sync.dma_start(
                txtc[:, :, :].rearrange("p a b -> p (a b)"), d_txtc[s]
            )
            txtT = inpool.tile([20, DS], BF16, name="txtT")
            nc.sync.dma_start(txtT[:, :], d_txtT[s])
            qw = inpool.tile([128, 2, VF], FP8, name="qw")
            nc.sync.dma_start(
                qw[:, :, :].rearrange("p a b -> p (a b)"), d_qw[s]
            )
            qwc2 = inpool.tile([3, 2, VF], FP8, name="qwc2")
            nc.sync.dma_start(
                qwc2[:, :, :].rearrange("p a b -> p (a b)"), d_qwc2[s]
            )
            return dict(xv=xv, xsp=xsp, txtc=txtc, txtT=txtT, qw=qw, qwc2=qwc2)

        preloaded = load_inputs(0)

        # ---------- remaining weights ----------
        w2_sb = wpool.tile([128, 9, 2, 2, 128], FP8, name="w2_sb")
        nc.sync.dma_start(
            w2_sb[:, :, :, :, :].rearrange("p a b c d -> p (a b c d)"), d_w2[:, :]
        )
        sv2 = wpool.tile([128, 6], dt, name="sv2")
        nc.sync.dma_start(sv2[:, :], d_sv2[:, :])
        g2_ap, b2_ap, b2s_ap = sv2[:, 0:2], sv2[:, 2:4], sv2[:, 4:6]
        gs2_sb = wpool.tile([128, 2, 16], dt, name="gs2_sb")
        nc.sync.dma_start(
            gs2_sb[:, :, :].rearrange("p a b -> p (a b)"), d_gs2[:, :]
        )
        gs2T_sb = wpool.tile([16, 2, 128], dt, name="gs2T_sb")
        nc.sync.dma_start(
            gs2T_sb[:, :, :].rearrange("p a b -> p (a b)"), d_gs2T[:, :]
        )
        rwh_sb = wpool.tile([128, 2, DS], FP8, name="rwh_sb")
        nc.sync.dma_start(rwh_sb[:, :, :].rearrange("p a b -> p (a b)"), d_rwh[:, :])
        rwl_sb = wpool.tile([128, 2, DS], FP8, name="rwl_sb")
        nc.sync.dma_start(rwl_sb[:, :, :].rearrange("p a b -> p (a b)"), d_rwl[:, :])
        rwc2_sb = wpool.tile([3, 2, DS], FP8, name="rwc2_sb")
        nc.sync.dma_start(rwc2_sb[:, :, :].rearrange("p a b -> p (a b)"), d_rwc2[:, :])
        kw_sb = wpool.tile([128, 2, VF], FP8, name="kw_sb")
        nc.sync.dma_start(kw_sb[:, :, :].rearrange("p a b -> p (a b)"), d_kw[:, :])
        kwc2_sb = wpool.tile([3, 2, VF], FP8, name="kwc2_sb")
        nc.sync.dma_start(kwc2_sb[:, :, :].rearrange("p a b -> p (a b)"), d_kwc2[:, :])
        vwh_sb = wpool.tile([128, 2, VF], FP8, name="vwh_sb")
        nc.sync.dma_start(vwh_sb[:, :, :].rearrange("p a b -> p (a b)"), d_vwh[:, :])
        vwl_sb = wpool.tile([128, 2, VF], FP8, name="vwl_sb")
        nc.sync.dma_start(vwl_sb[:, :, :].rearrange("p a b -> p (a b)"), d_vwl[:, :])
        vwc2_sb = wpool.tile([3, 2, VF], FP8, name="vwc2_sb")
        nc.sync.dma_start(vwc2_sb[:, :, :].rearrange("p a b -> p (a b)"), d_vwc2[:, :])
        coordp_sb = wpool.tile([3, 2, HW], FP8, name="coordp_sb")
        nc.sync.dma_start(
            coordp_sb[:, :, :].rearrange("p a b -> p (a b)"), d_coordp[:, :]
        )
        onesp_sb = wpool.tile([128, 2, 128], FP8, name="onesp_sb")
        nc.sync.dma_start(
            onesp_sb[:, :, :].rearrange("p a b -> p (a b)"), d_onesp[:, :]
        )
        ones20_sb = wpool.tile([20, 128], BF16, name="ones20_sb")
        nc.sync.dma_start(ones20_sb[:, :], d_ones20[:, :])

        # persistent zero-padded conv2 input (border stays zero forever)
        y1pad = wpool.tile([128, 1, 34, 34], FP8, name="y1pad")
        nc.sync.dma_start(
            y1pad[:, :, :, :].rearrange("p a h w -> p (a h w)"), d_zpad[:, :]
        )
        # persistent k/q tail tiles; plane 1 must stay zero (S' tail pairs)
        ktail = wpool.tile([2, 2, HW], FP8, name="ktail")
        nc.vector.memset(ktail[:, :, :].rearrange("p a b -> p (a b)"), 0)
        qtail = wpool.tile([2, 2, HW], FP8, name="qtail")
        nc.vector.memset(qtail[:, :, :].rearrange("p a b -> p (a b)"), 0)

        # ---------------- helpers ----------------
        def group_norm_finish(gstat_ps, cb_ps, gamma_ap, beta_ap, gsT_ap,
                              eps_s, groups, tag, bias_col=None):
            """gstat_ps: [G,2] PSUM (mean, E[x2]) per group (scaled domain);
            cb_ps: [128,2] PSUM for the broadcast-back.  Returns sc [128,2]
            SBUF: col0 = scale, col1 = bias for act(relu, psum-input).
            bias_col: [128,1] host column of b*S to subtract from the
            broadcast channel mean (act input psum is un-biased)."""
            gb = spool.tile([groups, 4], dt, name=f"gb_{tag}")
            nc.vector.tensor_copy(gb[:, 0:1], gstat_ps[:, 0:1])
            nc.vector.tensor_tensor(gb[:, 3:4], gb[:, 0:1], gb[:, 0:1], ALU.mult)
            nc.vector.tensor_tensor(
                gb[:, 1:2], gstat_ps[:, 1:2], gb[:, 3:4], ALU.subtract
            )
            nc.vector.tensor_scalar_add(gb[:, 1:2], gb[:, 1:2], float(eps_s))
            nc.scalar.activation(gb[:, 2:3], gb[:, 1:2], AF.Ln)
            nc.scalar.activation(gb[:, 1:2], gb[:, 2:3], AF.Exp, scale=-0.5)
            nc.tensor.matmul(cb_ps, gsT_ap, gb[:, 0:2], start=True, stop=True)
            sc = spool.tile([128, 3], dt, name=f"sc_{tag}")
            nc.vector.tensor_tensor(sc[:, 0:1], gamma_ap, cb_ps[:, 1:2], ALU.mult)
            if bias_col is not None:
                nc.vector.tensor_tensor(
                    sc[:, 2:3], cb_ps[:, 0:1], bias_col, ALU.subtract
                )
                mu_ap = sc[:, 2:3]
            else:
                mu_ap = cb_ps[:, 0:1]
            nc.vector.tensor_tensor(sc[:, 1:2], mu_ap, sc[:, 0:1], ALU.mult)
            nc.vector.tensor_tensor(sc[:, 1:2], beta_ap, sc[:, 1:2], ALU.subtract)
            return sc

        def channel_stats(ps_a, ps_b, tag, bias_col=None):
            """Two [128,512] PSUM halves -> st2 [128,2] = (mean_b, E_b[x^2])."""
            bnst = spool.tile([128, 2, 6], dt, name=f"bnst_{tag}")
            nc.vector.bn_stats(bnst[:, 0, :], ps_a)
            nc.vector.bn_stats(bnst[:, 1, :], ps_b)
            mv = spool.tile([128, 2], dt, name=f"mv_{tag}")
            nc.vector.bn_aggr(mv[:, :], bnst[:, :, :])
            st2 = spool.tile([128, 2], dt, name=f"st2_{tag}")
            if bias_col is not None:
                nc.vector.tensor_tensor(st2[:, 0:1], mv[:, 0:1], bias_col, ALU.add)
            else:
                nc.vector.tensor_copy(st2[:, 0:1], mv[:, 0:1])
            nc.vector.tensor_tensor(st2[:, 1:2], st2[:, 0:1], st2[:, 0:1], ALU.mult)
            nc.vector.tensor_tensor(st2[:, 1:2], st2[:, 1:2], mv[:, 1:2], ALU.add)
            return st2

        # ---------------- attention, jp-granular for interleaving ----------
        def attn_mm(sd, s, ni, jps):
            """S' + exp + wv accumulation for jp groups of one n-half."""
            kT, qT, vsb = sd["kT"], sd["qT"], sd["vsb"]
            if 0 in jps:
                sd["wv01"] = pbig.tile([128, 1024], F32, tag="big",
                                       name=f"wv01_{s}_{ni}")
                sd["wvD"] = pbig.tile([128, 1024], F32, tag="big",
                                      name=f"wvD_{s}_{ni}")
            wv01, wvD = sd["wv01"], sd["wvD"]
            for jp in jps:
                Ep = epool.tile([128, 2, 512], FP8, tag="E", name=f"E_{s}_{ni}_{jp}")
                for jj in range(2):
                    j = 2 * jp + jj
                    sps = pS.tile([128, 512], F32, tag="ps", name=f"sps_{s}_{ni}_{j}")
                    nc.tensor.matmul(
                        sps[:, :], qT[:, :, ts(j, 128)], kT[:, :, ts(ni, 512)],
                        start=True, stop=False, perf_mode=DR,
                    )
                    nc.tensor.matmul(
                        sps[:, :], qtail[:, :, ts(j, 128)], ktail[:, :, ts(ni, 512)],
                        start=False, stop=True, perf_mode=DR,
                    )
                    nc.scalar.activation(Ep[:, jj, :], sps[:, :], AF.Exp, scale=ESC_V)
                st, sp = (jp == 0), (jp == 3)
                nc.tensor.matmul(
                    wv01[:, 0:512], vsb[:, jp, :, 0:128], Ep[:, :, :],
                    start=st, stop=sp, perf_mode=DR,
                )
                nc.tensor.matmul(
                    wv01[:, 512:1024], vsb[:, jp, :, 128:256], Ep[:, :, :],
                    start=st, stop=sp, perf_mode=DR,
                )
                nc.tensor.matmul(
                    wvD[0:2, 512:1024], vsb[:, jp, :, 256:258], Ep[:, :, :],
                    start=st, stop=sp, perf_mode=DR,
                )
                nc.tensor.matmul(
                    wvD[:, 0:512], onesp_sb[:, :, :], Ep[:, :, :],
                    start=st, stop=sp, perf_mode=DR,
                )

        def attn_fin(sd, s, ni):
            wv01, wvD, wvout = sd["wv01"], sd["wvD"], sd["wvout"]
            rbc = spool.tile([128, 512], dt, name=f"rbcv_{ni}")
            nc.vector.reciprocal(rbc[:, :], wvD[:, 0:512])
            nc.vector.tensor_tensor(
                wvout[:, 0, ts(ni, 512)], wv01[:, 0:512], rbc[:, :], ALU.mult
            )
            nc.vector.tensor_tensor(
                wvout[:, 1, ts(ni, 512)], wv01[:, 512:1024], rbc[:, :], ALU.mult
            )
            nc.vector.tensor_tensor(
                wvout[0:2, 2, ts(ni, 512)], wvD[0:2, 512:1024], rbc[0:2, :], ALU.mult
            )

        def attn_out_dma(sd, s):
            wvout = sd["wvout"]
            nc.gpsimd.dma_start(d_out[s, 0:128, :], wvout[:, 0, :])
            nc.gpsimd.dma_start(d_out[s, 128:256, :], wvout[:, 1, :])
            nc.gpsimd.dma_start(d_out[s, 256:258, :], wvout[:2, 2, :])

        # ---------------- conv helpers ----------------
        def emit_conv1(s, io):
            xv, xsp = io["xv"], io["xsp"]
            ps = pbig.tile([128, 1024], F32, tag="big", name=f"c1ps_{s}")
            for ni in range(2):
                h0 = ni * 16
                for t, (ty, tx) in enumerate(TAPS):
                    nc.tensor.matmul(
                        ps[:, ts(ni, 512)],
                        w1v_sb[:, t, :, :],
                        xv[:, :, ty + h0 : ty + h0 + 16, tx : tx + 32],
                        start=(t == 0), stop=False, perf_mode=DR,
                    )
                nc.tensor.matmul(
                    ps[:, ts(ni, 512)], w1s_sb[:, :, :], xsp[:, :, ts(ni, 512)],
                    start=False, stop=True, perf_mode=DR,
                )
            return ps

        def emit_conv2_mb(s, mb):
            ps = pbig.tile([128, 1024], F32, tag="big", name=f"c2ps_{s}_{mb}")
            for ni in range(2):
                h0 = ni * 16
                for t, (ty, tx) in enumerate(TAPS):
                    nc.tensor.matmul(
                        ps[:, ts(ni, 512)],
                        w2_sb[:, t, mb, :, :],
                        y1pad[:, 0:1, ty + h0 : ty + h0 + 16, tx : tx + 32]
                        .to_broadcast((128, 2, 16, 32)),
                        start=(t == 0), stop=(t == 8), perf_mode=DR,
                    )
            return ps

        def gn2_finish_relu(s, mb, ps2, st2b, xfeat):
            gt2 = pS.tile([128, 4], dt, tag="ps", name=f"gst2_{s}_{mb}")
            nc.tensor.matmul(
                gt2[:16, 0:2], gs2_sb[:, mb, :], st2b[:, :], start=True, stop=True
            )
            sc2 = group_norm_finish(
                gt2[:16, 0:2], gt2[:, 2:4],
                g2_ap[:, mb : mb + 1], b2_ap[:, mb : mb + 1],
                gs2T_sb[:, mb, :], EPS2, 16, f"gn2_{s}_{mb}",
                bias_col=b2s_ap[:, mb : mb + 1],
            )
            nc.scalar.activation(
                xfeat[:, mb, :], ps2[:, :],
                AF.Relu, bias=sc2[:, 1:2], scale=sc2[:, 0:1],
            )

        # ---------------- back-phase pieces (sample b = B["s"]) ----------
        def back_vsrT(B):
            b, xfeat = B["s"], B["xfeat"]
            vsrT = spool.tile([128, 3, HW], BF16, name="vsrT")
            B["vsrT"] = vsrT
            for mb in range(3):
                mr = DSCH[mb]
                m0 = mb * 128
                ps = pbig.tile([128, 1024], F32, tag="big", name=f"vsr_{b}_{mb}")
                for ni in range(2):
                    nc.tensor.matmul(
                        ps[:mr, ts(ni, 512)], rwh_sb[:, :, m0 : m0 + mr],
                        xfeat[:, :, ts(ni, 512)],
                        start=True, stop=False, perf_mode=DR,
                    )
                    nc.tensor.matmul(
                        ps[:mr, ts(ni, 512)], rwl_sb[:, :, m0 : m0 + mr],
                        xfeat[:, :, ts(ni, 512)],
                        start=False, stop=False, perf_mode=DR,
                    )
                    nc.tensor.matmul(
                        ps[:mr, ts(ni, 512)], rwc2_sb[:, :, m0 : m0 + mr],
                        coordp_sb[:, :, ts(ni, 512)],
                        start=False, stop=True, perf_mode=DR,
                    )
                if mb % 2 == 0:
                    nc.vector.tensor_copy(vsrT[:mr, mb, :], ps[:mr, :])
                else:
                    nc.scalar.activation(vsrT[:mr, mb, :], ps[:mr, :], AF.Copy)

        def back_text1(B):
            b, txtc, vsrT = B["s"], B["txtc"], B["vsrT"]
            E_t = spool.tile([20, HW], BF16, name="E_t")
            rbc_t = spool.tile([128, HW], dt, name="rbc_t")
            B["E_t"], B["rbc_t"] = E_t, rbc_t
            pet = pbig.tile([20, 1024], F32, tag="big", name=f"et_{b}")
            for ni in range(2):
                for kc in range(3):
                    kr = DSCH[kc]
                    nc.tensor.matmul(
                        pet[:, ts(ni, 512)], txtc[:kr, kc, :],
                        vsrT[:kr, kc, ts(ni, 512)],
                        start=(kc == 0), stop=(kc == 2),
                    )
            nc.scalar.activation(E_t[:, :], pet[:, :], AF.Exp, scale=ESC_T)
            prb = pbig.tile([128, 1024], F32, tag="big", name=f"rbt_{b}")
            for ni in range(2):
                nc.tensor.matmul(
                    prb[:, ts(ni, 512)], ones20_sb[:, :], E_t[:, ts(ni, 512)],
                    start=True, stop=True,
                )
            nc.vector.reciprocal(rbc_t[:, :], prb[:, :])

        def back_wt(B):
            b, txtT, E_t, rbc_t = B["s"], B["txtT"], B["E_t"], B["rbc_t"]
            wtout = opool.tile([128, 3, HW], BF16, name="wtout")
            WT_ROWS = (128, 128, 44)
            for mb in range(3):
                ps = pbig.tile([128, 1024], F32, tag="big", name=f"wt_{b}_{mb}")
                mr = WT_ROWS[mb]
                for ni in range(2):
                    nc.tensor.matmul(
                        ps[:mr, ts(ni, 512)],
                        txtT[:, mb * 128 : mb * 128 + mr],
                        E_t[:, ts(ni, 512)],
                        start=True, stop=True,
                    )
                nc.vector.tensor_tensor(
                    wtout[:mr, mb, :], ps[:mr, :], rbc_t[:mr, :], ALU.mult
                )
            nc.gpsimd.dma_start(d_out[b, 258:386, :], wtout[:, 0, :])
            nc.gpsimd.dma_start(d_out[b, 386:514, :], wtout[:, 1, :])
            nc.gpsimd.dma_start(d_out[b, 514:558, :], wtout[:44, 2, :])

        def back_kq(B):
            b, xfeat, qw_s, qwc2_s = B["s"], B["xfeat"], B["qw"], B["qwc2"]
            kT = spool.tile([128, 2, HW], FP8, name="kT")
            qT = spool.tile([128, 2, HW], FP8, name="qT")
            B["kT"], B["qT"] = kT, qT
            for wi, (w_sb, wc2_sb, dstT) in enumerate(
                ((kw_sb, kwc2_sb, kT), (qw_s, qwc2_s, qT))
            ):
                for mb in range(2):
                    ps = pbig.tile([128, 1024], F32, tag="big",
                                   name=f"kq_{b}_{wi}_{mb}")
                    for ni in range(2):
                        nc.tensor.matmul(
                            ps[:, ts(ni, 512)], w_sb[:, :, ts(mb, 128)],
                            xfeat[:, :, ts(ni, 512)],
                            start=True, stop=False, perf_mode=DR,
                        )
                        nc.tensor.matmul(
                            ps[:, ts(ni, 512)], wc2_sb[:, :, ts(mb, 128)],
                            coordp_sb[:, :, ts(ni, 512)],
                            start=False, stop=True, perf_mode=DR,
                        )
                    if (wi + mb) % 2 == 0:
                        nc.scalar.activation(
                            dstT[:, mb, :], ps[:, :], AF.Copy, scale=SKQV_EVAC
                        )
                    else:
                        nc.vector.tensor_scalar_mul(
                            dstT[:, mb, :], ps[:, :], SKQV_EVAC
                        )
            pkt = pbig.tile([2, 1024], F32, tag="big", name=f"ktp_{b}")
            pqt = pbig.tile([2, 1024], F32, tag="big", name=f"qtp_{b}")
            for ni in range(2):
                nc.tensor.matmul(
                    pkt[:, ts(ni, 512)], kw_sb[:, :, 256:258],
                    xfeat[:, :, ts(ni, 512)],
                    start=True, stop=False, perf_mode=DR,
                )
                nc.tensor.matmul(
                    pkt[:, ts(ni, 512)], kwc2_sb[:, :, 256:258],
                    coordp_sb[:, :, ts(ni, 512)],
                    start=False, stop=True, perf_mode=DR,
                )
                nc.tensor.matmul(
                    pqt[:, ts(ni, 512)], qw_s[:, :, 256:258],
                    xfeat[:, :, ts(ni, 512)],
                    start=True, stop=False, perf_mode=DR,
                )
                nc.tensor.matmul(
                    pqt[:, ts(ni, 512)], qwc2_s[:, :, 256:258],
                    coordp_sb[:, :, ts(ni, 512)],
                    start=False, stop=True, perf_mode=DR,
                )
            nc.vector.tensor_scalar_mul(ktail[:, 0, :], pkt[:, :], SKQV_EVAC)
            nc.scalar.activation(
                qtail[:, 0, :], pqt[:, :], AF.Copy, scale=SKQV_EVAC
            )

        def back_v(B):
            b, xfeat = B["s"], B["xfeat"]
            vsb = spool.tile([128, 4, 2, VF], FP8, name="vsb")
            B["vsb"] = vsb
            for jp in range(4):
                ps = pbig.tile([128, 1024], F32, tag="big", name=f"v_{b}_{jp}")
                psv = ps[:, :].rearrange("p (a b) -> p a b", a=2)
                for jj in range(2):
                    j = 2 * jp + jj
                    nc.tensor.matmul(
                        psv[:, jj, 0:VF], xfeat[:, :, ts(j, 128)], vwh_sb[:, :, :],
                        start=True, stop=False, perf_mode=DR,
                    )
                    nc.tensor.matmul(
                        psv[:, jj, 0:VF], xfeat[:, :, ts(j, 128)], vwl_sb[:, :, :],
                        start=False, stop=False, perf_mode=DR,
                    )
                    nc.tensor.matmul(
                        psv[:, jj, 0:VF], coordp_sb[:, :, ts(j, 128)],
                        vwc2_sb[:, :, :],
                        start=False, stop=True, perf_mode=DR,
                    )
                if jp % 2 == 0:
                    nc.vector.tensor_scalar_mul(
                        vsb[:, jp, :, :], psv[:, :, 0:VF], SKQV_EVAC
                    )
                else:
                    nc.scalar.activation(
                        vsb[:, jp, :, :], psv[:, :, 0:VF], AF.Copy,
                        scale=SKQV_EVAC,
                    )
            return {
                "kT": B["kT"], "qT": B["qT"], "vsb": vsb,
                "wvout": opool.tile([128, 3, HW], BF16, name="wvout"),
            }

        DSCH = (128, 128, 44)

        # ------- pipeline: FRONT(s) || BACK(s-1) || ATTN(s-2) -------
        A = None   # attention state (sample s-2)
        B = None   # back-phase state (sample s-1)
        ios = {0: preloaded}
        for s in range(SPC):
            io = ios.pop(s)
            if s + 1 < SPC:
                ios[s + 1] = load_inputs(s + 1)

            ps1 = emit_conv1(s, io)
            st2 = channel_stats(ps1[:, 0:512], ps1[:, 512:1024], f"gn1_{s}")
            if B is not None:
                back_vsrT(B)
            gt1 = pS.tile([128, 4], dt, tag="ps", name=f"gst1_{s}")
            nc.tensor.matmul(
                gt1[:32, 0:2], gs1_sb[:, :], st2[:, :], start=True, stop=True
            )
            sc1 = group_norm_finish(
                gt1[:32, 0:2], gt1[:, 2:4],
                g1_ap, b1_ap, gs1T_sb[:, :], EPS1, 32, f"gn1_{s}",
            )
            nc.scalar.activation(
                y1pad[:, 0, 1:33, 1:33],
                ps1[:, :].rearrange("p (h w) -> p h w", h=32),
                AF.Relu, bias=sc1[:, 1:2], scale=sc1[:, 0:1],
            )
            if B is not None:
                back_text1(B)
            if A is not None:
                attn_mm(A, A["s"], 0, [0, 1, 2, 3])
                attn_fin(A, A["s"], 0)

            xfeat = spool.tile([128, 2, HW], FP8, name=f"xfeat_{s % 2}")
            ps2a = emit_conv2_mb(s, 0)
            st2a = channel_stats(
                ps2a[:, 0:512], ps2a[:, 512:1024], f"gn2_{s}_0",
                bias_col=b2s_ap[:, 0:1],
            )
            if B is not None:
                back_wt(B)
            gn2_finish_relu(s, 0, ps2a, st2a, xfeat)
            if A is not None:
                attn_mm(A, A["s"], 1, [0, 1])
            ps2b = emit_conv2_mb(s, 1)
            st2b = channel_stats(
                ps2b[:, 0:512], ps2b[:, 512:1024], f"gn2_{s}_1",
                bias_col=b2s_ap[:, 1:2],
            )
            if A is not None:
                attn_mm(A, A["s"], 1, [2, 3])
                attn_fin(A, A["s"], 1)
                attn_out_dma(A, A["s"])
            gn2_finish_relu(s, 1, ps2b, st2b, xfeat)
            if B is not None:
                back_kq(B)
                A = back_v(B)
                A["s"] = B["s"]
            B = {"s": s, "xfeat": xfeat, "txtc": io["txtc"],
                 "txtT": io["txtT"], "qw": io["qw"], "qwc2": io["qwc2"]}

        # ------- drain: BACK(3) || ATTN(2), then ATTN(3) -------
        back_vsrT(B)
        back_text1(B)
        attn_mm(A, A["s"], 0, [0, 1, 2, 3])
        attn_fin(A, A["s"], 0)
        back_wt(B)
        attn_mm(A, A["s"], 1, [0, 1])
        attn_mm(A, A["s"], 1, [2, 3])
        attn_fin(A, A["s"], 1)
        attn_out_dma(A, A["s"])
        back_kq(B)
        A2 = back_v(B)
        A2["s"] = B["s"]
        attn_mm(A2, A2["s"], 0, [0, 1, 2, 3])
        attn_fin(A2, A2["s"], 0)
        attn_mm(A2, A2["s"], 1, [0, 1, 2, 3])
        attn_fin(A2, A2["s"], 1)
        attn_out_dma(A2, A2["s"])

    nc.finalize()
    return nc


def _q8(x):
    return np.asarray(x, np.float32).astype(NP_F8)


def _hilo(x):
    h = _q8(x)
    l = _q8(np.asarray(x, np.float32) - h.astype(np.float32))
    return h, l


def _prep_inputs(inputs):
    """Host-side marshalling: shard over batch, scale + quantize weights,
    im2col the spatial channels, fold mod^2 into q_w, hi/lo-split the
    error-critical weights."""
    f = np.float32
    video = np.asarray(inputs["video_feat"], f)
    spat = np.asarray(inputs["spatial_feat"], f)
    txt = np.asarray(inputs["txt"], f)
    B = video.shape[0]

    # conv1 inputs: video padded, x SXV, fp8, partition-major [128, 2, 1156]
    xv = np.zeros((B, 256, 34, 34), f)
    xv[:, :, 1:33, 1:33] = video * SXV
    xv = _q8(np.ascontiguousarray(
        xv.reshape(B, 2, 128, 34 * 34).transpose(0, 2, 1, 3)
    ).reshape(B, 128, 2 * 34 * 34))

    # spatial: host im2col (9 taps x 8 ch = 72 rows) + 2 bias-ones rows
    sp_pad = np.zeros((B, 8, 34, 34), f)
    sp_pad[:, :, 1:33, 1:33] = spat * SXV
    xsp_v = np.stack(
        [sp_pad[:, :, ty : ty + 32, tx : tx + 32] for (ty, tx) in TAPS], axis=1
    ).reshape(B, 72, HW)
    xsp = np.zeros((B, 74, 2, HW), f)
    xsp[:, :72, 0, :] = xsp_v
    xsp[:, 72, 0, :] = SXV
    xsp[:, 73, 0, :] = SXV
    xsp = _q8(xsp.reshape(B, 74, 2 * HW))

    # conv1 weights: [c_in(128), tap, chunk, c_out] x FW1 single fp8
    w1 = np.asarray(inputs["conv1_w"], f)
    w1v9 = w1[:, :256].transpose(2, 3, 1, 0).reshape(9, 2, 128, 128)  # t,c,ci,co
    w1v = _q8(np.ascontiguousarray(
        w1v9.transpose(2, 0, 1, 3)).reshape(128, 9 * 2 * 128) * FW1)
    # spatial weights + bias rows (hi/lo of b1*FW1, moving value SXV both)
    b1 = np.asarray(inputs["conv1_b"], f)
    w1s_rows = np.zeros((74, 2, 128), f)
    w1s_rows[:72, 0, :] = w1[:, 256:].transpose(2, 3, 1, 0).reshape(72, 128) * FW1
    bh = _q8(b1 * FW1).astype(f)
    w1s_rows[72, 0, :] = bh
    w1s_rows[73, 0, :] = b1 * FW1 - bh
    w1s = _q8(w1s_rows.reshape(74, 2 * 128))

    # conv2 weights: [c_in, tap, mb, hl, c_out], hi/lo exact, x FW2
    w29 = np.asarray(inputs["conv2_w"], f).transpose(2, 3, 1, 0).reshape(9, 128, 256)
    w2s = w29 * FW2
    w2h = _q8(w2s)
    w2l = _q8(w2s - w2h.astype(f))
    w2 = np.zeros((128, 9, 2, 2, 128), NP_F8)
    for mb in range(2):
        w2[:, :, mb, 0, :] = w2h.transpose(1, 0, 2)[:, :, mb * 128 : (mb + 1) * 128]
        w2[:, :, mb, 1, :] = w2l.transpose(1, 0, 2)[:, :, mb * 128 : (mb + 1) * 128]
    w2 = w2.reshape(128, 9 * 2 * 2 * 128)

    # reduce_w: [in, 2(chunk), 300]: feature rows x FRW hi/lo; coord+bias
    # chunk separately x (SVSR/SC)
    rw = np.asarray(inputs["reduce_w"], f)     # [300, 258]
    rb = np.asarray(inputs["reduce_b"], f)
    rwT = rw.T                                  # [258, 300]
    rw_feat = np.stack([rwT[0:128], rwT[128:256]], axis=1) * FRW  # [128,2,300]
    rwh, rwl = _hilo(rw_feat)
    rw_c2 = np.zeros((3, 2, DS), f)
    c2 = np.stack([rwT[256], rwT[257], rb], axis=0) * (SVSR / SC)  # [3,300]
    c2h = _q8(c2).astype(f)
    rw_c2[:, 0, :] = c2h
    rw_c2[:, 1, :] = c2 - c2h
    rwc2 = _q8(rw_c2)

    def kq_pack(wmat, bias, f_w, f_c2):
        """wmat [258,258] torch (out,in); returns main [128,2,258] and
        c2 [3,2,258] (plane1 zeros) fp8."""
        wT = np.asarray(wmat, f).T  # [in 258, out 258]
        main = np.stack([wT[0:128], wT[128:256]], axis=1) * f_w
        c2m = np.zeros((3, 2, VF), f)
        c2m[0:2, 0, :] = wT[256:258] * f_c2
        c2m[2, 0, :] = np.asarray(bias, f) * f_c2
        return _q8(main), _q8(c2m)

    # k coord rows: (coord*SC)*(w*g) = w_contrib*(SX*FKW) -> g = SX*FKW/SC
    kw, kwc2 = kq_pack(inputs["k_w"], inputs["k_b"], FKW, SX * FKW / SC)

    # v_w hi/lo: main [128,2,258] x FVW ; c2 [3,2,258] = (h,l) planes
    vwT = np.asarray(inputs["v_w"], f).T
    vb = np.asarray(inputs["v_b"], f)
    v_feat = np.stack([vwT[0:128], vwT[128:256]], axis=1) * FVW
    vwh, vwl = _hilo(v_feat)
    vc2 = np.zeros((3, 2, VF), f)
    c2v = np.concatenate([vwT[256:258], vb[None]], axis=0) * (SX * FVW / SC)
    c2vh = _q8(c2v).astype(f)
    vc2[:, 0, :] = c2vh
    vc2[:, 1, :] = c2v - c2vh
    vwc2 = _q8(vc2)

    # q_w with mod^2 folded, per sample
    incw = np.asarray(inputs["inc_w"], f)
    incb = np.asarray(inputs["inc_b"], f)
    mod = np.maximum.reduce(txt, axis=2) @ incw.T + incb   # [B, 258]
    qwT = np.asarray(inputs["q_w"], f).T                    # [in, out]
    qb_ = np.asarray(inputs["q_b"], f)
    qw_all = np.zeros((B, 128, 2, VF), NP_F8)
    qwc2_all = np.zeros((B, 3, 2, VF), NP_F8)
    for b in range(B):
        m2 = (mod[b] ** 2)[None, :]                         # [1, out]
        qmain = np.stack([qwT[0:128], qwT[128:256]], axis=1) * (FQW * m2[:, None, :])
        qw_all[b] = _q8(qmain)
        qc2 = np.zeros((3, 2, VF), f)
        qc2[0:2, 0, :] = qwT[256:258] * (SX * FQW / SC) * m2
        qc2[2, 0, :] = qb_ * (SX * FQW / SC) * m2[0]
        qwc2_all[b] = _q8(qc2)

    # coords pair tile: rows (x, y, ones) x SC, both planes identical
    xr = np.linspace(-1.0, 1.0, 32, dtype=f)
    yy, xx = np.meshgrid(xr, xr, indexing="ij")
    coord3 = np.stack([xx.ravel(), yy.ravel(), np.ones(HW, f)]).astype(f) * SC
    coordp = _q8(np.stack([coord3, coord3], axis=1).reshape(3, 2 * HW))

    onesp = _q8(np.full((128, 2 * 128), SV, f))

    # text tensors bf16
    txtc = np.zeros((B, 128, 3, 20), f)
    txtc[:, :, 0, :] = txt[:, 0:128]
    txtc[:, :, 1, :] = txt[:, 128:256]
    txtc[:, :44, 2, :] = txt[:, 256:300]
    txtc = txtc.reshape(B, 128, 60).astype(NP_BF)
    txtT = np.ascontiguousarray(txt.transpose(0, 2, 1)).astype(NP_BF)

    # GN selectors + affine columns
    cidx = np.arange(128)
    gsel1 = np.zeros((128, 32), f)
    gsel1[cidx, cidx // 4] = 0.25
    gsel1T = np.zeros((32, 128), f)
    gsel1T[cidx // 4, cidx] = 1.0
    gsel2 = np.zeros((128, 2, 16), f)
    gsel2T = np.zeros((16, 2, 128), f)
    for mb in range(2):
        g = cidx // 8
        gsel2[cidx, mb, g] = 0.125
        gsel2T[g, mb, cidx] = 1.0
    gsel2 = gsel2.reshape(128, 32)
    gsel2T = gsel2T.reshape(16, 256)

    sv1 = np.zeros((128, 2), f)
    sv1[:, 0] = np.asarray(inputs["gn1_g"], f) * S2A
    sv1[:, 1] = np.asarray(inputs["gn1_b"], f) * S2A
    b2 = np.asarray(inputs["conv2_b"], f)
    sv2 = np.zeros((128, 6), f)
    sv2[:, 0:2] = (np.asarray(inputs["gn2_g"], f) * SX).reshape(2, 128).T
    sv2[:, 2:4] = (np.asarray(inputs["gn2_b"], f) * SX).reshape(2, 128).T
    sv2[:, 4:6] = (b2 * S2).reshape(2, 128).T

    shared = {
        "w1v": w1v, "w1s": w1s, "w2": w2,
        "rwh": rwh.reshape(128, 2 * DS), "rwl": rwl.reshape(128, 2 * DS),
        "rwc2": rwc2.reshape(3, 2 * DS),
        "kw": kw.reshape(128, 2 * VF), "kwc2": kwc2.reshape(3, 2 * VF),
        "vwh": vwh.reshape(128, 2 * VF), "vwl": vwl.reshape(128, 2 * VF),
        "vwc2": vwc2.reshape(3, 2 * VF),
        "coordp": coordp, "onesp": onesp,
        "ones20": np.ones((20, 128), NP_BF),
        "gsel1": gsel1, "gsel1T": gsel1T, "gsel2": gsel2, "gsel2T": gsel2T,
        "svec1": sv1, "svec2": sv2,
        "zpad": np.zeros((128, 34 * 34), NP_F8),
    }

    in_maps = []
    for c in range(N_CORES):
        sl = slice(c * SPC, (c + 1) * SPC)
        m = dict(shared)
        m["xv"] = np.ascontiguousarray(xv[sl])
        m["xsp"] = np.ascontiguousarray(xsp[sl])
        m["txtc"] = np.ascontiguousarray(txtc[sl])
        m["txtT"] = np.ascontiguousarray(txtT[sl])
        m["qw"] = np.ascontiguousarray(qw_all[sl].reshape(SPC, 128, 2 * VF))
        m["qwc2"] = np.ascontiguousarray(qwc2_all[sl].reshape(SPC, 3, 2 * VF))
        in_maps.append(m)
    return in_maps


def get_program():
    if "nc" not in _PROGRAM_CACHE:
        _PROGRAM_CACHE["nc"] = build_program()
    return _PROGRAM_CACHE["nc"]


def kernel(**inputs) -> np.ndarray:
    nc = get_program()
    in_maps = _prep_inputs(inputs)
    res = run_bass_kernel_spmd(nc, in_maps, list(range(N_CORES)))
    outs = [res.results[c]["out"].astype(np.float32) for c in range(N_CORES)]
    full = np.concatenate(outs, axis=0).reshape(32, 558, 32, 32)
    return full.astype(np.float32)
